# revision 1
# baseline (speedup 1.0000x reference)
# Trainium2 Bass kernel for nn_DiT_89086211653924 (windowed-attention video DiT).
#
# Sharding (8 cores, zero collectives): core c -> (batch b = c//4, temporal
# group t = c%4) = frames [4t, 4t+4), i.e. 1024 of the 4096 tokens of batch b.
# Both the spatial windows (one frame = 256 contiguous tokens) and the
# spatio-temporal 4x4x4 windows (within one temporal group) are core-local, and
# the action-context cross-attention K/V is a per-batch constant, so each core
# runs the full two-block transformer on its 1024 tokens independently.
#
# On-device layout: activations are feature-major [c_in partition (128), c_tile
# (4), token (1024)] so every linear is matmul(lhsT=W[k_tile, c_out_slice],
# rhs=xT[k_tile, tokens]) with no transposes anywhere.  Value projections emit
# token-major V by using the activation tile as the stationary operand instead.
# LayerNorm gamma/beta are folded into the following matmul's weights+bias on
# the host, so on-device LN is just (h - mu) * rsqrt(var+eps) with the
# per-token stats broadcast across partitions by a K=1 ones matmul on the PE.
import sys

sys.path.insert(0, "/opt/trn_rl_repo")

import numpy as np

import concourse.bacc as bacc
import concourse.mybir as mybir
import concourse.tile as tile
from concourse.bass_utils import run_bass_kernel_spmd

F32 = mybir.dt.float32
AF = mybir.ActivationFunctionType
ALU = mybir.AluOpType

B, CIN, F, IMG, P = 2, 3, 16, 64, 4
PD = IMG // P          # 16 patches per side
NP = PD * PD           # 256 patches per frame
C, NH = 512, 8
HD = C // NH           # 64
S = 16                 # action context length
SCALE = float(1.0 / np.sqrt(HD))
M = 1024               # tokens per core (4 frames)
CT = 4                 # c tiles of 128
EPS = 1e-5

_BUILD_CACHE = {}


# ---------------------------------------------------------------- host prep

def _fold_block(p, w):
    """Fold LN gamma/beta into the consuming projections for block prefix p.

    Returns dict of device arrays for this block."""
    g, b = w[p + "_ln_g"], w[p + "_ln_b"]          # [3, C]
    qkv1, wo1, bo1 = w[p + "_qkv1"], w[p + "_wo1"], w[p + "_bo1"]
    qkv2, wo2, bo2 = w[p + "_qkv2"], w[p + "_wo2"], w[p + "_bo2"]
    f1w, f1b = w[p + "_fc1w"], w[p + "_fc1b"]
    f2w, f2b = w[p + "_fc2w"], w[p + "_fc2b"]
    d = {}
    # self-attn: q (scaled by 1/sqrt(HD)), k, v from LN1(h)
    d["wq1"] = (g[0][:, None] * qkv1[0]) * SCALE
    d["bq1"] = (b[0] @ qkv1[0]) * SCALE
    d["wk1"] = g[0][:, None] * qkv1[1]
    d["bk1"] = b[0] @ qkv1[1]
    d["wv1"] = g[0][:, None] * qkv1[2]
    d["bv1"] = b[0] @ qkv1[2]
    d["wo1"], d["bo1"] = wo1, bo1
    # cross-attn: q from LN2(h) (scaled); k,v from raw ctx (no LN, no bias)
    d["wq2"] = (g[1][:, None] * qkv2[0]) * SCALE
    d["bq2"] = (b[1] @ qkv2[0]) * SCALE
    d["wk2"] = qkv2[1]
    d["wv2"] = qkv2[2]
    d["wo2"], d["bo2"] = wo2, bo2
    # mlp from LN3(h)
    d["wf1"] = g[2][:, None] * f1w
    d["bf1"] = b[2] @ f1w + f1b
    d["wf2"], d["bf2"] = f2w, f2b
    return d


def _w4(a):
    """[512, N] k-major weight -> [4, 128, N] (k_tile, k_in_tile, N)."""
    return np.ascontiguousarray(a.reshape(4, 128, -1).astype(np.float32))


def _colscal(a):
    """[512] bias -> [128, 4] per-partition scalar layout (col = c_tile)."""
    return np.ascontiguousarray(a.reshape(4, 128).T.astype(np.float32))


def _mprime_index():
    """m' (block-g window-major order) -> m (frame-major order)."""
    a, bb, f, i, j = np.meshgrid(
        np.arange(4), np.arange(4), np.arange(4), np.arange(4), np.arange(4),
        indexing="ij",
    )
    return (f * 256 + (4 * a + i) * 16 + 4 * bb + j).reshape(-1)


def host_prep(inputs):
    """Full inputs -> (shared weight map, list of 8 per-core input maps)."""
    w = {k: np.asarray(v) for k, v in inputs.items()}
    x = w["x"].astype(np.float32)                  # (B, CIN, F, IMG, IMG)
    actions = np.asarray(w["actions"])             # (B, S) int

    shared = {}
    shared["wp"] = np.ascontiguousarray(
        w["conv_w"].reshape(C, CIN * P * P).T.astype(np.float32))   # [48, 512]
    shared["convb"] = _colscal(w["conv_b"])
    shared["sppos"] = np.ascontiguousarray(
        w["sp_pos"].T.reshape(4, 128, NP).astype(np.float32))       # [4,128,256]
    shared["wh"] = _w4(w["head_w"])                                 # [4,128,48]
    shared["bh"] = np.ascontiguousarray(
        w["head_b"].reshape(1, 48).astype(np.float32))
    for p in ("s", "g"):
        fb = _fold_block(p, w)
        for nm in ("wq1", "wk1", "wv1", "wo1", "wq2", "wk2", "wv2", "wo2",
                   "wf1", "wf2"):
            shared[p + nm] = _w4(fb[nm])
        for nm in ("bq1", "bk1", "bo1", "bq2", "bo2", "bf1", "bf2"):
            shared[p + nm] = _colscal(fb[nm])
        # v biases enter via a K=1 matmul row (token-major output)
        shared[p + "bv1r"] = np.ascontiguousarray(
            fb["bv1"].reshape(1, C).astype(np.float32))

    per_core = []
    for c in range(8):
        b, t = c // 4, c % 4
        m = dict(shared)
        xs = x[b, :, 4 * t:4 * t + 4]              # (CIN, 4, IMG, IMG)
        xs = xs.reshape(CIN, 4, PD, P, PD, P)
        xs = xs.transpose(0, 3, 5, 1, 2, 4)        # (c, p, q, f, ph, pw)
        m["xpt"] = np.ascontiguousarray(
            xs.reshape(CIN * P * P, M).astype(np.float32))          # [48, 1024]
        m["tpos"] = np.ascontiguousarray(
            w["t_pos"][4 * t:4 * t + 4].T.reshape(4, 128, 4)
            .astype(np.float32))                                    # [4,128,4]
        ctx = (w["act_table"][actions[b]] + w["act_pos"][0]).astype(np.float32)
        m["ctxt"] = np.ascontiguousarray(ctx.T.reshape(4, 128, S))  # [4,128,16]
        per_core.append(m)
    return per_core


# ---------------------------------------------------------------- device build

class Dev:
    """Holds nc handles used across the builder functions."""


def _declare_inputs(nc):
    d = Dev()
    d.nc = nc
    mk = lambda name, shape: nc.dram_tensor(name, shape, F32,
                                            kind="ExternalInput")
    d.xpt = mk("xpt", [48, M])
    d.tpos = mk("tpos", [4, 128, 4])
    d.sppos = mk("sppos", [4, 128, NP])
    d.ctxt = mk("ctxt", [4, 128, S])
    d.wp = mk("wp", [48, C])
    d.convb = mk("convb", [128, 4])
    d.wh = mk("wh", [4, 128, 48])
    d.bh = mk("bh", [1, 48])
    for p in ("s", "g"):
        for nm in ("wq1", "wk1", "wv1", "wo1", "wq2", "wk2", "wv2", "wo2",
                   "wf1", "wf2"):
            setattr(d, p + nm, mk(p + nm, [4, 128, C]))
        for nm in ("bq1", "bk1", "bo1", "bq2", "bo2", "bf1", "bf2"):
            setattr(d, p + nm, mk(p + nm, [128, 4]))
        setattr(d, p + "bv1r", mk(p + "bv1r", [1, C]))
    d.out = nc.dram_tensor("out", [48, M], F32, kind="ExternalOutput")
    return d


def _load_w(d, pools, dram, name):
    """DMA a [4,128,N] weight into sbuf [128, 4, N]."""
    nc = d.nc
    n = dram.shape[2]
    t = pools.wpool.tile([128, 4, n], F32, name=name, tag="w" + str(n))
    nc.sync.dma_start(out=t, in_=dram.ap().transpose([1, 0, 2]))
    return t


def _load_small(d, pools, dram, name):
    nc = d.nc
    shape = list(dram.shape)
    if len(shape) == 3:
        # [4, 128, N] dram -> [128, 4, N] sbuf (partition-major)
        t = pools.cpool.tile([shape[1], shape[0], shape[2]], F32, name=name)
        nc.sync.dma_start(out=t, in_=dram.ap().transpose([1, 0, 2]))
    else:
        t = pools.cpool.tile(shape, F32, name=name)
        nc.sync.dma_start(out=t, in_=dram.ap())
    return t


def _linear_fm(d, pools, w_sb, x_sb, epilogue):
    """Feature-major linear: for each (ct, ms) produce psum [128,512] =
    sum_kt w_sb[:,kt,ct*128:+128].T @ x_sb[:,kt,ms*512:+512], then call
    epilogue(ct, ms, psum)."""
    nc = d.nc
    for ct in range(CT):
        for ms in range(2):
            ps = pools.ps.tile([128, 512], F32, name="lin_ps", tag="ps")
            for kt in range(CT):
                nc.tensor.matmul(
                    ps,
                    w_sb[:, kt, ct * 128:(ct + 1) * 128],
                    x_sb[:, kt, ms * 512:(ms + 1) * 512],
                    start=(kt == 0), stop=(kt == 3),
                )
            epilogue(ct, ms, ps)


def _proj_act(d, pools, w_sb, b_sb, x_sb, out_sb):
    """Linear + per-c_out bias via ACT Identity, into feature-major out."""
    nc = d.nc

    def ep(ct, ms, ps):
        nc.scalar.activation(
            out=out_sb[:, ct, ms * 512:(ms + 1) * 512], in_=ps,
            func=AF.Identity, bias=b_sb[:, ct:ct + 1], scale=1.0)

    _linear_fm(d, pools, w_sb, x_sb, ep)


def _proj_residual(d, pools, w_sb, b_sb, x_sb, h_sb):
    """h += x @ W + b  (one fused DVE op per tile)."""
    nc = d.nc

    def ep(ct, ms, ps):
        sl = h_sb[:, ct, ms * 512:(ms + 1) * 512]
        nc.vector.scalar_tensor_tensor(
            out=sl, in0=ps, scalar=b_sb[:, ct:ct + 1], in1=sl,
            op0=ALU.add, op1=ALU.add)

    _linear_fm(d, pools, w_sb, x_sb, ep)


def _layernorm(d, pools, h_sb, z_sb, tag):
    """z = (h - mean_c(h)) * rsqrt(var_c(h) + eps), feature-major."""
    nc = d.nc
    sq = pools.state.tile([128, CT, M], F32, name="lnsq", tag="scratch16")
    nc.scalar.activation(out=sq, in_=h_sb, func=AF.Square)
    if _STAGE == 201:
        nc.vector.tensor_copy(out=z_sb, in_=sq)
        return
    rows = pools.rows.tile([1, 4, M], F32, name="lnrows_" + tag,
                           tag="lnrows", bufs=1)
    MU, A, BR, PSI = range(4)           # A ends as r, BR is scratch
    for ms in range(2):
        msl = slice(ms * 512, (ms + 1) * 512)
        for which, src in ((0, h_sb), (1, sq)):
            ps = pools.ps.tile([128, 512], F32, name="ln_ps", tag="ps")
            for kt in range(CT):
                nc.tensor.matmul(ps[0:1, :], d.ones_col,
                                 src[:, kt, msl],
                                 start=(kt == 0), stop=(kt == 3))
            dst = MU if which == 0 else A
            nc.scalar.activation(out=rows[:, dst, msl], in_=ps[0:1, :],
                                 func=AF.Copy, scale=1.0 / C)
        nc.vector.tensor_mul(rows[:, BR, msl], rows[:, MU, msl],
                             rows[:, MU, msl])
        nc.vector.tensor_sub(rows[:, A, msl], rows[:, A, msl],
                             rows[:, BR, msl])
        nc.scalar.activation(out=rows[:, BR, msl], in_=rows[:, A, msl],
                             func=AF.Sqrt, bias=d.eps_row)
        nc.vector.reciprocal(rows[:, A, msl], rows[:, BR, msl])
        nc.vector.scalar_tensor_tensor(
            out=rows[:, PSI, msl], in0=rows[:, MU, msl], scalar=-1.0,
            in1=rows[:, A, msl], op0=ALU.mult, op1=ALU.mult)
    if _STAGE == 202:
        nc.vector.tensor_copy(out=z_sb, in_=h_sb)
        return
    # broadcast r and psi across partitions via K=1 ones matmul
    rb = pools.state.tile([128, M], F32, name="ln_rb", tag="bc_r")
    psib = pools.state.tile([128, M], F32, name="ln_psib", tag="bc_p")
    for ms in range(2):
        msl = slice(ms * 512, (ms + 1) * 512)
        for src, dst in ((A, rb), (PSI, psib)):
            ps = pools.ps.tile([128, 512], F32, name="bc_ps", tag="ps")
            nc.tensor.matmul(ps, d.ones_row[:, 0:128], rows[:, src, msl],
                             start=True, stop=True)
            nc.scalar.copy(out=dst[:, msl], in_=ps)
    if _STAGE == 203:
        nc.vector.tensor_copy(out=z_sb, in_=h_sb)
        return
    # z = h * rb + psib   (rb/psib broadcast over the 4 c-tiles)
    rbv = rb.unsqueeze(1).broadcast_to([128, CT, M])
    psv = psib.unsqueeze(1).broadcast_to([128, CT, M])
    nc.vector.tensor_mul(z_sb, h_sb, rbv)
    nc.vector.tensor_add(z_sb, z_sb, psv)


def _v_proj_tokmajor(d, pools, w_sb, brow_sb, x_sb, v_sb):
    """Token-major value projection: v_sb[128 tok, mt, C] with bias via an
    extra K=1 matmul against the bias row (brow_sb may be None)."""
    nc = d.nc
    for mt in range(8):
        ps = pools.ps.tile([128, 512], F32, name="v_ps", tag="ps")
        for kt in range(CT):
            nc.tensor.matmul(
                ps, x_sb[:, kt, mt * 128:(mt + 1) * 128],
                w_sb[:, kt, :], start=(kt == 0),
                stop=(kt == 3 and brow_sb is None))
        if brow_sb is not None:
            nc.tensor.matmul(ps, d.ones_row[:, 0:128], brow_sb,
                             start=False, stop=True)
        nc.scalar.copy(out=v_sb[:, mt, :], in_=ps)


def _self_attn_s(d, pools, z_sb, attn_sb):
    """Block-s self attention: 4 windows (frames) x 256 tokens, 8 heads."""
    nc = d.nc
    v_sb = pools.state.tile([128, 8, C], F32, name="v_s", tag="vbuf")
    _v_proj_tokmajor(d, pools, _load_w(d, pools, d.swv1, "wv1"),
                 d.sbv1r_sb, z_sb, v_sb)
    for w in range(4):
        for h in range(NH):
            hp, ct = 64 * (h % 2), h // 2
            krows = slice(hp, hp + 64)
            sc = pools.ps.tile([128, 2, 256], F32, name="sc", tag="ps")
            for st in range(2):
                nc.tensor.matmul(
                    sc[:, st, :],
                    d.skt_sb[krows, ct, w * 256 + st * 128:
                             w * 256 + st * 128 + 128],
                    d.sqt_sb[krows, ct, w * 256:w * 256 + 256],
                    start=True, stop=True)
            ex = pools.epool.tile([128, 2, 256], F32, name="ex", tag="ex")
            nc.scalar.activation(out=ex, in_=sc, func=AF.Exp)
            dn = pools.ps.tile([128, 512], F32, name="dn", tag="ps")
            ob = pools.ps.tile([128, 512], F32, name="ob", tag="ps")
            for st in range(2):
                nc.tensor.matmul(dn[0:1, 0:256], d.ones_col, ex[:, st, :],
                                 start=(st == 0), stop=(st == 1))
                nc.tensor.matmul(
                    ob[hp:hp + 64, 0:256],
                    v_sb[:, 2 * w + st, h * 64:h * 64 + 64],
                    ex[:, st, :], start=(st == 0), stop=(st == 1),
                    tile_position=(0, hp))
            rrow = pools.rows.tile([1, 512], F32, name="rrow", tag="rr")
            nc.vector.reciprocal(rrow[:, 0:256], dn[0:1, 0:256])
            rb = pools.ps.tile([128, 512], F32, name="rb", tag="ps")
            nc.tensor.matmul(rb[:, 0:256], d.ones_row[:, 0:128], rrow[:, 0:256],
                             start=True, stop=True)
            rbs = pools.epool.tile([128, 2, 256], F32, name="rbs", tag="rbs")
            nc.scalar.copy(out=rbs[:, 0, :], in_=rb[:, 0:256])
            nc.vector.tensor_mul(
                attn_sb[krows, ct, w * 256:w * 256 + 256],
                ob[hp:hp + 64, 0:256], rbs[hp:hp + 64, 0, :])


def _self_attn_g(d, pools, z_sb, attn_sb):
    """Block-g self attention: 16 windows x 64 tokens (m' order), 8 heads,
    processed as 8 window-pairs.

    PSUM hazard rule (HW-verified): within one bank, concurrent matmuls with
    different row groups but overlapping column strips fault.  So scores are
    banked by head parity (source rows fixed per bank) and o/denominator by
    window parity (rows = 64j fixed per bank)."""
    nc = d.nc
    v_sb = pools.state.tile([128, 8, C], F32, name="v_g", tag="vbuf")
    _v_proj_tokmajor(d, pools, _load_w(d, pools, d.gwv1, "wv1"),
                 d.gbv1r_sb, z_sb, v_sb)
    if _STAGE == 33:
        nc.vector.memset(attn_sb, 0.0)
        return
    if 34 <= _STAGE <= 37:
        nc.vector.memset(attn_sb, 0.0)
    for wp in range(8):
        # scores: bank per head-parity p; window parity j selects out rows
        scs = [pools.ps.tile([128, 4, 64], F32, name=f"sc_g{p}", tag="ps")
               for p in range(2)]
        for h in range(NH):
            p, ct = h % 2, h // 2
            hp = 64 * p
            for j in range(2):
                w = 2 * wp + j
                nc.tensor.matmul(
                    scs[p][64 * j:64 * j + 64, ct, :],
                    d.gkt_sb[hp:hp + 64, ct, w * 64:w * 64 + 64],
                    d.gqt_sb[hp:hp + 64, ct, w * 64:w * 64 + 64],
                    start=True, stop=True, tile_position=(hp, 64 * j))
        if _STAGE == 34:
            continue
        exs = []
        for p in range(2):
            ex = pools.epool.tile([128, 4, 64], F32, name=f"ex_g{p}",
                                  tag="ex")
            nc.scalar.activation(out=ex, in_=scs[p], func=AF.Exp)
            exs.append(ex)
        if _STAGE == 35:
            continue
        # o and denominators: bank per window parity j (rows 64j fixed)
        ogs = [pools.ps.tile([128, 4, 64], F32, name=f"og{j}", tag="ps")
               for j in range(2)]
        dns = [pools.ps.tile([128, 512], F32, name=f"dn_g{j}", tag="ps")
               for j in range(2)]
        for h in range(NH):
            p, ct = h % 2, h // 2
            for j in range(2):
                jr = slice(64 * j, 64 * j + 64)
                nc.tensor.matmul(
                    dns[j][0:1, h * 64:h * 64 + 64],
                    d.ones_col[jr, :], exs[p][jr, ct, :],
                    start=True, stop=True, tile_position=(64 * j, 0))
                nc.tensor.matmul(
                    ogs[j][64 * p:64 * p + 64, ct, :],
                    v_sb[jr, wp, h * 64:h * 64 + 64],
                    exs[p][jr, ct, :],
                    start=True, stop=True, tile_position=(64 * j, 64 * p))
        if _STAGE == 36:
            continue
        for j in range(2):
            rrow = pools.rows.tile([1, 512], F32, name=f"rr_g{j}", tag="rr")
            nc.vector.reciprocal(rrow, dns[j][0:1, :])
            rb = pools.ps.tile([128, 512], F32, name=f"rb_g{j}", tag="ps")
            nc.tensor.matmul(rb, d.ones_row[:, 0:128], rrow,
                             start=True, stop=True)
            rbs = pools.epool.tile([128, 512], F32, name=f"rbs_g{j}",
                                   tag="rbs")
            nc.scalar.copy(out=rbs, in_=rb)
            if _STAGE == 37:
                continue
            for p in range(2):
                pr = slice(64 * p, 64 * p + 64)
                out_view = attn_sb[pr, :, (2 * wp + j) * 64:
                                   (2 * wp + j) * 64 + 64]
                in1 = rbs[pr, :].rearrange(
                    "q (ct two l) -> q ct two l", two=2, l=64)[:, :, p, :]
                nc.vector.tensor_mul(out_view, ogs[j][pr, :, :], in1)


def _cross_attn(d, pools, prefix, z_sb, attn_sb):
    """Cross attention to the 16 action-context tokens (shared by all
    windows).  Heads packed 4-per-PSUM-bank at 32-partition strips."""
    nc = d.nc
    wk = _load_w(d, pools, getattr(d, prefix + "wk2"), "wk2")
    wv = _load_w(d, pools, getattr(d, prefix + "wv2"), "wv2")
    qt = getattr(d, prefix + "qt2_sb")
    # K/V from ctx (tiny): kT [128, 4, 16] feature-major; V [16, C] tok-major
    ktc = pools.state.tile([128, 4, S], F32, name=prefix + "ktc", tag="ktc")
    for ct in range(CT):
        ps = pools.ps.tile([128, 512], F32, name="ktc_ps", tag="ps")
        for kt in range(CT):
            nc.tensor.matmul(ps[:, 0:S], wk[:, kt, ct * 128:(ct + 1) * 128],
                             d.ctxt_sb[:, kt, :], start=(kt == 0),
                             stop=(kt == 3))
        nc.scalar.copy(out=ktc[:, ct, :], in_=ps[:, 0:S])
    vc = pools.state.tile([128, 8, 64], F32, name=prefix + "vc", tag="vc")
    ps = pools.ps.tile([128, 512], F32, name="vc_ps", tag="ps")
    for kt in range(CT):
        nc.tensor.matmul(ps[0:S, :], d.ctxt_sb[:, kt, :], wv[:, kt, :],
                         start=(kt == 0), stop=(kt == 3))
    vcv = vc.rearrange("p h d -> p (h d)")
    nc.scalar.copy(out=vcv[0:S, :], in_=ps[0:S, :])
    for q in range(1, 4):
        nc.sync.dma_start(out=vcv[32 * q:32 * q + S, :], in_=vcv[0:S, :])
    for ms in range(2):
        msl = slice(ms * 512, (ms + 1) * 512)
        exs = []
        for grp in range(2):                       # heads 4*grp .. 4*grp+3
            sc = pools.ps.tile([128, 512], F32, name="sc_x", tag="ps")
            nc.vector.memset(sc, 0.0)
            for hh in range(4):
                h = 4 * grp + hh
                hp, ct = 64 * (h % 2), h // 2
                nc.tensor.matmul(
                    sc[32 * hh:32 * hh + S, :],
                    ktc[64 * (h % 2):64 * (h % 2) + 64, ct, :],
                    qt[64 * (h % 2):64 * (h % 2) + 64, ct, msl],
                    start=True, stop=True,
                    tile_position=(hp, 32 * hh))
            ex = pools.epool.tile([128, 512], F32, name="ex_x", tag="ex")
            nc.scalar.activation(out=ex, in_=sc, func=AF.Exp)
            exs.append(ex)
        for h in range(NH):
            grp, hh = h // 4, h % 4
            sr = slice(32 * hh, 32 * hh + S)
            hp, ct = 64 * (h % 2), h // 2
            dn = pools.ps.tile([128, 512], F32, name="dn_x", tag="ps")
            nc.tensor.matmul(dn[0:1, :], d.ones_col[sr, :], exs[grp][sr, :],
                             start=True, stop=True,
                             tile_position=(32 * hh, 0))
            ob = pools.ps.tile([128, 512], F32, name="ob_x", tag="ps")
            nc.tensor.matmul(ob[hp:hp + 64, :], vc[sr, h, :],
                             exs[grp][sr, :], start=True, stop=True,
                             tile_position=(32 * hh, hp))
            rrow = pools.rows.tile([1, 512], F32, name="rr_x", tag="rr")
            nc.vector.reciprocal(rrow, dn[0:1, :])
            rb = pools.ps.tile([128, 512], F32, name="rb_x", tag="ps")
            nc.tensor.matmul(rb, d.ones_row[:, 0:128], rrow, start=True, stop=True)
            rbs = pools.epool.tile([128, 512], F32, name="rbs_x", tag="rbs")
            nc.scalar.copy(out=rbs, in_=rb)
            nc.vector.tensor_mul(attn_sb[64 * (h % 2):64 * (h % 2) + 64,
                                         ct, msl],
                                 ob[hp:hp + 64, :], rbs[hp:hp + 64, :])


def _block(d, pools, prefix, h_sb):
    nc = d.nc
    z = pools.state.tile([128, CT, M], F32, name="z", tag="zbuf")
    attn = pools.state.tile([128, CT, M], F32, name="attn", tag="attnbuf")
    qt = pools.state.tile([128, CT, M], F32, name=prefix + "qt", tag="qbuf")
    kt_ = pools.state.tile([128, CT, M], F32, name=prefix + "kt", tag="kbuf")
    setattr(d, prefix + "qt_sb", qt)
    setattr(d, prefix + "kt_sb", kt_)

    # ---- self attention
    _layernorm(d, pools, h_sb, z, prefix + "1")
    if prefix == "s" and _STAGE == 20:
        return _dbg_out(d, pools, z[0:48, 0, :])
    if prefix == "g" and _STAGE == 30:
        return _dbg_out(d, pools, z[0:48, 0, :])
    _proj_act(d, pools, _load_w(d, pools, getattr(d, prefix + "wq1"), "wq1"),
              getattr(d, prefix + "bq1_sb"), z, qt)
    _proj_act(d, pools, _load_w(d, pools, getattr(d, prefix + "wk1"), "wk1"),
              getattr(d, prefix + "bk1_sb"), z, kt_)
    if prefix == "g" and _STAGE == 31:
        return _dbg_out(d, pools, qt[0:48, 0, :])
    if prefix == "s" and _STAGE == 21:
        return _dbg_out(d, pools, qt[0:48, 0, :])
    if prefix == "s":
        _self_attn_s(d, pools, z, attn)
    else:
        _self_attn_g(d, pools, z, attn)
    if prefix == "g" and _STAGE == 32:
        return _dbg_out(d, pools, attn[0:48, 0, :])
    if prefix == "s" and _STAGE == 22:
        return _dbg_out(d, pools, attn[0:48, 0, :])
    _proj_residual(d, pools, _load_w(d, pools, getattr(d, prefix + "wo1"),
                                     "wo1"),
                   getattr(d, prefix + "bo1_sb"), attn, h_sb)

    # ---- cross attention
    if prefix == "s" and _STAGE == 23:
        return _dbg_out(d, pools, h_sb[0:48, 0, :])
    _layernorm(d, pools, h_sb, z, prefix + "2")
    qt2 = pools.state.tile([128, CT, M], F32, name=prefix + "qt2", tag="kbuf")
    setattr(d, prefix + "qt2_sb", qt2)
    _proj_act(d, pools, _load_w(d, pools, getattr(d, prefix + "wq2"), "wq2"),
              getattr(d, prefix + "bq2_sb"), z, qt2)
    _cross_attn(d, pools, prefix, z, attn)
    _proj_residual(d, pools, _load_w(d, pools, getattr(d, prefix + "wo2"),
                                     "wo2"),
                   getattr(d, prefix + "bo2_sb"), attn, h_sb)
    if prefix == "s" and _STAGE == 24:
        return _dbg_out(d, pools, h_sb[0:48, 0, :])

    # ---- mlp
    _layernorm(d, pools, h_sb, z, prefix + "3")
    gbuf = pools.state.tile([128, CT, M], F32, name=prefix + "g",
                            tag="scratch16")

    def ep_gelu(ct, ms, ps):
        nc.scalar.activation(
            out=gbuf[:, ct, ms * 512:(ms + 1) * 512], in_=ps, func=AF.Gelu,
            bias=getattr(d, prefix + "bf1_sb")[:, ct:ct + 1], scale=1.0)

    _linear_fm(d, pools, _load_w(d, pools, getattr(d, prefix + "wf1"),
                                 "wf1"), z, ep_gelu)
    _proj_residual(d, pools, _load_w(d, pools, getattr(d, prefix + "wf2"),
                                     "wf2"),
                   getattr(d, prefix + "bf2_sb"), gbuf, h_sb)


class Pools:
    pass


import os as _os
_STAGE = int(_os.environ.get("DIT_STAGE", "99"))


def _dbg_out(d, pools, t_sb):
    """Debug: write [48, 1024]-shaped slice of a state tile to out and stop."""
    nc = d.nc
    outT = pools.cpool.tile([48, M], F32, name="outT")
    nc.vector.tensor_copy(out=outT, in_=t_sb)
    nc.sync.dma_start(out=d.out.ap(), in_=outT)


def _body(d, pools):
    """One full forward pass for this core's 1024 tokens."""
    nc = d.nc

    # constants and small tensors first (the weight DMAs are emitted lazily
    # at their use sites so the shared DMA queue can never head-of-line block
    # a load that an earlier weight's consumer depends on)
    xpt = pools.cpool.tile([48, M], F32, name="xpt_sb")
    nc.sync.dma_start(out=xpt, in_=d.xpt.ap())
    wp = pools.cpool.tile([48, C], F32, name="wp_sb")
    nc.sync.dma_start(out=wp, in_=d.wp.ap())
    d.convb_sb = _load_small(d, pools, d.convb, "convb")
    d.ctxt_sb = _load_small(d, pools, d.ctxt, "ctxt")
    tpos = _load_small(d, pools, d.tpos, "tpos")
    sppos = _load_small(d, pools, d.sppos, "sppos")
    for p in ("s", "g"):
        for nm in ("bq1", "bk1", "bo1", "bq2", "bo2", "bf1", "bf2"):
            setattr(d, p + nm + "_sb",
                    _load_small(d, pools, getattr(d, p + nm), p + nm))
        setattr(d, p + "bv1r_sb",
                _load_small(d, pools, getattr(d, p + "bv1r"), p + "bv1r"))
    d.bh_sb = _load_small(d, pools, d.bh, "bh")

    h = pools.state.tile([128, CT, M], F32, name="h", tag="hbuf")

    # ---- patch embedding + positional embeddings
    for ct in range(CT):
        for ms in range(2):
            msl = slice(ms * 512, (ms + 1) * 512)
            ps = pools.ps.tile([128, 512], F32, name="pe_ps", tag="ps")
            nc.tensor.matmul(ps, wp[:, ct * 128:(ct + 1) * 128],
                             xpt[:, msl], start=True, stop=True)
            # + conv_b (per-partition) + t_pos (per frame: 2 frames per ms)
            tp = tpos[:, ct, 2 * ms:2 * ms + 2]
            tpv = tp.unsqueeze(2).broadcast_to([128, 2, 256])
            nc.vector.scalar_tensor_tensor(
                out=h[:, ct, msl].rearrange("p (f l) -> p f l", f=2),
                in0=ps.rearrange("p (f l) -> p f l", f=2),
                scalar=d.convb_sb[:, ct:ct + 1], in1=tpv,
                op0=ALU.add, op1=ALU.add)
        # + sp_pos (same 256 values for each of the 4 frames)
        spv = sppos[:, ct, :].unsqueeze(1).broadcast_to([128, 4, NP])
        hv = h[:, ct, :].rearrange("p (f l) -> p f l", f=4)
        nc.vector.tensor_add(hv, hv, spv)

    if _STAGE <= 1:
        return _dbg_out(d, pools, h[0:48, 0, :])

    # ---- block s (spatial windows = frames)
    _block(d, pools, "s", h)
    if _STAGE <= 2:
        return _dbg_out(d, pools, h[0:48, 0, :])

    # ---- permute tokens to m' (window-major for the 4x4x4 windows)
    hg = pools.state.tile([128, CT, M], F32, name="hg", tag="zbuf2")
    for ct in range(CT):
        for a in range(4):
            src = bass_view_perm(h, ct, a)
            nc.vector.tensor_copy(
                out=hg[:, ct, a * 256:(a + 1) * 256].rearrange(
                    "p (b f i j) -> p b f i j", b=4, f=4, i=4),
                in_=src)
    if _STAGE <= 3:
        return _dbg_out(d, pools, hg[0:48, 0, :])
    _block(d, pools, "g", hg)
    if _STAGE <= 4:
        return _dbg_out(d, pools, hg[0:48, 0, :])

    # ---- output head (m' order; host undoes the permutation)
    d.wh_sb = _load_w(d, pools, d.wh, "wh")
    outT = pools.cpool.tile([48, M], F32, name="outT")
    for ms in range(2):
        msl = slice(ms * 512, (ms + 1) * 512)
        ps = pools.ps.tile([128, 512], F32, name="hd_ps", tag="ps")
        for kt in range(CT):
            nc.tensor.matmul(ps[0:48, :], d.wh_sb[:, kt, :], hg[:, kt, msl],
                             start=(kt == 0), stop=False)
        nc.tensor.matmul(ps[0:48, :], d.bh_sb, d.ones_row,
                         start=False, stop=True)
        nc.scalar.copy(out=outT[:, msl], in_=ps[0:48, :])
    nc.sync.dma_start(out=d.out.ap(), in_=outT)


def bass_view_perm(h, ct, a):
    """View of h[:, ct, :] selecting m' block a*256..a*256+255 in (b,f,i,j)
    nested order: m = f*256 + (4a+i)*16 + 4b + j."""
    base = h[:, ct, :]
    return bass_ap_nest(base, a)


def bass_ap_nest(base, a):
    import concourse.bass as bass  # noqa
    v = base.rearrange("p (f ph pw) -> p f ph pw", f=4, ph=16)
    # dims: f(stride 256), ph=4a+i (stride 16), pw=4b+j (stride 1)
    v2 = v[:, :, 4 * a:4 * a + 4, :].rearrange(
        "p f i (b j) -> p b f i j", b=4)
    return v2


def _build_nc(loop_n=1):
    key = loop_n
    if key in _BUILD_CACHE:
        return _BUILD_CACHE[key]
    nc = bacc.Bacc("TRN2", target_bir_lowering=False, debug=False,
                   num_devices=8)
    d = _declare_inputs(nc)
    with tile.TileContext(nc) as tc:
        pools = Pools()
        import contextlib
        stack = contextlib.ExitStack()
        with stack:
            pools.ps = stack.enter_context(
                tc.tile_pool(name="ps", bufs=6, space="PSUM"))
            pools.wpool = stack.enter_context(
                tc.tile_pool(name="wpool", bufs=2))
            pools.cpool = stack.enter_context(
                tc.tile_pool(name="cpool", bufs=1))
            pools.state = stack.enter_context(
                tc.tile_pool(name="state", bufs=1))
            pools.epool = stack.enter_context(
                tc.tile_pool(name="epool", bufs=2))
            pools.rows = stack.enter_context(
                tc.tile_pool(name="rows", bufs=2))
            ones_col = pools.cpool.tile([128, 1], F32, name="ones_col")
            nc.vector.memset(ones_col, 1.0)
            ones_row = pools.cpool.tile([1, 512], F32, name="ones_row")
            nc.vector.memset(ones_row, 1.0)
            d.ones_col = ones_col
            d.ones_row = ones_row
            eps_row = pools.cpool.tile([1, 1], F32, name="eps_row")
            nc.vector.memset(eps_row, EPS)
            d.eps_row = eps_row
            # python-level unroll for timing variants (a tc.For_i back
            # edge wedges the device here; the unrolled NEFF is equivalent)
            for _ in range(loop_n):
                _body(d, pools)
    nc.compile()
    _BUILD_CACHE[key] = nc
    return nc


# ---------------------------------------------------------------- entry point

_MPRIME = None


def kernel(**inputs) -> np.ndarray:
    global _MPRIME
    if _MPRIME is None:
        _MPRIME = _mprime_index()
    nc = _build_nc(1)
    in_maps = host_prep(inputs)
    res = run_bass_kernel_spmd(nc, in_maps, core_ids=list(range(8)))
    # reassemble: per-core outT [48, 1024] in m' order -> full output
    out = np.zeros((B, CIN, F, IMG, IMG), np.float32)
    for c in range(8):
        b, t = c // 4, c % 4
        ot = res.results[c]["out"]                 # [48, M] m'-order
        om = np.empty_like(ot)
        om[:, _MPRIME] = ot                        # -> m order
        # om[(p,q,cc), (f,ph,pw)] -> out[b, cc, 4t+f, ph*4+p, pw*4+q]
        om = om.reshape(P, P, CIN, 4, PD, PD)
        om = om.transpose(2, 3, 4, 0, 5, 1)        # (cc, f, ph, p, pw, q)
        out[b, :, 4 * t:4 * t + 4] = om.reshape(CIN, 4, IMG, IMG)
    return out


if __name__ == "__main__":
    import reference
    ins = reference.setup_inputs()
    ins = {k: np.asarray(v) for k, v in ins.items()}
    exp = np.asarray(reference.reference(**ins))
    act = kernel(**ins)
    err = np.abs(act - exp).max() / (np.abs(exp).max() + 1e-12)
    print("Relative error:", err)



# revision 45
# speedup vs baseline: 532.2523x; 532.2523x over previous
# Trainium2 Bass kernel for nn_DiT_89086211653924 (windowed-attention video DiT).
#
# Sharding (8 cores, zero collectives): core c -> (batch b = c//4, temporal
# group t = c%4) = frames [4t, 4t+4), i.e. 1024 of the 4096 tokens of batch b.
# Both the spatial windows (one frame = 256 contiguous tokens) and the
# spatio-temporal 4x4x4 windows (within one temporal group) are core-local, and
# the action-context cross-attention K/V is a per-batch constant, so each core
# runs the full two-block transformer on its 1024 tokens independently.
#
# On-device layout: activations are feature-major [c_in partition (128), c_tile
# (4), token (1024)] so every linear is matmul(lhsT=W[k_tile, c_out_slice],
# rhs=xT[k_tile, tokens]) with no transposes anywhere.  Value projections emit
# token-major V by using the activation tile as the stationary operand instead.
# LayerNorm gamma/beta are folded into the following matmul's weights+bias on
# the host.
#
# Performance notes (all validated against the TimelineSim cost model and HW):
# - All matmul operands are bf16 (weights converted on the host): 1 PE
#   cycle/row at every size vs 4 for fp32, and half the weight DMA.  The
#   residual stream h is bf16 as well; PSUM accumulation stays fp32.
#   Measured HW rel err 7.8e-3 vs the 2e-2 gate.
# - Engines execute their queues in emission order, so emission IS the
#   schedule: attention is emitted as a two-stage software pipeline
#   (scores+exp of unit i before denominators/AV of unit i-lag) so the PE
#   never head-blocks on an Act exp round trip.
# - Softmax and LN partition-broadcasts run on the otherwise-idle GPSIMD
#   engine (partition_broadcast); LN z = h*r + psi reads all-bf16 SBUF
#   operands, hitting the DVE 2x/4x 16-bit modes.
# - Cross-attention K/V depend only on the 16 action tokens and are emitted
#   in PE gaps at the self-attention tail.
# - PSUM pool depth 8 (all banks), weight pool depth 5, with emission order
#   arranged so pool-rotation WAR edges always point forward (no deadlock).
import sys

sys.path.insert(0, "/opt/trn_rl_repo")

import numpy as np

import concourse.bacc as bacc
import concourse.mybir as mybir
import concourse.tile as tile
from concourse.bass_utils import run_bass_kernel_spmd

F32 = mybir.dt.float32
F32R = mybir.dt.float32r
BF16 = mybir.dt.bfloat16
AF = mybir.ActivationFunctionType
ALU = mybir.AluOpType


def _mm(nc, out, lhsT, rhs, **kw):
    """Plain matmul; operand dtypes come from the tiles.  Tiles feeding
    matmuls are allocated float32r (1 cycle/row on the PE vs 4 for fp32 when
    the output free size is >= 256); the BIR verifier requires the producing
    instruction to emit f32r, so the rounding happens at the producer."""
    nc.tensor.matmul(out, lhsT, rhs, **kw)

B, CIN, F, IMG, P = 2, 3, 16, 64, 4
PD = IMG // P          # 16 patches per side
NP = PD * PD           # 256 patches per frame
C, NH = 512, 8
HD = C // NH           # 64
S = 16                 # action context length
SCALE = float(1.0 / np.sqrt(HD))
M = 1024               # tokens per core (4 frames)
CT = 4                 # c tiles of 128
EPS = 1e-5

_BUILD_CACHE = {}
_PHASE_MARKS = []


def _mark(d, name):
    """Record (phase, instruction counter) for trace attribution; consumes
    one instruction name (gaps are harmless)."""
    n = int(d.nc.get_next_instruction_name().split("-")[1])
    _PHASE_MARKS.append((name, n))


# ---------------------------------------------------------------- host prep

def _fold_block(p, w):
    """Fold LN gamma/beta into the consuming projections for block prefix p.

    Returns dict of device arrays for this block."""
    g, b = w[p + "_ln_g"], w[p + "_ln_b"]          # [3, C]
    qkv1, wo1, bo1 = w[p + "_qkv1"], w[p + "_wo1"], w[p + "_bo1"]
    qkv2, wo2, bo2 = w[p + "_qkv2"], w[p + "_wo2"], w[p + "_bo2"]
    f1w, f1b = w[p + "_fc1w"], w[p + "_fc1b"]
    f2w, f2b = w[p + "_fc2w"], w[p + "_fc2b"]
    d = {}
    # self-attn: q (scaled by 1/sqrt(HD)), k, v from LN1(h)
    d["wq1"] = (g[0][:, None] * qkv1[0]) * SCALE
    d["bq1"] = (b[0] @ qkv1[0]) * SCALE
    d["wk1"] = g[0][:, None] * qkv1[1]
    d["bk1"] = b[0] @ qkv1[1]
    d["wv1"] = g[0][:, None] * qkv1[2]
    d["bv1"] = b[0] @ qkv1[2]
    d["wo1"], d["bo1"] = wo1, bo1
    # cross-attn: q from LN2(h) (scaled); k,v from raw ctx (no LN, no bias)
    d["wq2"] = (g[1][:, None] * qkv2[0]) * SCALE
    d["bq2"] = (b[1] @ qkv2[0]) * SCALE
    d["wk2"] = qkv2[1]
    d["wv2"] = qkv2[2]
    d["wo2"], d["bo2"] = wo2, bo2
    # mlp from LN3(h)
    d["wf1"] = g[2][:, None] * f1w
    d["bf1"] = b[2] @ f1w + f1b
    d["wf2"], d["bf2"] = f2w, f2b
    return d


def _w4(a):
    """[512, N] k-major weight -> [4, 128, N] (k_tile, k_in_tile, N), bf16."""
    import ml_dtypes
    return np.ascontiguousarray(
        np.asarray(a).reshape(4, 128, -1).astype(ml_dtypes.bfloat16))


def _colscal(a):
    """[512] bias -> [128, 4] per-partition scalar layout (col = c_tile)."""
    return np.ascontiguousarray(a.reshape(4, 128).T.astype(np.float32))


def _mprime_index():
    """m' (block-g window-major order) -> m (frame-major order)."""
    a, bb, f, i, j = np.meshgrid(
        np.arange(4), np.arange(4), np.arange(4), np.arange(4), np.arange(4),
        indexing="ij",
    )
    return (f * 256 + (4 * a + i) * 16 + 4 * bb + j).reshape(-1)


def host_prep(inputs):
    """Full inputs -> (shared weight map, list of 8 per-core input maps)."""
    w = {k: np.asarray(v) for k, v in inputs.items()}
    x = w["x"].astype(np.float32)                  # (B, CIN, F, IMG, IMG)
    actions = np.asarray(w["actions"])             # (B, S) int

    import ml_dtypes
    bf = ml_dtypes.bfloat16
    shared = {}
    shared["wp"] = np.ascontiguousarray(
        w["conv_w"].reshape(C, CIN * P * P).T.astype(bf))           # [48, 512]
    shared["convb"] = _colscal(w["conv_b"])
    shared["sppos"] = np.ascontiguousarray(
        w["sp_pos"].T.reshape(4, 128, NP).astype(np.float32))       # [4,128,256]
    shared["wh"] = _w4(w["head_w"])                                 # [4,128,48]
    shared["bh"] = np.ascontiguousarray(
        w["head_b"].reshape(1, 48).astype(bf))
    for p in ("s", "g"):
        fb = _fold_block(p, w)
        for nm in ("wq1", "wk1", "wv1", "wo1", "wq2", "wk2", "wv2", "wo2",
                   "wf1", "wf2"):
            shared[p + nm] = _w4(fb[nm])
        for nm in ("bq1", "bk1", "bo1", "bq2", "bo2", "bf1", "bf2"):
            shared[p + nm] = _colscal(fb[nm])
        # v biases enter via a K=1 matmul row (token-major output)
        shared[p + "bv1r"] = np.ascontiguousarray(
            fb["bv1"].reshape(1, C).astype(bf))

    per_core = []
    for c in range(8):
        b, t = c // 4, c % 4
        m = dict(shared)
        xs = x[b, :, 4 * t:4 * t + 4]              # (CIN, 4, IMG, IMG)
        xs = xs.reshape(CIN, 4, PD, P, PD, P)
        xs = xs.transpose(0, 3, 5, 1, 2, 4)        # (c, p, q, f, ph, pw)
        m["xpt"] = np.ascontiguousarray(
            xs.reshape(CIN * P * P, M).astype(ml_dtypes.bfloat16))  # [48, 1024]
        m["tpos"] = np.ascontiguousarray(
            w["t_pos"][4 * t:4 * t + 4].T.reshape(4, 128, 4)
            .astype(np.float32))                                    # [4,128,4]
        ctx = (w["act_table"][actions[b]] + w["act_pos"][0]).astype(np.float32)
        m["ctxt"] = np.ascontiguousarray(
            ctx.T.reshape(4, 128, S).astype(ml_dtypes.bfloat16))    # [4,128,16]
        per_core.append(m)
    return per_core


# ---------------------------------------------------------------- device build

class Dev:
    """Holds nc handles used across the builder functions."""


def _declare_inputs(nc):
    d = Dev()
    d.nc = nc
    mk = lambda name, shape, dt=F32: nc.dram_tensor(name, shape, dt,
                                                    kind="ExternalInput")
    d.xpt = mk("xpt", [48, M], BF16)
    d.tpos = mk("tpos", [4, 128, 4])
    d.sppos = mk("sppos", [4, 128, NP])
    d.ctxt = mk("ctxt", [4, 128, S], BF16)
    d.wp = mk("wp", [48, C], BF16)
    d.convb = mk("convb", [128, 4])
    d.wh = mk("wh", [4, 128, 48], BF16)
    d.bh = mk("bh", [1, 48], BF16)
    for p in ("s", "g"):
        for nm in ("wq1", "wk1", "wv1", "wo1", "wq2", "wk2", "wv2", "wo2",
                   "wf1", "wf2"):
            setattr(d, p + nm, mk(p + nm, [4, 128, C], BF16))
        for nm in ("bq1", "bk1", "bo1", "bq2", "bo2", "bf1", "bf2"):
            setattr(d, p + nm, mk(p + nm, [128, 4]))
        setattr(d, p + "bv1r", mk(p + "bv1r", [1, C], BF16))
    d.out = nc.dram_tensor("out", [48, M], F32, kind="ExternalOutput")
    return d


def _load_w(d, pools, dram, name):
    """DMA a [4,128,N] weight into sbuf [128, 4, N]."""
    nc = d.nc
    n = dram.shape[2]
    t = pools.wpool.tile([128, 4, n], dram.dtype, name=name, tag="w" + str(n))
    nc.sync.dma_start(out=t, in_=dram.ap().transpose([1, 0, 2]))
    return t


def _load_small(d, pools, dram, name):
    nc = d.nc
    shape = list(dram.shape)
    if len(shape) == 3:
        # [4, 128, N] dram -> [128, 4, N] sbuf (partition-major)
        t = pools.cpool.tile([shape[1], shape[0], shape[2]], dram.dtype,
                             name=name)
        nc.sync.dma_start(out=t, in_=dram.ap().transpose([1, 0, 2]))
    else:
        t = pools.cpool.tile(shape, dram.dtype, name=name)
        nc.sync.dma_start(out=t, in_=dram.ap())
    return t


def _linear_fm_ms(d, pools, w_sb, x_sb, epilogue, ms):
    """Feature-major linear for one token half ms: for each ct produce psum
    [128,512] = sum_kt w_sb[:,kt,ct*128:+128].T @ x_sb[:,kt,ms*512:+512],
    then call epilogue(ct, ms, psum)."""
    nc = d.nc
    for ct in range(CT):
        ps = pools.ps.tile([128, 512], F32, name="lin_ps", tag="ps")
        for kt in range(CT):
            _mm(nc,
                ps,
                w_sb[:, kt, ct * 128:(ct + 1) * 128],
                x_sb[:, kt, ms * 512:(ms + 1) * 512],
                start=(kt == 0), stop=(kt == 3),
            )
        epilogue(ct, ms, ps)


def _proj_act_ms(d, pools, w_sb, b_sb, x_sb, out_sb, ms):
    """Linear + per-c_out bias via ACT Identity, into feature-major out."""
    nc = d.nc

    def ep(ct, ms_, ps):
        nc.scalar.activation(
            out=out_sb[:, ct, ms_ * 512:(ms_ + 1) * 512], in_=ps,
            func=AF.Identity, bias=b_sb[:, ct:ct + 1], scale=1.0)

    _linear_fm_ms(d, pools, w_sb, x_sb, ep, ms)


def _proj_residual_ms(d, pools, w_sb, b_sb, x_sb, h_sb, ms):
    """h += x @ W + b  (one fused DVE op per tile)."""
    nc = d.nc

    def ep(ct, ms_, ps):
        sl = h_sb[:, ct, ms_ * 512:(ms_ + 1) * 512]
        nc.vector.scalar_tensor_tensor(
            out=sl, in0=ps, scalar=b_sb[:, ct:ct + 1], in1=sl,
            op0=ALU.add, op1=ALU.add)

    _linear_fm_ms(d, pools, w_sb, x_sb, ep, ms)


def _layernorm_ms(d, pools, h_sb, z_sb, tag, ms):
    """LN for one token half; kept for pipelined call sites."""
    _layernorm_emit(d, pools, h_sb, z_sb, tag, (ms,))


def _layernorm(d, pools, h_sb, z_sb, tag):
    _layernorm_emit(d, pools, h_sb, z_sb, tag, (0, 1))


def _layernorm_emit(d, pools, h_sb, z_sb, tag, mss):
    """z = (h - mean_c(h)) * rsqrt(var_c(h) + eps), feature-major.

    h stays fp32 (residual precision), so its column-sum matmul runs fp32;
    the squared-sum runs bf16 off the bf16 Square output.  Emission is
    batched stage-by-stage across the token halves so each engine gets both
    halves' work back-to-back; the broadcast PSUM tiles are consumed
    directly by the z DVE ops (one PSUM operand per DVE op)."""
    nc = d.nc
    if not hasattr(d, "_ln_sq"):
        d._ln_sq = {}
        d._ln_rows = {}
    if tag not in d._ln_sq or mss[0] == 0:
        d._ln_sq[tag] = pools.state.tile([128, CT, M], BF16, name="lnsq",
                                         tag="scratch16")
        d._ln_rows[tag] = pools.rows.tile([1, 4, M], BF16,
                                          name="lnrows_" + tag,
                                          tag="lnrows", bufs=1)
    sq = d._ln_sq[tag]
    rows = d._ln_rows[tag]
    MU, A, R, PSI = range(4)
    sl = lambda ms: slice(ms * 512, (ms + 1) * 512)
    mps = {}
    for ms in mss:
        ps = pools.ps.tile([128, 512], F32, name="ln_mps", tag="ps")
        for kt in range(CT):
            _mm(nc, ps[0:1, :], d.ones_col_b, h_sb[:, kt, sl(ms)],
                start=(kt == 0), stop=(kt == 3))
        mps[ms] = ps
    for ms in mss:
        nc.scalar.activation(out=sq[:, :, sl(ms)], in_=h_sb[:, :, sl(ms)],
                             func=AF.Square)
    vps = {}
    for ms in mss:
        nc.vector.tensor_scalar_mul(rows[:, MU, sl(ms)], mps[ms][0:1, :],
                                    1.0 / C)
        ps = pools.ps.tile([128, 512], F32, name="ln_vps", tag="ps")
        for kt in range(CT):
            _mm(nc, ps[0:1, :], d.ones_col_b, sq[:, kt, sl(ms)],
                start=(kt == 0), stop=(kt == 3))
        vps[ms] = ps
    for ms in mss:
        msl = sl(ms)
        nc.vector.tensor_scalar_mul(rows[:, A, msl], vps[ms][0:1, :], 1.0 / C)
    # var = A - mu^2 ; r = 1/sqrt(var + eps) ; psi = -mu * r
    # (kept per-half: interleaving the halves' row chains with the broadcast
    # matmuls keeps the PE warm through the LN, which matters more than the
    # fixed-cost row ops -- a merged full-M chain leaves a >3us PE gap that
    # resets the p-state ramp for the following projection)
    for msl in [sl(ms) for ms in mss]:
        nc.vector.tensor_mul(rows[:, R, msl], rows[:, MU, msl],
                             rows[:, MU, msl])
        nc.vector.tensor_sub(rows[:, A, msl], rows[:, A, msl],
                             rows[:, R, msl])
        nc.scalar.activation(out=rows[:, A, msl], in_=rows[:, A, msl],
                             func=AF.Sqrt, bias=d.eps_row)
        nc.vector.reciprocal(rows[:, R, msl], rows[:, A, msl])
        nc.vector.scalar_tensor_tensor(
            out=rows[:, PSI, msl], in0=rows[:, MU, msl], scalar=-1.0,
            in1=rows[:, R, msl], op0=ALU.mult, op1=ALU.mult)
    # broadcast r and psi across partitions on the idle GPSIMD engine;
    # all-bf16 SBUF operands put the big z DVE ops in the 2x/4x perf modes
    for ms in mss:
        msl = sl(ms)
        rbs = pools.epool.tile([128, 2, 512], BF16, name="lnb", tag="lnb")
        nc.gpsimd.partition_broadcast(rbs[:, 0, :], rows[:, R, msl])
        nc.gpsimd.partition_broadcast(rbs[:, 1, :], rows[:, PSI, msl])
        zv = z_sb[:, :, msl]
        rv = rbs[:, 0, :].unsqueeze(1).broadcast_to([128, CT, 512])
        pv = rbs[:, 1, :].unsqueeze(1).broadcast_to([128, CT, 512])
        nc.vector.tensor_mul(zv, h_sb[:, :, msl], rv)
        nc.vector.tensor_add(zv, zv, pv)


def _v_proj_ms(d, pools, w_sb, brow_sb, x_sb, v_sb, ms):
    """Token-major value projection for token half ms: v_sb[128 tok, mt, C]
    with bias via an extra K=1 matmul against the bias row."""
    nc = d.nc
    for mt in range(4 * ms, 4 * ms + 4):
        ps = pools.ps.tile([128, 512], F32, name="v_ps", tag="ps")
        for kt in range(CT):
            _mm(nc,
                ps, x_sb[:, kt, mt * 128:(mt + 1) * 128],
                w_sb[:, kt, :], start=(kt == 0),
                stop=(kt == 3 and brow_sb is None))
        if brow_sb is not None:
            _mm(nc, ps, d.ones_row_b[:, 0:128], brow_sb,
                             start=False, stop=True)
        nc.scalar.copy(out=v_sb[:, mt, :], in_=ps)


def _attn_s_stage1(d, pools, w, pair):
    """Scores + exp for heads (2*pair, 2*pair+1) of window w."""
    nc = d.nc
    out = {}
    for ho in range(2):
        h = 2 * pair + ho
        hp, ct = 64 * (h % 2), h // 2
        krows = slice(hp, hp + 64)
        sc = pools.ps.tile([128, 2, 256], F32, name="sc", tag="ps")
        for st in range(2):
            _mm(nc,
                sc[:, st, :],
                d.skt_sb[krows, ct, w * 256 + st * 128:
                         w * 256 + st * 128 + 128],
                d.sqt_sb[krows, ct, w * 256:w * 256 + 256],
                start=True, stop=True)
        ex = pools.epool.tile([128, 2, 256], BF16, name="ex", tag="ex")
        nc.scalar.activation(out=ex, in_=sc, func=AF.Exp)
        out[ho] = ex
    return out


def _attn_s_stage2(d, pools, v_sb, attn_sb, w, pair, exs):
    """dn/ob matmuls, reciprocal, broadcast and normalize for one pair."""
    nc = d.nc
    dn = pools.ps.tile([128, 512], F32, name="dn", tag="ps")
    obs = {}
    for ho in range(2):
        h = 2 * pair + ho
        hp = 64 * (h % 2)
        ob = pools.ps.tile([128, 512], F32, name="ob", tag="ps")
        obs[ho] = ob
        for st in range(2):
            _mm(nc, dn[0:1, 256 * ho:256 * ho + 256], d.ones_col_b,
                exs[ho][:, st, :], start=(st == 0), stop=(st == 1),
                tile_position=(0, 0))
            _mm(nc,
                ob[hp:hp + 64, 0:256],
                v_sb[:, 2 * w + st, h * 64:h * 64 + 64],
                exs[ho][:, st, :], start=(st == 0), stop=(st == 1),
                tile_position=(0, hp))
    rrow = pools.rows.tile([1, 512], F32, name="rrow", tag="rr")
    nc.vector.reciprocal(rrow, dn[0:1, :])
    rbs = pools.epool.tile([128, 2, 256], F32, name="rbs", tag="rbs")
    for ho in range(2):
        h = 2 * pair + ho
        hp, ct = 64 * (h % 2), h // 2
        krows = slice(hp, hp + 64)
        nc.gpsimd.partition_broadcast(rbs[:, ho, :],
                                      rrow[:, 256 * ho:256 * ho + 256])
        nc.vector.tensor_mul(
            attn_sb[krows, ct, w * 256:w * 256 + 256],
            obs[ho][hp:hp + 64, 0:256], rbs[hp:hp + 64, ho, :])


def _attn_s(d, pools, v_sb, attn_sb, lag=1):
    """Block-s self attention, software-pipelined: stage1 (scores+exp) of
    unit i is emitted before stage2 of unit i-lag, so the in-order PE queue
    always has ready score matmuls ahead of denominator matmuls that wait
    on the Act exp round-trip."""
    units = [(w, pair) for w in range(4) for pair in range(4)]
    pend = {}
    for i in range(len(units) + lag):
        if i < len(units):
            w, pair = units[i]
            pend[i] = _attn_s_stage1(d, pools, w, pair)
        j = i - lag
        if j >= 0:
            w, pair = units[j]
            _attn_s_stage2(d, pools, v_sb, attn_sb, w, pair, pend.pop(j))


def _attn_g_stage1(d, pools, wp):
    """Scores + exp for one window-pair of block-g self attention.

    PSUM hazard rule (HW-verified): within one bank, concurrent matmuls with
    different row groups but overlapping column strips fault.  So scores are
    banked by head parity (source rows fixed per bank) and o/denominator by
    window parity (rows = 64j fixed per bank)."""
    nc = d.nc
    scs = [pools.ps.tile([128, 4, 64], F32, name=f"sc_g{p}", tag="ps")
           for p in range(2)]
    for h in range(NH):
        p, ct = h % 2, h // 2
        hp = 64 * p
        for j in range(2):
            w = 2 * wp + j
            _mm(nc,
                scs[p][64 * j:64 * j + 64, ct, :],
                d.gkt_sb[hp:hp + 64, ct, w * 64:w * 64 + 64],
                d.gqt_sb[hp:hp + 64, ct, w * 64:w * 64 + 64],
                start=True, stop=True, tile_position=(hp, 64 * j))
    exs = []
    for p in range(2):
        ex = pools.epool.tile([128, 4, 64], BF16, name=f"ex_g{p}",
                              tag="ex")
        nc.scalar.activation(out=ex, in_=scs[p], func=AF.Exp)
        exs.append(ex)
    return exs


def _attn_g_stage2(d, pools, v_sb, attn_sb, wp, exs):
    nc = d.nc
    ogs = [pools.ps.tile([128, 4, 64], F32, name=f"og{j}", tag="ps")
           for j in range(2)]
    dns = [pools.ps.tile([128, 512], F32, name=f"dn_g{j}", tag="ps")
           for j in range(2)]
    for h in range(NH):
        p, ct = h % 2, h // 2
        for j in range(2):
            jr = slice(64 * j, 64 * j + 64)
            _mm(nc,
                dns[j][0:1, h * 64:h * 64 + 64],
                d.ones_col_b[jr, :], exs[p][jr, ct, :],
                start=True, stop=True, tile_position=(64 * j, 0))
            _mm(nc,
                ogs[j][64 * p:64 * p + 64, ct, :],
                v_sb[jr, wp, h * 64:h * 64 + 64],
                exs[p][jr, ct, :],
                start=True, stop=True, tile_position=(64 * j, 64 * p))
    for j in range(2):
        rrow = pools.rows.tile([1, 512], F32, name=f"rr_g{j}", tag="rr")
        nc.vector.reciprocal(rrow, dns[j][0:1, :])
        rbs = pools.epool.tile([128, 512], F32, name=f"rbs_g{j}",
                               tag="rbs")
        nc.gpsimd.partition_broadcast(rbs, rrow)
        for p in range(2):
            pr = slice(64 * p, 64 * p + 64)
            out_view = attn_sb[pr, :, (2 * wp + j) * 64:
                               (2 * wp + j) * 64 + 64]
            in1 = rbs[pr, :].rearrange(
                "q (ct two l) -> q ct two l", two=2, l=64)[:, :, p, :]
            nc.vector.tensor_mul(out_view, ogs[j][pr, :, :], in1)


def _attn_g(d, pools, v_sb, attn_sb, lag=1):
    pend = {}
    for i in range(8 + lag):
        if i < 8:
            pend[i] = _attn_g_stage1(d, pools, i)
        j = i - lag
        if j >= 0:
            _attn_g_stage2(d, pools, v_sb, attn_sb, j, pend.pop(j))


def _ctx_kv(d, pools, prefix, wk, wv):
    """Cross-attention K/V from the 16 action-context tokens.  Independent
    of the token stream, so emitted during the LN1 row chain to stay off
    the critical path.  kT [128, 4, 16] feature-major; V [16, C] token-
    major, replicated to the four 32-partition strips."""
    nc = d.nc
    ktc = pools.state.tile([128, 4, S], BF16, name=prefix + "ktc",
                           tag=prefix + "ktc")
    for ct in range(CT):
        ps = pools.ps.tile([128, 512], F32, name="ktc_ps", tag="ps")
        for kt in range(CT):
            _mm(nc, ps[:, 0:S], wk[:, kt, ct * 128:(ct + 1) * 128],
                             d.ctxt_sb[:, kt, :], start=(kt == 0),
                             stop=(kt == 3))
        nc.scalar.copy(out=ktc[:, ct, :], in_=ps[:, 0:S])
    vc = pools.state.tile([128, 8, 64], BF16, name=prefix + "vc",
                          tag=prefix + "vc")
    ps = pools.ps.tile([128, 512], F32, name="vc_ps", tag="ps")
    for kt in range(CT):
        _mm(nc, ps[0:S, :], d.ctxt_sb[:, kt, :], wv[:, kt, :],
                         start=(kt == 0), stop=(kt == 3))
    vcv = vc.rearrange("p h d -> p (h d)")
    nc.scalar.copy(out=vcv[0:S, :], in_=ps[0:S, :])
    for q in range(1, 4):
        nc.sync.dma_start(out=vcv[32 * q:32 * q + S, :], in_=vcv[0:S, :])
    setattr(d, prefix + "ktc_sb", ktc)
    setattr(d, prefix + "vc_sb", vc)


def _xattn_ms(d, pools, prefix, attn_sb, ms):
    """Cross attention for token half ms (keys/values precomputed by
    _ctx_kv).  Heads packed 4-per-PSUM-bank at 32-partition strips."""
    nc = d.nc
    qt = getattr(d, prefix + "qt2_sb")
    ktc = getattr(d, prefix + "ktc_sb")
    vc = getattr(d, prefix + "vc_sb")
    msl = slice(ms * 512, (ms + 1) * 512)
    exs = []
    for grp in range(2):                       # heads 4*grp .. 4*grp+3
        # no memset: rows outside the four 16-row strips are garbage but
        # are never read (dn/ob matmuls slice sr exactly)
        sc = pools.ps.tile([128, 512], F32, name="sc_x", tag="ps")
        for hh in range(4):
            h = 4 * grp + hh
            hp, ct = 64 * (h % 2), h // 2
            _mm(nc,
                sc[32 * hh:32 * hh + S, :],
                ktc[64 * (h % 2):64 * (h % 2) + 64, ct, :],
                qt[64 * (h % 2):64 * (h % 2) + 64, ct, msl],
                start=True, stop=True,
                tile_position=(hp, 32 * hh))
        ex = pools.epool.tile([128, 512], BF16, name="ex_x", tag="ex")
        nc.scalar.activation(out=ex, in_=sc, func=AF.Exp)
        exs.append(ex)
    for h in range(NH):
        grp, hh = h // 4, h % 4
        sr = slice(32 * hh, 32 * hh + S)
        hp, ct = 64 * (h % 2), h // 2
        dn = pools.ps.tile([128, 512], F32, name="dn_x", tag="ps")
        _mm(nc, dn[0:1, :], d.ones_col_b[sr, :], exs[grp][sr, :],
                         start=True, stop=True,
                         tile_position=(32 * hh, 0))
        ob = pools.ps.tile([128, 512], F32, name="ob_x", tag="ps")
        _mm(nc, ob[hp:hp + 64, :], vc[sr, h, :],
                         exs[grp][sr, :], start=True, stop=True,
                         tile_position=(32 * hh, hp))
        rrow = pools.rows.tile([1, 512], F32, name="rr_x", tag="rr")
        nc.vector.reciprocal(rrow, dn[0:1, :])
        rbs = pools.epool.tile([128, 512], F32, name="rbs_x", tag="rbs")
        nc.gpsimd.partition_broadcast(rbs, rrow)
        nc.vector.tensor_mul(attn_sb[64 * (h % 2):64 * (h % 2) + 64,
                                     ct, msl],
                             ob[hp:hp + 64, :], rbs[hp:hp + 64, :])


def _block(d, pools, prefix, h_sb):
    """One transformer block, software-pipelined at token-half (ms)
    granularity: the second half of each primitive overlaps the first half
    of its consumer, and the self-attention windows for ms0 run while the
    ms1 projections are still in flight."""
    nc = d.nc
    z = pools.state.tile([128, CT, M], BF16, name="z", tag="zbuf")
    attn = pools.state.tile([128, CT, M], BF16, name="attn", tag="attnbuf")
    qt = pools.state.tile([128, CT, M], BF16, name=prefix + "qt", tag="qbuf")
    kt_ = pools.state.tile([128, CT, M], BF16, name=prefix + "kt", tag="kbuf")
    v_sb = pools.state.tile([128, 8, C], BF16, name=prefix + "v", tag="vbuf")
    setattr(d, prefix + "qt_sb", qt)
    setattr(d, prefix + "kt_sb", kt_)

    wk2 = _load_w(d, pools, getattr(d, prefix + "wk2"), "wk2")
    wv2 = _load_w(d, pools, getattr(d, prefix + "wv2"), "wv2")
    wq1 = _load_w(d, pools, getattr(d, prefix + "wq1"), "wq1")
    wk1 = _load_w(d, pools, getattr(d, prefix + "wk1"), "wk1")
    wv1 = _load_w(d, pools, getattr(d, prefix + "wv1"), "wv1")
    bq1 = getattr(d, prefix + "bq1_sb")
    bk1 = getattr(d, prefix + "bk1_sb")
    bv1r = getattr(d, prefix + "bv1r_sb")

    # ---- LN1 + qkv + self attention, pipelined over ms
    _mark(d, prefix + ":ln1")
    _layernorm(d, pools, h_sb, z, prefix + "1")
    # context K/V: independent of the token stream; emitted here so its
    # matmuls fill the PE gap during the LN row chain and keep the PE
    # p-state warm into the projections
    _mark(d, prefix + ":qkproj")
    _proj_act_ms(d, pools, wq1, bq1, z, qt, 0)
    _proj_act_ms(d, pools, wk1, bk1, z, kt_, 0)
    _v_proj_ms(d, pools, wv1, bv1r, z, v_sb, 0)
    _proj_act_ms(d, pools, wq1, bq1, z, qt, 1)
    _proj_act_ms(d, pools, wk1, bk1, z, kt_, 1)
    _v_proj_ms(d, pools, wv1, bv1r, z, v_sb, 1)
    _mark(d, prefix + ":selfattn")
    if prefix == "s":
        _attn_s(d, pools, v_sb, attn)
    else:
        _attn_g(d, pools, v_sb, attn)

    # context K/V: independent of the token stream; emitted here so its
    # matmuls fill PE gaps at the softmax tail (weights DMA'd long before)
    _ctx_kv(d, pools, prefix, wk2, wv2)


    # ---- o-projection (+res), LN2, q2, pipelined
    wo1 = _load_w(d, pools, getattr(d, prefix + "wo1"), "wo1")
    bo1 = getattr(d, prefix + "bo1_sb")
    _mark(d, prefix + ":oproj1")
    _proj_residual_ms(d, pools, wo1, bo1, attn, h_sb, 0)
    _proj_residual_ms(d, pools, wo1, bo1, attn, h_sb, 1)
    _mark(d, prefix + ":ln2")
    _layernorm(d, pools, h_sb, z, prefix + "2")
    wq2 = _load_w(d, pools, getattr(d, prefix + "wq2"), "wq2")
    bq2 = getattr(d, prefix + "bq2_sb")
    qt2 = pools.state.tile([128, CT, M], BF16, name=prefix + "qt2",
                           tag="kbuf")
    setattr(d, prefix + "qt2_sb", qt2)
    _mark(d, prefix + ":q2proj")
    _proj_act_ms(d, pools, wq2, bq2, z, qt2, 0)
    _proj_act_ms(d, pools, wq2, bq2, z, qt2, 1)
    _mark(d, prefix + ":xattn")
    _xattn_ms(d, pools, prefix, attn, 0)
    wo2 = _load_w(d, pools, getattr(d, prefix + "wo2"), "wo2")
    bo2 = getattr(d, prefix + "bo2_sb")
    _mark(d, prefix + ":oproj2")
    _proj_residual_ms(d, pools, wo2, bo2, attn, h_sb, 0)
    _xattn_ms(d, pools, prefix, attn, 1)
    _proj_residual_ms(d, pools, wo2, bo2, attn, h_sb, 1)

    # ---- mlp, pipelined
    wf1 = _load_w(d, pools, getattr(d, prefix + "wf1"), "wf1")
    wf2 = _load_w(d, pools, getattr(d, prefix + "wf2"), "wf2")
    bf1 = getattr(d, prefix + "bf1_sb")
    bf2 = getattr(d, prefix + "bf2_sb")
    _mark(d, prefix + ":ln3")
    _layernorm(d, pools, h_sb, z, prefix + "3")
    gbuf = pools.state.tile([128, CT, M], BF16, name=prefix + "g",
                            tag="gbuf")

    def ep_gelu(ct, ms, ps):
        nc.scalar.activation(
            out=gbuf[:, ct, ms * 512:(ms + 1) * 512], in_=ps, func=AF.Gelu,
            bias=bf1[:, ct:ct + 1], scale=1.0)

    _mark(d, prefix + ":mlp1")
    _linear_fm_ms(d, pools, wf1, z, ep_gelu, 0)
    _linear_fm_ms(d, pools, wf1, z, ep_gelu, 1)
    _mark(d, prefix + ":mlp2")
    _proj_residual_ms(d, pools, wf2, bf2, gbuf, h_sb, 0)
    _proj_residual_ms(d, pools, wf2, bf2, gbuf, h_sb, 1)


class Pools:
    pass


def _body(d, pools):
    """One full forward pass for this core's 1024 tokens."""
    nc = d.nc

    # constants and small tensors first (the weight DMAs are emitted lazily
    # at their use sites so the shared DMA queue can never head-of-line block
    # a load that an earlier weight's consumer depends on)
    xpt = pools.cpool.tile([48, M], BF16, name="xpt_sb")
    nc.sync.dma_start(out=xpt, in_=d.xpt.ap())
    wp = pools.cpool.tile([48, C], BF16, name="wp_sb")
    nc.sync.dma_start(out=wp, in_=d.wp.ap())
    d.convb_sb = _load_small(d, pools, d.convb, "convb")
    d.ctxt_sb = _load_small(d, pools, d.ctxt, "ctxt")
    tpos = _load_small(d, pools, d.tpos, "tpos")
    sppos = _load_small(d, pools, d.sppos, "sppos")
    for p in ("s", "g"):
        for nm in ("bq1", "bk1", "bo1", "bq2", "bo2", "bf1", "bf2"):
            setattr(d, p + nm + "_sb",
                    _load_small(d, pools, getattr(d, p + nm), p + nm))
        setattr(d, p + "bv1r_sb",
                _load_small(d, pools, getattr(d, p + "bv1r"), p + "bv1r"))
    d.bh_sb = _load_small(d, pools, d.bh, "bh")

    h = pools.state.tile([128, CT, M], BF16, name="h", tag="hbuf")

    # ---- patch embedding + positional embeddings
    _mark(d, "embed")
    for ct in range(CT):
        for ms in range(2):
            msl = slice(ms * 512, (ms + 1) * 512)
            ps = pools.ps.tile([128, 512], F32, name="pe_ps", tag="ps")
            _mm(nc, ps, wp[:, ct * 128:(ct + 1) * 128],
                             xpt[:, msl], start=True, stop=True)
            # + conv_b (per-partition) + t_pos (per frame: 2 frames per ms)
            tp = tpos[:, ct, 2 * ms:2 * ms + 2]
            tpv = tp.unsqueeze(2).broadcast_to([128, 2, 256])
            nc.vector.scalar_tensor_tensor(
                out=h[:, ct, msl].rearrange("p (f l) -> p f l", f=2),
                in0=ps.rearrange("p (f l) -> p f l", f=2),
                scalar=d.convb_sb[:, ct:ct + 1], in1=tpv,
                op0=ALU.add, op1=ALU.add)
        # + sp_pos (same 256 values for each of the 4 frames)
        spv = sppos[:, ct, :].unsqueeze(1).broadcast_to([128, 4, NP])
        hv = h[:, ct, :].rearrange("p (f l) -> p f l", f=4)
        nc.vector.tensor_add(hv, hv, spv)

    # ---- block s (spatial windows = frames)
    _block(d, pools, "s", h)

    # ---- permute tokens to m' (window-major for the 4x4x4 windows)
    _mark(d, "permute")
    hg = pools.state.tile([128, CT, M], BF16, name="hg", tag="zbuf2")
    for ct in range(CT):
        for a in range(4):
            src = bass_view_perm(h, ct, a)
            nc.vector.tensor_copy(
                out=hg[:, ct, a * 256:(a + 1) * 256].rearrange(
                    "p (b f i j) -> p b f i j", b=4, f=4, i=4),
                in_=src)
    _block(d, pools, "g", hg)

    # ---- output head (m' order; host undoes the permutation)
    _mark(d, "head")
    d.wh_sb = _load_w(d, pools, d.wh, "wh")
    outT = pools.cpool.tile([48, M], F32, name="outT")
    for ms in range(2):
        msl = slice(ms * 512, (ms + 1) * 512)
        ps = pools.ps.tile([128, 512], F32, name="hd_ps", tag="ps")
        for kt in range(CT):
            _mm(nc, ps[0:48, :], d.wh_sb[:, kt, :], hg[:, kt, msl],
                             start=(kt == 0), stop=False)
        _mm(nc, ps[0:48, :], d.bh_sb, d.ones_row_b,
                         start=False, stop=True)
        nc.scalar.copy(out=outT[:, msl], in_=ps[0:48, :])
    nc.sync.dma_start(out=d.out.ap(), in_=outT)


def bass_view_perm(h, ct, a):
    """View of h[:, ct, :] selecting m' block a*256..a*256+255 in (b,f,i,j)
    nested order: m = f*256 + (4a+i)*16 + 4b + j."""
    base = h[:, ct, :]
    return bass_ap_nest(base, a)


def bass_ap_nest(base, a):
    import concourse.bass as bass  # noqa
    v = base.rearrange("p (f ph pw) -> p f ph pw", f=4, ph=16)
    # dims: f(stride 256), ph=4a+i (stride 16), pw=4b+j (stride 1)
    v2 = v[:, :, 4 * a:4 * a + 4, :].rearrange(
        "p f i (b j) -> p b f i j", b=4)
    return v2


def _build_nc(loop_n=1):
    key = loop_n
    if key in _BUILD_CACHE:
        return _BUILD_CACHE[key]
    nc = bacc.Bacc("TRN2", target_bir_lowering=False, debug=False,
                   num_devices=8)
    d = _declare_inputs(nc)
    with tile.TileContext(nc) as tc, nc.allow_low_precision(
            reason="f32r matmul operands; output tolerance is 2e-2"):
        pools = Pools()
        import contextlib
        stack = contextlib.ExitStack()
        with stack:
            pools.ps = stack.enter_context(
                tc.tile_pool(name="ps", bufs=8, space="PSUM"))
            pools.wpool = stack.enter_context(
                tc.tile_pool(name="wpool", bufs=5))
            pools.cpool = stack.enter_context(
                tc.tile_pool(name="cpool", bufs=1))
            pools.state = stack.enter_context(
                tc.tile_pool(name="state", bufs=1))
            pools.epool = stack.enter_context(
                tc.tile_pool(name="epool", bufs=6))
            pools.rows = stack.enter_context(
                tc.tile_pool(name="rows", bufs=4))
            ones_col = pools.cpool.tile([128, 1], F32, name="ones_col")
            nc.vector.memset(ones_col, 1.0)
            ones_row = pools.cpool.tile([1, 512], F32, name="ones_row")
            nc.vector.memset(ones_row, 1.0)
            d.ones_col = ones_col
            d.ones_row = ones_row
            ones_col_b = pools.cpool.tile([128, 1], BF16, name="ones_col_b")
            nc.vector.tensor_copy(out=ones_col_b, in_=ones_col)
            d.ones_col_b = ones_col_b
            ones_row_b = pools.cpool.tile([1, 512], BF16, name="ones_row_b")
            nc.vector.tensor_copy(out=ones_row_b, in_=ones_row)
            d.ones_row_b = ones_row_b
            eps_row = pools.cpool.tile([1, 1], F32, name="eps_row")
            nc.vector.memset(eps_row, EPS)
            d.eps_row = eps_row
            # python-level unroll for timing variants (a tc.For_i back
            # edge wedges the device here; the unrolled NEFF is equivalent)
            for _ in range(loop_n):
                _body(d, pools)
    nc.compile()
    _BUILD_CACHE[key] = nc
    return nc


# ---------------------------------------------------------------- entry point

_MPRIME = None


def kernel(**inputs) -> np.ndarray:
    global _MPRIME
    if _MPRIME is None:
        _MPRIME = _mprime_index()
    nc = _build_nc(1)
    in_maps = host_prep(inputs)
    res = run_bass_kernel_spmd(nc, in_maps, core_ids=list(range(8)))
    # reassemble: per-core outT [48, 1024] in m' order -> full output
    out = np.zeros((B, CIN, F, IMG, IMG), np.float32)
    for c in range(8):
        b, t = c // 4, c % 4
        ot = res.results[c]["out"]                 # [48, M] m'-order
        om = np.empty_like(ot)
        om[:, _MPRIME] = ot                        # -> m order
        # om[(p,q,cc), (f,ph,pw)] -> out[b, cc, 4t+f, ph*4+p, pw*4+q]
        om = om.reshape(P, P, CIN, 4, PD, PD)
        om = om.transpose(2, 3, 4, 0, 5, 1)        # (cc, f, ph, p, pw, q)
        out[b, :, 4 * t:4 * t + 4] = om.reshape(CIN, 4, IMG, IMG)
    return out


if __name__ == "__main__":
    import reference
    ins = reference.setup_inputs()
    ins = {k: np.asarray(v) for k, v in ins.items()}
    exp = np.asarray(reference.reference(**ins))
    act = kernel(**ins)
    err = np.abs(act - exp).max() / (np.abs(exp).max() + 1e-12)
    print("Relative error:", err)



# revision 60
# speedup vs baseline: 563.2091x; 1.0582x over previous
# Trainium2 Bass kernel for nn_DiT_89086211653924 (windowed-attention video DiT).
#
# Sharding (8 cores, zero collectives): core c -> (batch b = c//4, temporal
# group t = c%4) = frames [4t, 4t+4), i.e. 1024 of the 4096 tokens of batch b.
# Both the spatial windows (one frame = 256 contiguous tokens) and the
# spatio-temporal 4x4x4 windows (within one temporal group) are core-local, and
# the action-context cross-attention K/V is a per-batch constant, so each core
# runs the full two-block transformer on its 1024 tokens independently.
#
# On-device layout: activations are feature-major [c_in partition (128), c_tile
# (4), token (1024)] so every linear is matmul(lhsT=W[k_tile, c_out_slice],
# rhs=xT[k_tile, tokens]) with no transposes anywhere.  Value projections emit
# token-major V by using the activation tile as the stationary operand instead.
# LayerNorm gamma/beta are folded into the following matmul's weights+bias on
# the host.
#
# Performance notes (all validated against the TimelineSim cost model and HW):
# - All matmul operands are bf16 (weights converted on the host): 1 PE
#   cycle/row at every size vs 4 for fp32, and half the weight DMA.  The
#   residual stream h is bf16 as well; PSUM accumulation stays fp32.
#   Measured HW rel err 7.8e-3 vs the 2e-2 gate.
# - Engines execute their queues in emission order, so emission IS the
#   schedule: attention is emitted as a two-stage software pipeline
#   (scores+exp of unit i before denominators/AV of unit i-lag) so the PE
#   never head-blocks on an Act exp round trip.
# - Softmax and LN partition-broadcasts run on the otherwise-idle GPSIMD
#   engine (partition_broadcast); LN z = h*r + psi reads all-bf16 SBUF
#   operands, hitting the DVE 2x/4x 16-bit modes.
# - Cross-attention K/V depend only on the 16 action tokens and are emitted
#   in PE gaps at the self-attention tail.
# - PSUM pool depth 8 (all banks), weight pool depth 5, with emission order
#   arranged so pool-rotation WAR edges always point forward (no deadlock).
# - The PE clock ramps with continuous busy time (p-state): idle gaps during
#   LN row chains would hand the next projection a half-speed PE, so bursts
#   of dependency-free burner matmuls into a dead PSUM row (_pe_warm) bridge
#   those gaps.  Sim-tuned to 24 matmuls per site; worth ~60us end to end.
import sys

sys.path.insert(0, "/opt/trn_rl_repo")

import numpy as np

import concourse.bacc as bacc
import concourse.mybir as mybir
import concourse.tile as tile
from concourse.bass_utils import run_bass_kernel_spmd

F32 = mybir.dt.float32
F32R = mybir.dt.float32r
BF16 = mybir.dt.bfloat16
AF = mybir.ActivationFunctionType
ALU = mybir.AluOpType


def _mm(nc, out, lhsT, rhs, **kw):
    """Plain matmul; operand dtypes come from the tiles (all bf16 here:
    1 PE cycle/row at any size, arbitrary dst partition, fp32 PSUM)."""
    nc.tensor.matmul(out, lhsT, rhs, **kw)

B, CIN, F, IMG, P = 2, 3, 16, 64, 4
PD = IMG // P          # 16 patches per side
NP = PD * PD           # 256 patches per frame
C, NH = 512, 8
HD = C // NH           # 64
S = 16                 # action context length
SCALE = float(1.0 / np.sqrt(HD))
M = 1024               # tokens per core (4 frames)
CT = 4                 # c tiles of 128
EPS = 1e-5

_BUILD_CACHE = {}
_PHASE_MARKS = []


def _pe_warm(d, pools, n):
    """n dependency-free back-to-back matmuls into a scratch PSUM tile.

    The PE clock ramps (p-state): ~2.4ns/row after 3us of continuous busy,
    2x slower otherwise.  LN row chains leave the PE idle long enough to
    reset the ramp, so the projections that follow run at half clock.  These
    burner matmuls execute during the (otherwise idle) LN tail and hand the
    following projection a warm PE.  They write a dead PSUM tile and depend
    only on constants, so they never delay real work ahead of them in the
    queue -- only the slight risk of outliving the LN chain."""
    nc = d.nc
    ps = pools.ps.tile([128, 512], F32, name="warm_ps", tag="ps")
    for _ in range(n):
        _mm(nc, ps[0:1, :], d.ones_col_b, d.warm_sb, start=True, stop=True)


def _mark(d, name):
    """Record (phase, instruction counter) for trace attribution; consumes
    one instruction name (gaps are harmless)."""
    n = int(d.nc.get_next_instruction_name().split("-")[1])
    _PHASE_MARKS.append((name, n))


# ---------------------------------------------------------------- host prep

def _fold_block(p, w):
    """Fold LN gamma/beta into the consuming projections for block prefix p.

    Returns dict of device arrays for this block."""
    g, b = w[p + "_ln_g"], w[p + "_ln_b"]          # [3, C]
    qkv1, wo1, bo1 = w[p + "_qkv1"], w[p + "_wo1"], w[p + "_bo1"]
    qkv2, wo2, bo2 = w[p + "_qkv2"], w[p + "_wo2"], w[p + "_bo2"]
    f1w, f1b = w[p + "_fc1w"], w[p + "_fc1b"]
    f2w, f2b = w[p + "_fc2w"], w[p + "_fc2b"]
    d = {}
    # self-attn: q (scaled by 1/sqrt(HD)), k, v from LN1(h)
    d["wq1"] = (g[0][:, None] * qkv1[0]) * SCALE
    d["bq1"] = (b[0] @ qkv1[0]) * SCALE
    d["wk1"] = g[0][:, None] * qkv1[1]
    d["bk1"] = b[0] @ qkv1[1]
    d["wv1"] = g[0][:, None] * qkv1[2]
    d["bv1"] = b[0] @ qkv1[2]
    d["wo1"], d["bo1"] = wo1, bo1
    # cross-attn: q from LN2(h) (scaled); k,v from raw ctx (no LN, no bias)
    d["wq2"] = (g[1][:, None] * qkv2[0]) * SCALE
    d["bq2"] = (b[1] @ qkv2[0]) * SCALE
    d["wk2"] = qkv2[1]
    d["wv2"] = qkv2[2]
    d["wo2"], d["bo2"] = wo2, bo2
    # mlp from LN3(h)
    d["wf1"] = g[2][:, None] * f1w
    d["bf1"] = b[2] @ f1w + f1b
    d["wf2"], d["bf2"] = f2w, f2b
    return d


def _w4(a):
    """[512, N] k-major weight -> [4, 128, N] (k_tile, k_in_tile, N), bf16."""
    import ml_dtypes
    return np.ascontiguousarray(
        np.asarray(a).reshape(4, 128, -1).astype(ml_dtypes.bfloat16))


def _colscal(a):
    """[512] bias -> [128, 4] per-partition scalar layout (col = c_tile)."""
    return np.ascontiguousarray(a.reshape(4, 128).T.astype(np.float32))


def _mprime_index():
    """m' (block-g window-major order) -> m (frame-major order)."""
    a, bb, f, i, j = np.meshgrid(
        np.arange(4), np.arange(4), np.arange(4), np.arange(4), np.arange(4),
        indexing="ij",
    )
    return (f * 256 + (4 * a + i) * 16 + 4 * bb + j).reshape(-1)


def host_prep(inputs):
    """Full inputs -> (shared weight map, list of 8 per-core input maps)."""
    w = {k: np.asarray(v) for k, v in inputs.items()}
    x = w["x"].astype(np.float32)                  # (B, CIN, F, IMG, IMG)
    actions = np.asarray(w["actions"])             # (B, S) int

    import ml_dtypes
    bf = ml_dtypes.bfloat16
    shared = {}
    shared["wp"] = np.ascontiguousarray(
        w["conv_w"].reshape(C, CIN * P * P).T.astype(bf))           # [48, 512]
    shared["convb"] = _colscal(w["conv_b"])
    shared["sppos"] = np.ascontiguousarray(
        w["sp_pos"].T.reshape(4, 128, NP).astype(np.float32))       # [4,128,256]
    shared["wh"] = _w4(w["head_w"])                                 # [4,128,48]
    shared["bh"] = np.ascontiguousarray(
        w["head_b"].reshape(1, 48).astype(bf))
    for p in ("s", "g"):
        fb = _fold_block(p, w)
        for nm in ("wq1", "wk1", "wv1", "wo1", "wq2", "wk2", "wv2", "wo2",
                   "wf1", "wf2"):
            shared[p + nm] = _w4(fb[nm])
        for nm in ("bq1", "bk1", "bo1", "bq2", "bo2", "bf1", "bf2"):
            shared[p + nm] = _colscal(fb[nm])
        # v biases enter via a K=1 matmul row (token-major output)
        shared[p + "bv1r"] = np.ascontiguousarray(
            fb["bv1"].reshape(1, C).astype(bf))

    per_core = []
    for c in range(8):
        b, t = c // 4, c % 4
        m = dict(shared)
        xs = x[b, :, 4 * t:4 * t + 4]              # (CIN, 4, IMG, IMG)
        xs = xs.reshape(CIN, 4, PD, P, PD, P)
        xs = xs.transpose(0, 3, 5, 1, 2, 4)        # (c, p, q, f, ph, pw)
        m["xpt"] = np.ascontiguousarray(
            xs.reshape(CIN * P * P, M).astype(ml_dtypes.bfloat16))  # [48, 1024]
        m["tpos"] = np.ascontiguousarray(
            w["t_pos"][4 * t:4 * t + 4].T.reshape(4, 128, 4)
            .astype(np.float32))                                    # [4,128,4]
        ctx = (w["act_table"][actions[b]] + w["act_pos"][0]).astype(np.float32)
        m["ctxt"] = np.ascontiguousarray(
            ctx.T.reshape(4, 128, S).astype(ml_dtypes.bfloat16))    # [4,128,16]
        per_core.append(m)
    return per_core


# ---------------------------------------------------------------- device build

class Dev:
    """Holds nc handles used across the builder functions."""


def _declare_inputs(nc):
    d = Dev()
    d.nc = nc
    mk = lambda name, shape, dt=F32: nc.dram_tensor(name, shape, dt,
                                                    kind="ExternalInput")
    d.xpt = mk("xpt", [48, M], BF16)
    d.tpos = mk("tpos", [4, 128, 4])
    d.sppos = mk("sppos", [4, 128, NP])
    d.ctxt = mk("ctxt", [4, 128, S], BF16)
    d.wp = mk("wp", [48, C], BF16)
    d.convb = mk("convb", [128, 4])
    d.wh = mk("wh", [4, 128, 48], BF16)
    d.bh = mk("bh", [1, 48], BF16)
    for p in ("s", "g"):
        for nm in ("wq1", "wk1", "wv1", "wo1", "wq2", "wk2", "wv2", "wo2",
                   "wf1", "wf2"):
            setattr(d, p + nm, mk(p + nm, [4, 128, C], BF16))
        for nm in ("bq1", "bk1", "bo1", "bq2", "bo2", "bf1", "bf2"):
            setattr(d, p + nm, mk(p + nm, [128, 4]))
        setattr(d, p + "bv1r", mk(p + "bv1r", [1, C], BF16))
    d.out = nc.dram_tensor("out", [48, M], F32, kind="ExternalOutput")
    return d


def _load_w(d, pools, dram, name):
    """DMA a [4,128,N] weight into sbuf [128, 4, N]."""
    nc = d.nc
    n = dram.shape[2]
    t = pools.wpool.tile([128, 4, n], dram.dtype, name=name, tag="w" + str(n))
    nc.sync.dma_start(out=t, in_=dram.ap().transpose([1, 0, 2]))
    return t


def _load_small(d, pools, dram, name):
    nc = d.nc
    shape = list(dram.shape)
    if len(shape) == 3:
        # [4, 128, N] dram -> [128, 4, N] sbuf (partition-major)
        t = pools.cpool.tile([shape[1], shape[0], shape[2]], dram.dtype,
                             name=name)
        nc.sync.dma_start(out=t, in_=dram.ap().transpose([1, 0, 2]))
    else:
        t = pools.cpool.tile(shape, dram.dtype, name=name)
        nc.sync.dma_start(out=t, in_=dram.ap())
    return t


def _linear_fm_ms(d, pools, w_sb, x_sb, epilogue, ms):
    """Feature-major linear for one token half ms: for each ct produce psum
    [128,512] = sum_kt w_sb[:,kt,ct*128:+128].T @ x_sb[:,kt,ms*512:+512],
    then call epilogue(ct, ms, psum)."""
    nc = d.nc
    for ct in range(CT):
        ps = pools.ps.tile([128, 512], F32, name="lin_ps", tag="ps")
        for kt in range(CT):
            _mm(nc,
                ps,
                w_sb[:, kt, ct * 128:(ct + 1) * 128],
                x_sb[:, kt, ms * 512:(ms + 1) * 512],
                start=(kt == 0), stop=(kt == 3),
            )
        epilogue(ct, ms, ps)


def _proj_act_ms(d, pools, w_sb, b_sb, x_sb, out_sb, ms):
    """Linear + per-c_out bias via ACT Identity, into feature-major out."""
    nc = d.nc

    def ep(ct, ms_, ps):
        nc.scalar.activation(
            out=out_sb[:, ct, ms_ * 512:(ms_ + 1) * 512], in_=ps,
            func=AF.Identity, bias=b_sb[:, ct:ct + 1], scale=1.0)

    _linear_fm_ms(d, pools, w_sb, x_sb, ep, ms)


def _proj_residual_ms(d, pools, w_sb, b_sb, x_sb, h_sb, ms):
    """h += x @ W + b  (one fused DVE op per tile)."""
    nc = d.nc

    def ep(ct, ms_, ps):
        sl = h_sb[:, ct, ms_ * 512:(ms_ + 1) * 512]
        nc.vector.scalar_tensor_tensor(
            out=sl, in0=ps, scalar=b_sb[:, ct:ct + 1], in1=sl,
            op0=ALU.add, op1=ALU.add)

    _linear_fm_ms(d, pools, w_sb, x_sb, ep, ms)


def _layernorm_ms(d, pools, h_sb, z_sb, tag, ms):
    """LN for one token half; kept for pipelined call sites."""
    _layernorm_emit(d, pools, h_sb, z_sb, tag, (ms,))


def _layernorm(d, pools, h_sb, z_sb, tag):
    _layernorm_emit(d, pools, h_sb, z_sb, tag, (0, 1))


def _layernorm_emit(d, pools, h_sb, z_sb, tag, mss):
    """z = (h - mean_c(h)) * rsqrt(var_c(h) + eps), feature-major.

    h stays fp32 (residual precision), so its column-sum matmul runs fp32;
    the squared-sum runs bf16 off the bf16 Square output.  Emission is
    batched stage-by-stage across the token halves so each engine gets both
    halves' work back-to-back; the broadcast PSUM tiles are consumed
    directly by the z DVE ops (one PSUM operand per DVE op)."""
    nc = d.nc
    if not hasattr(d, "_ln_sq"):
        d._ln_sq = {}
        d._ln_rows = {}
    if tag not in d._ln_sq or mss[0] == 0:
        d._ln_sq[tag] = pools.state.tile([128, CT, M], BF16, name="lnsq",
                                         tag="scratch16")
        d._ln_rows[tag] = pools.rows.tile([1, 4, M], BF16,
                                          name="lnrows_" + tag,
                                          tag="lnrows", bufs=1)
    sq = d._ln_sq[tag]
    rows = d._ln_rows[tag]
    MU, A, R, PSI = range(4)
    sl = lambda ms: slice(ms * 512, (ms + 1) * 512)
    mps = {}
    for ms in mss:
        ps = pools.ps.tile([128, 512], F32, name="ln_mps", tag="ps")
        for kt in range(CT):
            _mm(nc, ps[0:1, :], d.ones_col_b, h_sb[:, kt, sl(ms)],
                start=(kt == 0), stop=(kt == 3))
        mps[ms] = ps
    for ms in mss:
        nc.scalar.activation(out=sq[:, :, sl(ms)], in_=h_sb[:, :, sl(ms)],
                             func=AF.Square)
    vps = {}
    for ms in mss:
        nc.vector.tensor_scalar_mul(rows[:, MU, sl(ms)], mps[ms][0:1, :],
                                    1.0 / C)
        ps = pools.ps.tile([128, 512], F32, name="ln_vps", tag="ps")
        for kt in range(CT):
            _mm(nc, ps[0:1, :], d.ones_col_b, sq[:, kt, sl(ms)],
                start=(kt == 0), stop=(kt == 3))
        vps[ms] = ps
    for ms in mss:
        msl = sl(ms)
        nc.vector.tensor_scalar_mul(rows[:, A, msl], vps[ms][0:1, :], 1.0 / C)
    # var = A - mu^2 ; r = 1/sqrt(var + eps) ; psi = -mu * r
    # (kept per-half: z(ms0) must land as early as possible so the next
    # projection's ms0 matmuls can start while the ms1 chain still runs)
    for msl in [sl(ms) for ms in mss]:
        nc.vector.tensor_mul(rows[:, R, msl], rows[:, MU, msl],
                             rows[:, MU, msl])
        nc.vector.tensor_sub(rows[:, A, msl], rows[:, A, msl],
                             rows[:, R, msl])
        nc.scalar.activation(out=rows[:, A, msl], in_=rows[:, A, msl],
                             func=AF.Sqrt, bias=d.eps_row)
        nc.vector.reciprocal(rows[:, R, msl], rows[:, A, msl])
        nc.vector.scalar_tensor_tensor(
            out=rows[:, PSI, msl], in0=rows[:, MU, msl], scalar=-1.0,
            in1=rows[:, R, msl], op0=ALU.mult, op1=ALU.mult)
    # broadcast r and psi across partitions on the idle GPSIMD engine;
    # all-bf16 SBUF operands put the big z DVE ops in the 2x/4x perf modes
    for ms in mss:
        msl = sl(ms)
        rbs = pools.epool.tile([128, 2, 512], BF16, name="lnb", tag="lnb")
        nc.gpsimd.partition_broadcast(rbs[:, 0, :], rows[:, R, msl])
        nc.gpsimd.partition_broadcast(rbs[:, 1, :], rows[:, PSI, msl])
        zv = z_sb[:, :, msl]
        rv = rbs[:, 0, :].unsqueeze(1).broadcast_to([128, CT, 512])
        pv = rbs[:, 1, :].unsqueeze(1).broadcast_to([128, CT, 512])
        nc.vector.tensor_mul(zv, h_sb[:, :, msl], rv)
        nc.vector.tensor_add(zv, zv, pv)


def _v_proj_ms(d, pools, w_sb, brow_sb, x_sb, v_sb, ms):
    """Token-major value projection for token half ms: v_sb[128 tok, mt, C]
    with bias via an extra K=1 matmul against the bias row."""
    nc = d.nc
    for mt in range(4 * ms, 4 * ms + 4):
        ps = pools.ps.tile([128, 512], F32, name="v_ps", tag="ps")
        for kt in range(CT):
            _mm(nc,
                ps, x_sb[:, kt, mt * 128:(mt + 1) * 128],
                w_sb[:, kt, :], start=(kt == 0),
                stop=(kt == 3 and brow_sb is None))
        if brow_sb is not None:
            _mm(nc, ps, d.ones_row_b[:, 0:128], brow_sb,
                             start=False, stop=True)
        nc.scalar.copy(out=v_sb[:, mt, :], in_=ps)


def _attn_s_stage1(d, pools, w, pair):
    """Scores + exp for heads (2*pair, 2*pair+1) of window w."""
    nc = d.nc
    out = {}
    for ho in range(2):
        h = 2 * pair + ho
        hp, ct = 64 * (h % 2), h // 2
        krows = slice(hp, hp + 64)
        sc = pools.ps.tile([128, 2, 256], F32, name="sc", tag="ps")
        for st in range(2):
            _mm(nc,
                sc[:, st, :],
                d.skt_sb[krows, ct, w * 256 + st * 128:
                         w * 256 + st * 128 + 128],
                d.sqt_sb[krows, ct, w * 256:w * 256 + 256],
                start=True, stop=True)
        ex = pools.epool.tile([128, 2, 256], BF16, name="ex", tag="ex")
        nc.scalar.activation(out=ex, in_=sc, func=AF.Exp)
        out[ho] = ex
    return out


def _attn_s_stage2(d, pools, v_sb, attn_sb, w, pair, exs):
    """dn/ob matmuls, reciprocal, broadcast and normalize for one pair."""
    nc = d.nc
    dn = pools.ps.tile([128, 512], F32, name="dn", tag="ps")
    obs = {}
    for ho in range(2):
        h = 2 * pair + ho
        hp = 64 * (h % 2)
        ob = pools.ps.tile([128, 512], F32, name="ob", tag="ps")
        obs[ho] = ob
        for st in range(2):
            _mm(nc, dn[0:1, 256 * ho:256 * ho + 256], d.ones_col_b,
                exs[ho][:, st, :], start=(st == 0), stop=(st == 1),
                tile_position=(0, 0))
            _mm(nc,
                ob[hp:hp + 64, 0:256],
                v_sb[:, 2 * w + st, h * 64:h * 64 + 64],
                exs[ho][:, st, :], start=(st == 0), stop=(st == 1),
                tile_position=(0, hp))
    rrow = pools.rows.tile([1, 512], F32, name="rrow", tag="rr")
    nc.vector.reciprocal(rrow, dn[0:1, :])
    rbs = pools.epool.tile([128, 2, 256], F32, name="rbs", tag="rbs")
    for ho in range(2):
        h = 2 * pair + ho
        hp, ct = 64 * (h % 2), h // 2
        krows = slice(hp, hp + 64)
        nc.gpsimd.partition_broadcast(rbs[:, ho, :],
                                      rrow[:, 256 * ho:256 * ho + 256])
        nc.vector.tensor_mul(
            attn_sb[krows, ct, w * 256:w * 256 + 256],
            obs[ho][hp:hp + 64, 0:256], rbs[hp:hp + 64, ho, :])


def _attn_s(d, pools, v_sb, attn_sb, lag=1):
    """Block-s self attention, software-pipelined: stage1 (scores+exp) of
    unit i is emitted before stage2 of unit i-lag, so the in-order PE queue
    always has ready score matmuls ahead of denominator matmuls that wait
    on the Act exp round-trip."""
    units = [(w, pair) for w in range(4) for pair in range(4)]
    pend = {}
    for i in range(len(units) + lag):
        if i < len(units):
            w, pair = units[i]
            pend[i] = _attn_s_stage1(d, pools, w, pair)
        j = i - lag
        if j >= 0:
            w, pair = units[j]
            _attn_s_stage2(d, pools, v_sb, attn_sb, w, pair, pend.pop(j))


def _attn_g_stage1(d, pools, wp):
    """Scores + exp for one window-pair of block-g self attention.

    PSUM hazard rule (HW-verified): within one bank, concurrent matmuls with
    different row groups but overlapping column strips fault.  So scores are
    banked by head parity (source rows fixed per bank) and o/denominator by
    window parity (rows = 64j fixed per bank)."""
    nc = d.nc
    scs = [pools.ps.tile([128, 4, 64], F32, name=f"sc_g{p}", tag="ps")
           for p in range(2)]
    for h in range(NH):
        p, ct = h % 2, h // 2
        hp = 64 * p
        for j in range(2):
            w = 2 * wp + j
            _mm(nc,
                scs[p][64 * j:64 * j + 64, ct, :],
                d.gkt_sb[hp:hp + 64, ct, w * 64:w * 64 + 64],
                d.gqt_sb[hp:hp + 64, ct, w * 64:w * 64 + 64],
                start=True, stop=True, tile_position=(hp, 64 * j))
    exs = []
    for p in range(2):
        ex = pools.epool.tile([128, 4, 64], BF16, name=f"ex_g{p}",
                              tag="ex")
        nc.scalar.activation(out=ex, in_=scs[p], func=AF.Exp)
        exs.append(ex)
    return exs


def _attn_g_stage2(d, pools, v_sb, attn_sb, wp, exs):
    nc = d.nc
    ogs = [pools.ps.tile([128, 4, 64], F32, name=f"og{j}", tag="ps")
           for j in range(2)]
    dns = [pools.ps.tile([128, 512], F32, name=f"dn_g{j}", tag="ps")
           for j in range(2)]
    for h in range(NH):
        p, ct = h % 2, h // 2
        for j in range(2):
            jr = slice(64 * j, 64 * j + 64)
            _mm(nc,
                dns[j][0:1, h * 64:h * 64 + 64],
                d.ones_col_b[jr, :], exs[p][jr, ct, :],
                start=True, stop=True, tile_position=(64 * j, 0))
            _mm(nc,
                ogs[j][64 * p:64 * p + 64, ct, :],
                v_sb[jr, wp, h * 64:h * 64 + 64],
                exs[p][jr, ct, :],
                start=True, stop=True, tile_position=(64 * j, 64 * p))
    for j in range(2):
        rrow = pools.rows.tile([1, 512], F32, name=f"rr_g{j}", tag="rr")
        nc.vector.reciprocal(rrow, dns[j][0:1, :])
        rbs = pools.epool.tile([128, 512], F32, name=f"rbs_g{j}",
                               tag="rbs")
        nc.gpsimd.partition_broadcast(rbs, rrow)
        for p in range(2):
            pr = slice(64 * p, 64 * p + 64)
            out_view = attn_sb[pr, :, (2 * wp + j) * 64:
                               (2 * wp + j) * 64 + 64]
            in1 = rbs[pr, :].rearrange(
                "q (ct two l) -> q ct two l", two=2, l=64)[:, :, p, :]
            nc.vector.tensor_mul(out_view, ogs[j][pr, :, :], in1)


def _attn_g(d, pools, v_sb, attn_sb, lag=1):
    pend = {}
    for i in range(8 + lag):
        if i < 8:
            pend[i] = _attn_g_stage1(d, pools, i)
        j = i - lag
        if j >= 0:
            _attn_g_stage2(d, pools, v_sb, attn_sb, j, pend.pop(j))


def _ctx_kv(d, pools, prefix, wk, wv):
    """Cross-attention K/V from the 16 action-context tokens.  Independent
    of the token stream, so emitted during the LN1 row chain to stay off
    the critical path.  kT [128, 4, 16] feature-major; V [16, C] token-
    major, replicated to the four 32-partition strips."""
    nc = d.nc
    ktc = pools.state.tile([128, 4, S], BF16, name=prefix + "ktc",
                           tag=prefix + "ktc")
    for ct in range(CT):
        ps = pools.ps.tile([128, 512], F32, name="ktc_ps", tag="ps")
        for kt in range(CT):
            _mm(nc, ps[:, 0:S], wk[:, kt, ct * 128:(ct + 1) * 128],
                             d.ctxt_sb[:, kt, :], start=(kt == 0),
                             stop=(kt == 3))
        nc.scalar.copy(out=ktc[:, ct, :], in_=ps[:, 0:S])
    vc = pools.state.tile([128, 8, 64], BF16, name=prefix + "vc",
                          tag=prefix + "vc")
    ps = pools.ps.tile([128, 512], F32, name="vc_ps", tag="ps")
    for kt in range(CT):
        _mm(nc, ps[0:S, :], d.ctxt_sb[:, kt, :], wv[:, kt, :],
                         start=(kt == 0), stop=(kt == 3))
    vcv = vc.rearrange("p h d -> p (h d)")
    nc.scalar.copy(out=vcv[0:S, :], in_=ps[0:S, :])
    for q in range(1, 4):
        nc.sync.dma_start(out=vcv[32 * q:32 * q + S, :], in_=vcv[0:S, :])
    setattr(d, prefix + "ktc_sb", ktc)
    setattr(d, prefix + "vc_sb", vc)


def _xattn_s1(d, pools, prefix, ms):
    """Cross attention scores + exp for token half ms."""
    nc = d.nc
    qt = getattr(d, prefix + "qt2_sb")
    ktc = getattr(d, prefix + "ktc_sb")
    msl = slice(ms * 512, (ms + 1) * 512)
    exs = []
    for grp in range(2):                       # heads 4*grp .. 4*grp+3
        # no memset: rows outside the four 16-row strips are garbage but
        # are never read (dn/ob matmuls slice sr exactly)
        sc = pools.ps.tile([128, 512], F32, name="sc_x", tag="ps")
        for hh in range(4):
            h = 4 * grp + hh
            hp, ct = 64 * (h % 2), h // 2
            _mm(nc,
                sc[32 * hh:32 * hh + S, :],
                ktc[64 * (h % 2):64 * (h % 2) + 64, ct, :],
                qt[64 * (h % 2):64 * (h % 2) + 64, ct, msl],
                start=True, stop=True,
                tile_position=(hp, 32 * hh))
        ex = pools.epool.tile([128, 512], BF16, name="ex_x", tag="ex")
        nc.scalar.activation(out=ex, in_=sc, func=AF.Exp)
        exs.append(ex)
    setattr(d, prefix + "_xexs%d" % ms, exs)


def _xattn_s2(d, pools, prefix, attn_sb, ms):
    """Cross attention denominators/AV/normalize for token half ms."""
    nc = d.nc
    vc = getattr(d, prefix + "vc_sb")
    exs = getattr(d, prefix + "_xexs%d" % ms)
    msl = slice(ms * 512, (ms + 1) * 512)
    for h in range(NH):
        grp, hh = h // 4, h % 4
        sr = slice(32 * hh, 32 * hh + S)
        hp, ct = 64 * (h % 2), h // 2
        dn = pools.ps.tile([128, 512], F32, name="dn_x", tag="ps")
        _mm(nc, dn[0:1, :], d.ones_col_b[sr, :], exs[grp][sr, :],
                         start=True, stop=True,
                         tile_position=(32 * hh, 0))
        ob = pools.ps.tile([128, 512], F32, name="ob_x", tag="ps")
        _mm(nc, ob[hp:hp + 64, :], vc[sr, h, :],
                         exs[grp][sr, :], start=True, stop=True,
                         tile_position=(32 * hh, hp))
        rrow = pools.rows.tile([1, 512], F32, name="rr_x", tag="rr")
        nc.vector.reciprocal(rrow, dn[0:1, :])
        rbs = pools.epool.tile([128, 512], F32, name="rbs_x", tag="rbs")
        nc.gpsimd.partition_broadcast(rbs, rrow)
        nc.vector.tensor_mul(attn_sb[64 * (h % 2):64 * (h % 2) + 64,
                                     ct, msl],
                             ob[hp:hp + 64, :], rbs[hp:hp + 64, :])


def _block(d, pools, prefix, h_sb):
    """One transformer block, software-pipelined at token-half (ms)
    granularity: the second half of each primitive overlaps the first half
    of its consumer, and the self-attention windows for ms0 run while the
    ms1 projections are still in flight."""
    nc = d.nc
    z = pools.state.tile([128, CT, M], BF16, name="z", tag="zbuf")
    attn = pools.state.tile([128, CT, M], BF16, name="attn", tag="attnbuf")
    qt = pools.state.tile([128, CT, M], BF16, name=prefix + "qt", tag="qbuf")
    kt_ = pools.state.tile([128, CT, M], BF16, name=prefix + "kt", tag="kbuf")
    v_sb = pools.state.tile([128, 8, C], BF16, name=prefix + "v", tag="vbuf")
    setattr(d, prefix + "qt_sb", qt)
    setattr(d, prefix + "kt_sb", kt_)

    wk2 = _load_w(d, pools, getattr(d, prefix + "wk2"), "wk2")
    wv2 = _load_w(d, pools, getattr(d, prefix + "wv2"), "wv2")
    wq1 = _load_w(d, pools, getattr(d, prefix + "wq1"), "wq1")
    wk1 = _load_w(d, pools, getattr(d, prefix + "wk1"), "wk1")
    wv1 = _load_w(d, pools, getattr(d, prefix + "wv1"), "wv1")
    bq1 = getattr(d, prefix + "bq1_sb")
    bk1 = getattr(d, prefix + "bk1_sb")
    bv1r = getattr(d, prefix + "bv1r_sb")

    # ---- LN1 + qkv + self attention, pipelined over ms
    _mark(d, prefix + ":ln1")
    _layernorm(d, pools, h_sb, z, prefix + "1")
    # context K/V: independent of the token stream; emitted here so its
    # matmuls fill the PE gap during the LN row chain and keep the PE
    # p-state warm into the projections
    _pe_warm(d, pools, 24)
    _mark(d, prefix + ":qkproj")
    _proj_act_ms(d, pools, wq1, bq1, z, qt, 0)
    _proj_act_ms(d, pools, wk1, bk1, z, kt_, 0)
    _v_proj_ms(d, pools, wv1, bv1r, z, v_sb, 0)
    _proj_act_ms(d, pools, wq1, bq1, z, qt, 1)
    _proj_act_ms(d, pools, wk1, bk1, z, kt_, 1)
    _v_proj_ms(d, pools, wv1, bv1r, z, v_sb, 1)
    _mark(d, prefix + ":selfattn")
    if prefix == "s":
        _attn_s(d, pools, v_sb, attn)
    else:
        _attn_g(d, pools, v_sb, attn)

    # context K/V: independent of the token stream; emitted here so its
    # matmuls fill PE gaps at the softmax tail (weights DMA'd long before)
    _ctx_kv(d, pools, prefix, wk2, wv2)


    # ---- o-projection (+res), LN2, q2, pipelined
    wo1 = _load_w(d, pools, getattr(d, prefix + "wo1"), "wo1")
    bo1 = getattr(d, prefix + "bo1_sb")
    _mark(d, prefix + ":oproj1")
    _proj_residual_ms(d, pools, wo1, bo1, attn, h_sb, 0)
    _proj_residual_ms(d, pools, wo1, bo1, attn, h_sb, 1)
    _mark(d, prefix + ":ln2")
    _layernorm(d, pools, h_sb, z, prefix + "2")
    wq2 = _load_w(d, pools, getattr(d, prefix + "wq2"), "wq2")
    bq2 = getattr(d, prefix + "bq2_sb")
    qt2 = pools.state.tile([128, CT, M], BF16, name=prefix + "qt2",
                           tag="kbuf")
    setattr(d, prefix + "qt2_sb", qt2)
    _pe_warm(d, pools, 24)
    _mark(d, prefix + ":q2proj")
    _proj_act_ms(d, pools, wq2, bq2, z, qt2, 0)
    _mark(d, prefix + ":xattn")
    _xattn_s1(d, pools, prefix, 0)
    _proj_act_ms(d, pools, wq2, bq2, z, qt2, 1)
    _xattn_s2(d, pools, prefix, attn, 0)
    wo2 = _load_w(d, pools, getattr(d, prefix + "wo2"), "wo2")
    bo2 = getattr(d, prefix + "bo2_sb")
    _mark(d, prefix + ":oproj2")
    _proj_residual_ms(d, pools, wo2, bo2, attn, h_sb, 0)
    _xattn_s1(d, pools, prefix, 1)
    _xattn_s2(d, pools, prefix, attn, 1)
    _proj_residual_ms(d, pools, wo2, bo2, attn, h_sb, 1)

    # ---- mlp, pipelined
    wf1 = _load_w(d, pools, getattr(d, prefix + "wf1"), "wf1")
    wf2 = _load_w(d, pools, getattr(d, prefix + "wf2"), "wf2")
    bf1 = getattr(d, prefix + "bf1_sb")
    bf2 = getattr(d, prefix + "bf2_sb")
    _mark(d, prefix + ":ln3")
    _layernorm(d, pools, h_sb, z, prefix + "3")
    gbuf = pools.state.tile([128, CT, M], BF16, name=prefix + "g",
                            tag="gbuf")

    def ep_gelu(ct, ms, ps):
        nc.scalar.activation(
            out=gbuf[:, ct, ms * 512:(ms + 1) * 512], in_=ps, func=AF.Gelu,
            bias=bf1[:, ct:ct + 1], scale=1.0)

    _pe_warm(d, pools, 24)
    _mark(d, prefix + ":mlp1")
    _linear_fm_ms(d, pools, wf1, z, ep_gelu, 0)
    _linear_fm_ms(d, pools, wf1, z, ep_gelu, 1)
    _mark(d, prefix + ":mlp2")
    _proj_residual_ms(d, pools, wf2, bf2, gbuf, h_sb, 0)
    _proj_residual_ms(d, pools, wf2, bf2, gbuf, h_sb, 1)


class Pools:
    pass


def _body(d, pools):
    """One full forward pass for this core's 1024 tokens."""
    nc = d.nc

    # constants and small tensors first (the weight DMAs are emitted lazily
    # at their use sites so the shared DMA queue can never head-of-line block
    # a load that an earlier weight's consumer depends on)
    xpt = pools.cpool.tile([48, M], BF16, name="xpt_sb")
    nc.sync.dma_start(out=xpt, in_=d.xpt.ap())
    wp = pools.cpool.tile([48, C], BF16, name="wp_sb")
    nc.sync.dma_start(out=wp, in_=d.wp.ap())
    d.convb_sb = _load_small(d, pools, d.convb, "convb")
    d.ctxt_sb = _load_small(d, pools, d.ctxt, "ctxt")
    tpos = _load_small(d, pools, d.tpos, "tpos")
    sppos = _load_small(d, pools, d.sppos, "sppos")
    for p in ("s", "g"):
        for nm in ("bq1", "bk1", "bo1", "bq2", "bo2", "bf1", "bf2"):
            setattr(d, p + nm + "_sb",
                    _load_small(d, pools, getattr(d, p + nm), p + nm))
        setattr(d, p + "bv1r_sb",
                _load_small(d, pools, getattr(d, p + "bv1r"), p + "bv1r"))
    d.bh_sb = _load_small(d, pools, d.bh, "bh")

    h = pools.state.tile([128, CT, M], BF16, name="h", tag="hbuf")

    # ---- patch embedding + positional embeddings
    _mark(d, "embed")
    for ct in range(CT):
        for ms in range(2):
            msl = slice(ms * 512, (ms + 1) * 512)
            ps = pools.ps.tile([128, 512], F32, name="pe_ps", tag="ps")
            _mm(nc, ps, wp[:, ct * 128:(ct + 1) * 128],
                             xpt[:, msl], start=True, stop=True)
            # + conv_b (per-partition) + t_pos (per frame: 2 frames per ms)
            tp = tpos[:, ct, 2 * ms:2 * ms + 2]
            tpv = tp.unsqueeze(2).broadcast_to([128, 2, 256])
            nc.vector.scalar_tensor_tensor(
                out=h[:, ct, msl].rearrange("p (f l) -> p f l", f=2),
                in0=ps.rearrange("p (f l) -> p f l", f=2),
                scalar=d.convb_sb[:, ct:ct + 1], in1=tpv,
                op0=ALU.add, op1=ALU.add)
        # + sp_pos (same 256 values for each of the 4 frames)
        spv = sppos[:, ct, :].unsqueeze(1).broadcast_to([128, 4, NP])
        hv = h[:, ct, :].rearrange("p (f l) -> p f l", f=4)
        nc.vector.tensor_add(hv, hv, spv)

    # ---- block s (spatial windows = frames)
    _block(d, pools, "s", h)

    # ---- permute tokens to m' (window-major for the 4x4x4 windows)
    _mark(d, "permute")
    hg = pools.state.tile([128, CT, M], BF16, name="hg", tag="zbuf2")
    for ct in range(CT):
        for a in range(4):
            src = bass_view_perm(h, ct, a)
            nc.vector.tensor_copy(
                out=hg[:, ct, a * 256:(a + 1) * 256].rearrange(
                    "p (b f i j) -> p b f i j", b=4, f=4, i=4),
                in_=src)
    _block(d, pools, "g", hg)

    # ---- output head (m' order; host undoes the permutation)
    _mark(d, "head")
    d.wh_sb = _load_w(d, pools, d.wh, "wh")
    outT = pools.cpool.tile([48, M], F32, name="outT")
    for ms in range(2):
        msl = slice(ms * 512, (ms + 1) * 512)
        ps = pools.ps.tile([128, 512], F32, name="hd_ps", tag="ps")
        for kt in range(CT):
            _mm(nc, ps[0:48, :], d.wh_sb[:, kt, :], hg[:, kt, msl],
                             start=(kt == 0), stop=False)
        _mm(nc, ps[0:48, :], d.bh_sb, d.ones_row_b,
                         start=False, stop=True)
        nc.scalar.copy(out=outT[:, msl], in_=ps[0:48, :])
    nc.sync.dma_start(out=d.out.ap(), in_=outT)


def bass_view_perm(h, ct, a):
    """View of h[:, ct, :] selecting m' block a*256..a*256+255 in (b,f,i,j)
    nested order: m = f*256 + (4a+i)*16 + 4b + j."""
    base = h[:, ct, :]
    return bass_ap_nest(base, a)


def bass_ap_nest(base, a):
    import concourse.bass as bass  # noqa
    v = base.rearrange("p (f ph pw) -> p f ph pw", f=4, ph=16)
    # dims: f(stride 256), ph=4a+i (stride 16), pw=4b+j (stride 1)
    v2 = v[:, :, 4 * a:4 * a + 4, :].rearrange(
        "p f i (b j) -> p b f i j", b=4)
    return v2


def _build_nc(loop_n=1):
    key = loop_n
    if key in _BUILD_CACHE:
        return _BUILD_CACHE[key]
    nc = bacc.Bacc("TRN2", target_bir_lowering=False, debug=False,
                   num_devices=8)
    d = _declare_inputs(nc)
    with tile.TileContext(nc) as tc, nc.allow_low_precision(
            reason="f32r matmul operands; output tolerance is 2e-2"):
        pools = Pools()
        import contextlib
        stack = contextlib.ExitStack()
        with stack:
            pools.ps = stack.enter_context(
                tc.tile_pool(name="ps", bufs=8, space="PSUM"))
            pools.wpool = stack.enter_context(
                tc.tile_pool(name="wpool", bufs=5))
            pools.cpool = stack.enter_context(
                tc.tile_pool(name="cpool", bufs=1))
            pools.state = stack.enter_context(
                tc.tile_pool(name="state", bufs=1))
            pools.epool = stack.enter_context(
                tc.tile_pool(name="epool", bufs=6))
            pools.rows = stack.enter_context(
                tc.tile_pool(name="rows", bufs=4))
            ones_col = pools.cpool.tile([128, 1], F32, name="ones_col")
            nc.vector.memset(ones_col, 1.0)
            ones_row = pools.cpool.tile([1, 512], F32, name="ones_row")
            nc.vector.memset(ones_row, 1.0)
            d.ones_col = ones_col
            d.ones_row = ones_row
            ones_col_b = pools.cpool.tile([128, 1], BF16, name="ones_col_b")
            nc.vector.tensor_copy(out=ones_col_b, in_=ones_col)
            d.ones_col_b = ones_col_b
            ones_row_b = pools.cpool.tile([1, 512], BF16, name="ones_row_b")
            nc.vector.tensor_copy(out=ones_row_b, in_=ones_row)
            d.ones_row_b = ones_row_b
            warm_sb = pools.cpool.tile([128, 512], BF16, name="warm_sb")
            nc.vector.memset(warm_sb, 0.0)
            d.warm_sb = warm_sb
            eps_row = pools.cpool.tile([1, 1], F32, name="eps_row")
            nc.vector.memset(eps_row, EPS)
            d.eps_row = eps_row
            # python-level unroll for timing variants (a tc.For_i back
            # edge wedges the device here; the unrolled NEFF is equivalent)
            for _ in range(loop_n):
                _body(d, pools)
    nc.compile()
    _BUILD_CACHE[key] = nc
    return nc


# ---------------------------------------------------------------- entry point

_MPRIME = None


def kernel(**inputs) -> np.ndarray:
    global _MPRIME
    if _MPRIME is None:
        _MPRIME = _mprime_index()
    nc = _build_nc(1)
    in_maps = host_prep(inputs)
    res = run_bass_kernel_spmd(nc, in_maps, core_ids=list(range(8)))
    # reassemble: per-core outT [48, 1024] in m' order -> full output
    out = np.zeros((B, CIN, F, IMG, IMG), np.float32)
    for c in range(8):
        b, t = c // 4, c % 4
        ot = res.results[c]["out"]                 # [48, M] m'-order
        om = np.empty_like(ot)
        om[:, _MPRIME] = ot                        # -> m order
        # om[(p,q,cc), (f,ph,pw)] -> out[b, cc, 4t+f, ph*4+p, pw*4+q]
        om = om.reshape(P, P, CIN, 4, PD, PD)
        om = om.transpose(2, 3, 4, 0, 5, 1)        # (cc, f, ph, p, pw, q)
        out[b, :, 4 * t:4 * t + 4] = om.reshape(CIN, 4, IMG, IMG)
    return out


if __name__ == "__main__":
    import reference
    ins = reference.setup_inputs()
    ins = {k: np.asarray(v) for k, v in ins.items()}
    exp = np.asarray(reference.reference(**ins))
    act = kernel(**ins)
    err = np.abs(act - exp).max() / (np.abs(exp).max() + 1e-12)
    print("Relative error:", err)



# revision 66
# speedup vs baseline: 577.8683x; 1.0260x over previous
# Trainium2 Bass kernel for nn_DiT_89086211653924 (windowed-attention video DiT).
#
# Sharding (8 cores, zero collectives): core c -> (batch b = c//4, temporal
# group t = c%4) = frames [4t, 4t+4), i.e. 1024 of the 4096 tokens of batch b.
# Both the spatial windows (one frame = 256 contiguous tokens) and the
# spatio-temporal 4x4x4 windows (within one temporal group) are core-local, and
# the action-context cross-attention K/V is a per-batch constant, so each core
# runs the full two-block transformer on its 1024 tokens independently.
#
# On-device layout: activations are feature-major [c_in partition (128), c_tile
# (4), token (1024)] so every linear is matmul(lhsT=W[k_tile, c_out_slice],
# rhs=xT[k_tile, tokens]) with no transposes anywhere.  Value projections emit
# token-major V by using the activation tile as the stationary operand instead.
# LayerNorm gamma/beta are folded into the following matmul's weights+bias on
# the host.
#
# Performance notes (all validated against the TimelineSim cost model and HW):
# - All matmul operands are bf16 (weights converted on the host): 1 PE
#   cycle/row at every size vs 4 for fp32, and half the weight DMA.  The
#   residual stream h is bf16 as well; PSUM accumulation stays fp32.
#   Measured HW rel err 7.8e-3 vs the 2e-2 gate.
# - Engines execute their queues in emission order, so emission IS the
#   schedule: attention is emitted as a two-stage software pipeline
#   (scores+exp of unit i before denominators/AV of unit i-lag) so the PE
#   never head-blocks on an Act exp round trip.
# - Softmax and LN partition-broadcasts run on the otherwise-idle GPSIMD
#   engine (partition_broadcast); LN z = h*r + psi reads all-bf16 SBUF
#   operands, hitting the DVE 2x/4x 16-bit modes.
# - Cross-attention K/V depend only on the 16 action tokens and are emitted
#   in PE gaps at the self-attention tail.
# - PSUM pool depth 8 (all banks), weight pool depth 5, with emission order
#   arranged so pool-rotation WAR edges always point forward (no deadlock).
# - The PE clock ramps with continuous busy time (p-state): idle gaps during
#   LN row chains would hand the next projection a half-speed PE, so bursts
#   of dependency-free burner matmuls into a dead PSUM row (_pe_warm) bridge
#   those gaps.  Sim-tuned to 24 matmuls per site; worth ~60us end to end.
import sys

sys.path.insert(0, "/opt/trn_rl_repo")

import numpy as np

import concourse.bacc as bacc
import concourse.mybir as mybir
import concourse.tile as tile
from concourse.bass_utils import run_bass_kernel_spmd

F32 = mybir.dt.float32
F32R = mybir.dt.float32r
BF16 = mybir.dt.bfloat16
AF = mybir.ActivationFunctionType
ALU = mybir.AluOpType


def _mm(nc, out, lhsT, rhs, **kw):
    """Plain matmul; operand dtypes come from the tiles (all bf16 here:
    1 PE cycle/row at any size, arbitrary dst partition, fp32 PSUM)."""
    nc.tensor.matmul(out, lhsT, rhs, **kw)

B, CIN, F, IMG, P = 2, 3, 16, 64, 4
PD = IMG // P          # 16 patches per side
NP = PD * PD           # 256 patches per frame
C, NH = 512, 8
HD = C // NH           # 64
S = 16                 # action context length
SCALE = float(1.0 / np.sqrt(HD))
M = 1024               # tokens per core (4 frames)
CT = 4                 # c tiles of 128
EPS = 1e-5

_BUILD_CACHE = {}
_PHASE_MARKS = []


def _pe_warm(d, pools, n):
    """n dependency-free back-to-back matmuls into a scratch PSUM tile.

    The PE clock ramps (p-state): ~2.4ns/row after 3us of continuous busy,
    2x slower otherwise.  LN row chains leave the PE idle long enough to
    reset the ramp, so the projections that follow run at half clock.  These
    burner matmuls execute during the (otherwise idle) LN tail and hand the
    following projection a warm PE.  They write a dead PSUM tile and depend
    only on constants, so they never delay real work ahead of them in the
    queue -- only the slight risk of outliving the LN chain."""
    nc = d.nc
    ps = pools.ps.tile([128, 512], F32, name="warm_ps", tag="ps")
    for _ in range(n):
        _mm(nc, ps[0:1, :], d.ones_col_b, d.warm_sb, start=True, stop=True)


def _mark(d, name):
    """Record (phase, instruction counter) for trace attribution; consumes
    one instruction name (gaps are harmless)."""
    n = int(d.nc.get_next_instruction_name().split("-")[1])
    _PHASE_MARKS.append((name, n))


# ---------------------------------------------------------------- host prep

def _fold_block(p, w):
    """Fold LN gamma/beta into the consuming projections for block prefix p.

    Returns dict of device arrays for this block."""
    g, b = w[p + "_ln_g"], w[p + "_ln_b"]          # [3, C]
    qkv1, wo1, bo1 = w[p + "_qkv1"], w[p + "_wo1"], w[p + "_bo1"]
    qkv2, wo2, bo2 = w[p + "_qkv2"], w[p + "_wo2"], w[p + "_bo2"]
    f1w, f1b = w[p + "_fc1w"], w[p + "_fc1b"]
    f2w, f2b = w[p + "_fc2w"], w[p + "_fc2b"]
    d = {}
    # self-attn: q (scaled by 1/sqrt(HD)), k, v from LN1(h)
    d["wq1"] = (g[0][:, None] * qkv1[0]) * SCALE
    d["bq1"] = (b[0] @ qkv1[0]) * SCALE
    d["wk1"] = g[0][:, None] * qkv1[1]
    d["bk1"] = b[0] @ qkv1[1]
    d["wv1"] = g[0][:, None] * qkv1[2]
    d["bv1"] = b[0] @ qkv1[2]
    d["wo1"], d["bo1"] = wo1, bo1
    # cross-attn: q from LN2(h) (scaled); k,v from raw ctx (no LN, no bias)
    d["wq2"] = (g[1][:, None] * qkv2[0]) * SCALE
    d["bq2"] = (b[1] @ qkv2[0]) * SCALE
    d["wk2"] = qkv2[1]
    d["wv2"] = qkv2[2]
    d["wo2"], d["bo2"] = wo2, bo2
    # mlp from LN3(h)
    d["wf1"] = g[2][:, None] * f1w
    d["bf1"] = b[2] @ f1w + f1b
    d["wf2"], d["bf2"] = f2w, f2b
    return d


def _w4(a):
    """[512, N] k-major weight -> [4, 128, N] (k_tile, k_in_tile, N), bf16."""
    import ml_dtypes
    return np.ascontiguousarray(
        np.asarray(a).reshape(4, 128, -1).astype(ml_dtypes.bfloat16))


def _colscal(a):
    """[512] bias -> [128, 4] per-partition scalar layout (col = c_tile)."""
    return np.ascontiguousarray(a.reshape(4, 128).T.astype(np.float32))


def _mprime_index():
    """m' (block-g window-major order) -> m (frame-major order)."""
    a, bb, f, i, j = np.meshgrid(
        np.arange(4), np.arange(4), np.arange(4), np.arange(4), np.arange(4),
        indexing="ij",
    )
    return (f * 256 + (4 * a + i) * 16 + 4 * bb + j).reshape(-1)


def host_prep(inputs):
    """Full inputs -> (shared weight map, list of 8 per-core input maps)."""
    w = {k: np.asarray(v) for k, v in inputs.items()}
    x = w["x"].astype(np.float32)                  # (B, CIN, F, IMG, IMG)
    actions = np.asarray(w["actions"])             # (B, S) int

    import ml_dtypes
    bf = ml_dtypes.bfloat16
    shared = {}
    shared["wp"] = np.ascontiguousarray(
        w["conv_w"].reshape(C, CIN * P * P).T.astype(bf))           # [48, 512]

    shared["wh"] = _w4(w["head_w"])                                 # [4,128,48]
    shared["bh"] = np.ascontiguousarray(
        w["head_b"].reshape(1, 48).astype(bf))
    for p in ("s", "g"):
        fb = _fold_block(p, w)
        for nm in ("wq1", "wk1", "wv1", "wo1", "wq2", "wk2", "wv2", "wo2",
                   "wf1", "wf2"):
            shared[p + nm] = _w4(fb[nm])
        for nm in ("bq1", "bk1", "bo1", "bq2", "bo2", "bf1", "bf2"):
            shared[p + nm] = _colscal(fb[nm])
        # v biases enter via a K=1 matmul row (token-major output)
        shared[p + "bv1r"] = np.ascontiguousarray(
            fb["bv1"].reshape(1, C).astype(bf))

    per_core = []
    for c in range(8):
        b, t = c // 4, c % 4
        m = dict(shared)
        xs = x[b, :, 4 * t:4 * t + 4]              # (CIN, 4, IMG, IMG)
        xs = xs.reshape(CIN, 4, PD, P, PD, P)
        xs = xs.transpose(0, 3, 5, 1, 2, 4)        # (c, p, q, f, ph, pw)
        m["xpt"] = np.ascontiguousarray(
            xs.reshape(CIN * P * P, M).astype(ml_dtypes.bfloat16))  # [48, 1024]
        # conv_b + t_pos + sp_pos folded into one additive tensor in the
        # core's token order (m = f*256 + patch), feature-major
        pe = (w["conv_b"][None, None, :]
              + w["t_pos"][4 * t:4 * t + 4][:, None, :]
              + w["sp_pos"][None, :, :])                   # (4, NP, C)
        m["pef"] = np.ascontiguousarray(
            pe.reshape(M, C).T.reshape(4, 128, M)
            .astype(ml_dtypes.bfloat16))                            # [4,128,M]
        ctx = (w["act_table"][actions[b]] + w["act_pos"][0]).astype(np.float32)
        m["ctxt"] = np.ascontiguousarray(
            ctx.T.reshape(4, 128, S).astype(ml_dtypes.bfloat16))    # [4,128,16]
        per_core.append(m)
    return per_core


# ---------------------------------------------------------------- device build

class Dev:
    """Holds nc handles used across the builder functions."""


def _declare_inputs(nc):
    d = Dev()
    d.nc = nc
    mk = lambda name, shape, dt=F32: nc.dram_tensor(name, shape, dt,
                                                    kind="ExternalInput")
    d.xpt = mk("xpt", [48, M], BF16)
    d.pef = mk("pef", [4, 128, M], BF16)
    d.ctxt = mk("ctxt", [4, 128, S], BF16)
    d.wp = mk("wp", [48, C], BF16)
    d.wh = mk("wh", [4, 128, 48], BF16)
    d.bh = mk("bh", [1, 48], BF16)
    for p in ("s", "g"):
        for nm in ("wq1", "wk1", "wv1", "wo1", "wq2", "wk2", "wv2", "wo2",
                   "wf1", "wf2"):
            setattr(d, p + nm, mk(p + nm, [4, 128, C], BF16))
        for nm in ("bq1", "bk1", "bo1", "bq2", "bo2", "bf1", "bf2"):
            setattr(d, p + nm, mk(p + nm, [128, 4]))
        setattr(d, p + "bv1r", mk(p + "bv1r", [1, C], BF16))
    d.out = nc.dram_tensor("out", [48, M], F32, kind="ExternalOutput")
    return d


def _load_w(d, pools, dram, name):
    """DMA a [4,128,N] weight into sbuf [128, 4, N]."""
    nc = d.nc
    n = dram.shape[2]
    t = pools.wpool.tile([128, 4, n], dram.dtype, name=name, tag="w" + str(n))
    nc.sync.dma_start(out=t, in_=dram.ap().transpose([1, 0, 2]))
    return t


def _load_small(d, pools, dram, name):
    nc = d.nc
    shape = list(dram.shape)
    if len(shape) == 3:
        # [4, 128, N] dram -> [128, 4, N] sbuf (partition-major)
        t = pools.cpool.tile([shape[1], shape[0], shape[2]], dram.dtype,
                             name=name)
        nc.sync.dma_start(out=t, in_=dram.ap().transpose([1, 0, 2]))
    else:
        t = pools.cpool.tile(shape, dram.dtype, name=name)
        nc.sync.dma_start(out=t, in_=dram.ap())
    return t


def _linear_fm_ms(d, pools, w_sb, x_sb, epilogue, ms):
    """Feature-major linear for one token half ms: for each ct produce psum
    [128,512] = sum_kt w_sb[:,kt,ct*128:+128].T @ x_sb[:,kt,ms*512:+512],
    then call epilogue(ct, ms, psum)."""
    nc = d.nc
    for ct in range(CT):
        ps = pools.ps.tile([128, 512], F32, name="lin_ps", tag="ps")
        for kt in range(CT):
            _mm(nc,
                ps,
                w_sb[:, kt, ct * 128:(ct + 1) * 128],
                x_sb[:, kt, ms * 512:(ms + 1) * 512],
                start=(kt == 0), stop=(kt == 3),
            )
        epilogue(ct, ms, ps)


def _proj_act_ms(d, pools, w_sb, b_sb, x_sb, out_sb, ms):
    """Linear + per-c_out bias via ACT Identity, into feature-major out."""
    nc = d.nc

    def ep(ct, ms_, ps):
        nc.scalar.activation(
            out=out_sb[:, ct, ms_ * 512:(ms_ + 1) * 512], in_=ps,
            func=AF.Identity, bias=b_sb[:, ct:ct + 1], scale=1.0)

    _linear_fm_ms(d, pools, w_sb, x_sb, ep, ms)


def _proj_residual_ms(d, pools, w_sb, b_sb, x_sb, h_sb, ms):
    """h += x @ W + b  (one fused DVE op per tile)."""
    nc = d.nc

    def ep(ct, ms_, ps):
        sl = h_sb[:, ct, ms_ * 512:(ms_ + 1) * 512]
        nc.vector.scalar_tensor_tensor(
            out=sl, in0=ps, scalar=b_sb[:, ct:ct + 1], in1=sl,
            op0=ALU.add, op1=ALU.add)

    _linear_fm_ms(d, pools, w_sb, x_sb, ep, ms)


def _layernorm_ms(d, pools, h_sb, z_sb, tag, ms):
    """LN for one token half; kept for pipelined call sites."""
    _layernorm_emit(d, pools, h_sb, z_sb, tag, (ms,))


def _layernorm(d, pools, h_sb, z_sb, tag):
    _layernorm_emit(d, pools, h_sb, z_sb, tag, (0, 1))


def _layernorm_emit(d, pools, h_sb, z_sb, tag, mss):
    """z = (h - mean_c(h)) * rsqrt(var_c(h) + eps), feature-major.

    h stays fp32 (residual precision), so its column-sum matmul runs fp32;
    the squared-sum runs bf16 off the bf16 Square output.  Emission is
    batched stage-by-stage across the token halves so each engine gets both
    halves' work back-to-back; the broadcast PSUM tiles are consumed
    directly by the z DVE ops (one PSUM operand per DVE op)."""
    nc = d.nc
    if not hasattr(d, "_ln_sq"):
        d._ln_sq = {}
        d._ln_rows = {}
    if tag not in d._ln_sq or mss[0] == 0:
        d._ln_sq[tag] = pools.state.tile([128, CT, M], BF16, name="lnsq",
                                         tag="scratch16")
        d._ln_rows[tag] = pools.rows.tile([1, 4, M], BF16,
                                          name="lnrows_" + tag,
                                          tag="lnrows", bufs=1)
    sq = d._ln_sq[tag]
    rows = d._ln_rows[tag]
    MU, A, R, PSI = range(4)
    sl = lambda ms: slice(ms * 512, (ms + 1) * 512)
    mps = {}
    for ms in mss:
        ps = pools.ps.tile([128, 512], F32, name="ln_mps", tag="ps")
        for kt in range(CT):
            _mm(nc, ps[0:1, :], d.ones_col_b, h_sb[:, kt, sl(ms)],
                start=(kt == 0), stop=(kt == 3))
        mps[ms] = ps
    for ms in mss:
        nc.scalar.activation(out=sq[:, :, sl(ms)], in_=h_sb[:, :, sl(ms)],
                             func=AF.Square)
    vps = {}
    for ms in mss:
        nc.vector.tensor_scalar_mul(rows[:, MU, sl(ms)], mps[ms][0:1, :],
                                    1.0 / C)
        ps = pools.ps.tile([128, 512], F32, name="ln_vps", tag="ps")
        for kt in range(CT):
            _mm(nc, ps[0:1, :], d.ones_col_b, sq[:, kt, sl(ms)],
                start=(kt == 0), stop=(kt == 3))
        vps[ms] = ps
    for ms in mss:
        msl = sl(ms)
        nc.vector.tensor_scalar_mul(rows[:, A, msl], vps[ms][0:1, :], 1.0 / C)
    # var = A - mu^2 ; r = 1/sqrt(var + eps) ; psi = -mu * r
    # (kept per-half: z(ms0) must land as early as possible so the next
    # projection's ms0 matmuls can start while the ms1 chain still runs)
    for msl in [sl(ms) for ms in mss]:
        nc.vector.tensor_mul(rows[:, R, msl], rows[:, MU, msl],
                             rows[:, MU, msl])
        nc.vector.tensor_sub(rows[:, A, msl], rows[:, A, msl],
                             rows[:, R, msl])
        nc.scalar.activation(out=rows[:, A, msl], in_=rows[:, A, msl],
                             func=AF.Sqrt, bias=d.eps_row)
        nc.vector.reciprocal(rows[:, R, msl], rows[:, A, msl])
        nc.vector.scalar_tensor_tensor(
            out=rows[:, PSI, msl], in0=rows[:, MU, msl], scalar=-1.0,
            in1=rows[:, R, msl], op0=ALU.mult, op1=ALU.mult)
    # broadcast r and psi across partitions on the idle GPSIMD engine;
    # all-bf16 SBUF operands put the big z DVE ops in the 2x/4x perf modes
    for ms in mss:
        msl = sl(ms)
        rbs = pools.epool.tile([128, 2, 512], BF16, name="lnb", tag="lnb")
        nc.gpsimd.partition_broadcast(rbs[:, 0, :], rows[:, R, msl])
        nc.gpsimd.partition_broadcast(rbs[:, 1, :], rows[:, PSI, msl])
        zv = z_sb[:, :, msl]
        rv = rbs[:, 0, :].unsqueeze(1).broadcast_to([128, CT, 512])
        pv = rbs[:, 1, :].unsqueeze(1).broadcast_to([128, CT, 512])
        nc.vector.tensor_mul(zv, h_sb[:, :, msl], rv)
        nc.vector.tensor_add(zv, zv, pv)


def _v_proj_ms(d, pools, w_sb, brow_sb, x_sb, v_sb, ms):
    """Token-major value projection for token half ms: v_sb[128 tok, mt, C]
    with bias via an extra K=1 matmul against the bias row."""
    nc = d.nc
    for mt in range(4 * ms, 4 * ms + 4):
        ps = pools.ps.tile([128, 512], F32, name="v_ps", tag="ps")
        for kt in range(CT):
            _mm(nc,
                ps, x_sb[:, kt, mt * 128:(mt + 1) * 128],
                w_sb[:, kt, :], start=(kt == 0),
                stop=(kt == 3 and brow_sb is None))
        if brow_sb is not None:
            _mm(nc, ps, d.ones_row_b[:, 0:128], brow_sb,
                             start=False, stop=True)
        nc.scalar.copy(out=v_sb[:, mt, :], in_=ps)


def _attn_s_stage1(d, pools, w, pair):
    """Scores + exp for heads (2*pair, 2*pair+1) of window w."""
    nc = d.nc
    out = {}
    for ho in range(2):
        h = 2 * pair + ho
        hp, ct = 64 * (h % 2), h // 2
        krows = slice(hp, hp + 64)
        sc = pools.ps.tile([128, 2, 256], F32, name="sc", tag="ps")
        for st in range(2):
            _mm(nc,
                sc[:, st, :],
                d.skt_sb[krows, ct, w * 256 + st * 128:
                         w * 256 + st * 128 + 128],
                d.sqt_sb[krows, ct, w * 256:w * 256 + 256],
                start=True, stop=True)
        ex = pools.epool.tile([128, 2, 256], BF16, name="ex", tag="ex")
        nc.scalar.activation(out=ex, in_=sc, func=AF.Exp)
        out[ho] = ex
    return out


def _attn_s_stage2(d, pools, v_sb, attn_sb, w, pair, exs):
    """dn/ob matmuls, reciprocal, broadcast and normalize for one pair."""
    nc = d.nc
    dn = pools.ps.tile([128, 512], F32, name="dn", tag="ps")
    obs = {}
    for ho in range(2):
        h = 2 * pair + ho
        hp = 64 * (h % 2)
        ob = pools.ps.tile([128, 512], F32, name="ob", tag="ps")
        obs[ho] = ob
        for st in range(2):
            _mm(nc, dn[0:1, 256 * ho:256 * ho + 256], d.ones_col_b,
                exs[ho][:, st, :], start=(st == 0), stop=(st == 1),
                tile_position=(0, 0))
            _mm(nc,
                ob[hp:hp + 64, 0:256],
                v_sb[:, 2 * w + st, h * 64:h * 64 + 64],
                exs[ho][:, st, :], start=(st == 0), stop=(st == 1),
                tile_position=(0, hp))
    rrow = pools.rows.tile([1, 512], F32, name="rrow", tag="rr")
    nc.vector.reciprocal(rrow, dn[0:1, :])
    rbs = pools.epool.tile([128, 2, 256], F32, name="rbs", tag="rbs")
    for ho in range(2):
        h = 2 * pair + ho
        hp, ct = 64 * (h % 2), h // 2
        krows = slice(hp, hp + 64)
        nc.gpsimd.partition_broadcast(rbs[:, ho, :],
                                      rrow[:, 256 * ho:256 * ho + 256])
        nc.vector.tensor_mul(
            attn_sb[krows, ct, w * 256:w * 256 + 256],
            obs[ho][hp:hp + 64, 0:256], rbs[hp:hp + 64, ho, :])


def _attn_s(d, pools, v_sb, attn_sb, lag=1):
    """Block-s self attention, software-pipelined: stage1 (scores+exp) of
    unit i is emitted before stage2 of unit i-lag, so the in-order PE queue
    always has ready score matmuls ahead of denominator matmuls that wait
    on the Act exp round-trip."""
    units = [(w, pair) for w in range(4) for pair in range(4)]
    pend = {}
    for i in range(len(units) + lag):
        if i < len(units):
            w, pair = units[i]
            pend[i] = _attn_s_stage1(d, pools, w, pair)
        j = i - lag
        if j >= 0:
            w, pair = units[j]
            _attn_s_stage2(d, pools, v_sb, attn_sb, w, pair, pend.pop(j))


def _attn_g_stage1(d, pools, wp):
    """Scores + exp for one window-pair of block-g self attention.

    PSUM hazard rule (HW-verified): within one bank, concurrent matmuls with
    different row groups but overlapping column strips fault.  So scores are
    banked by head parity (source rows fixed per bank) and o/denominator by
    window parity (rows = 64j fixed per bank)."""
    nc = d.nc
    scs = [pools.ps.tile([128, 4, 64], F32, name=f"sc_g{p}", tag="ps")
           for p in range(2)]
    for h in range(NH):
        p, ct = h % 2, h // 2
        hp = 64 * p
        for j in range(2):
            w = 2 * wp + j
            _mm(nc,
                scs[p][64 * j:64 * j + 64, ct, :],
                d.gkt_sb[hp:hp + 64, ct, w * 64:w * 64 + 64],
                d.gqt_sb[hp:hp + 64, ct, w * 64:w * 64 + 64],
                start=True, stop=True, tile_position=(hp, 64 * j))
    exs = []
    for p in range(2):
        ex = pools.epool.tile([128, 4, 64], BF16, name=f"ex_g{p}",
                              tag="ex")
        nc.scalar.activation(out=ex, in_=scs[p], func=AF.Exp)
        exs.append(ex)
    return exs


def _attn_g_stage2(d, pools, v_sb, attn_sb, wp, exs):
    nc = d.nc
    ogs = [pools.ps.tile([128, 4, 64], F32, name=f"og{j}", tag="ps")
           for j in range(2)]
    dns = [pools.ps.tile([128, 512], F32, name=f"dn_g{j}", tag="ps")
           for j in range(2)]
    for h in range(NH):
        p, ct = h % 2, h // 2
        for j in range(2):
            jr = slice(64 * j, 64 * j + 64)
            _mm(nc,
                dns[j][0:1, h * 64:h * 64 + 64],
                d.ones_col_b[jr, :], exs[p][jr, ct, :],
                start=True, stop=True, tile_position=(64 * j, 0))
            _mm(nc,
                ogs[j][64 * p:64 * p + 64, ct, :],
                v_sb[jr, wp, h * 64:h * 64 + 64],
                exs[p][jr, ct, :],
                start=True, stop=True, tile_position=(64 * j, 64 * p))
    for j in range(2):
        rrow = pools.rows.tile([1, 512], F32, name=f"rr_g{j}", tag="rr")
        nc.vector.reciprocal(rrow, dns[j][0:1, :])
        rbs = pools.epool.tile([128, 512], F32, name=f"rbs_g{j}",
                               tag="rbs")
        nc.gpsimd.partition_broadcast(rbs, rrow)
        for p in range(2):
            pr = slice(64 * p, 64 * p + 64)
            out_view = attn_sb[pr, :, (2 * wp + j) * 64:
                               (2 * wp + j) * 64 + 64]
            in1 = rbs[pr, :].rearrange(
                "q (ct two l) -> q ct two l", two=2, l=64)[:, :, p, :]
            nc.vector.tensor_mul(out_view, ogs[j][pr, :, :], in1)


def _attn_g(d, pools, v_sb, attn_sb, lag=1):
    pend = {}
    for i in range(8 + lag):
        if i < 8:
            pend[i] = _attn_g_stage1(d, pools, i)
        j = i - lag
        if j >= 0:
            _attn_g_stage2(d, pools, v_sb, attn_sb, j, pend.pop(j))


def _ctx_kv(d, pools, prefix, wk, wv):
    """Cross-attention K/V from the 16 action-context tokens.  Independent
    of the token stream, so emitted during the LN1 row chain to stay off
    the critical path.  kT [128, 4, 16] feature-major; V [16, C] token-
    major, replicated to the four 32-partition strips."""
    nc = d.nc
    ktc = pools.state.tile([128, 4, S], BF16, name=prefix + "ktc",
                           tag=prefix + "ktc")
    for ct in range(CT):
        ps = pools.ps.tile([128, 512], F32, name="ktc_ps", tag="ps")
        for kt in range(CT):
            _mm(nc, ps[:, 0:S], wk[:, kt, ct * 128:(ct + 1) * 128],
                             d.ctxt_sb[:, kt, :], start=(kt == 0),
                             stop=(kt == 3))
        nc.scalar.copy(out=ktc[:, ct, :], in_=ps[:, 0:S])
    vc = pools.state.tile([128, 8, 64], BF16, name=prefix + "vc",
                          tag=prefix + "vc")
    ps = pools.ps.tile([128, 512], F32, name="vc_ps", tag="ps")
    for kt in range(CT):
        _mm(nc, ps[0:S, :], d.ctxt_sb[:, kt, :], wv[:, kt, :],
                         start=(kt == 0), stop=(kt == 3))
    vcv = vc.rearrange("p h d -> p (h d)")
    nc.scalar.copy(out=vcv[0:S, :], in_=ps[0:S, :])
    for q in range(1, 4):
        nc.sync.dma_start(out=vcv[32 * q:32 * q + S, :], in_=vcv[0:S, :])
    setattr(d, prefix + "ktc_sb", ktc)
    setattr(d, prefix + "vc_sb", vc)


def _xattn_s1(d, pools, prefix, ms):
    """Cross attention scores + exp for token half ms."""
    nc = d.nc
    qt = getattr(d, prefix + "qt2_sb")
    ktc = getattr(d, prefix + "ktc_sb")
    msl = slice(ms * 512, (ms + 1) * 512)
    exs = []
    for grp in range(2):                       # heads 4*grp .. 4*grp+3
        # no memset: rows outside the four 16-row strips are garbage but
        # are never read (dn/ob matmuls slice sr exactly)
        sc = pools.ps.tile([128, 512], F32, name="sc_x", tag="ps")
        for hh in range(4):
            h = 4 * grp + hh
            hp, ct = 64 * (h % 2), h // 2
            _mm(nc,
                sc[32 * hh:32 * hh + S, :],
                ktc[64 * (h % 2):64 * (h % 2) + 64, ct, :],
                qt[64 * (h % 2):64 * (h % 2) + 64, ct, msl],
                start=True, stop=True,
                tile_position=(hp, 32 * hh))
        ex = pools.epool.tile([128, 512], BF16, name="ex_x", tag="ex")
        nc.scalar.activation(out=ex, in_=sc, func=AF.Exp)
        exs.append(ex)
    setattr(d, prefix + "_xexs%d" % ms, exs)


def _xattn_s2(d, pools, prefix, attn_sb, ms):
    """Cross attention denominators/AV/normalize for token half ms."""
    nc = d.nc
    vc = getattr(d, prefix + "vc_sb")
    exs = getattr(d, prefix + "_xexs%d" % ms)
    msl = slice(ms * 512, (ms + 1) * 512)
    for h in range(NH):
        grp, hh = h // 4, h % 4
        sr = slice(32 * hh, 32 * hh + S)
        hp, ct = 64 * (h % 2), h // 2
        dn = pools.ps.tile([128, 512], F32, name="dn_x", tag="ps")
        _mm(nc, dn[0:1, :], d.ones_col_b[sr, :], exs[grp][sr, :],
                         start=True, stop=True,
                         tile_position=(32 * hh, 0))
        ob = pools.ps.tile([128, 512], F32, name="ob_x", tag="ps")
        _mm(nc, ob[hp:hp + 64, :], vc[sr, h, :],
                         exs[grp][sr, :], start=True, stop=True,
                         tile_position=(32 * hh, hp))
        rrow = pools.rows.tile([1, 512], F32, name="rr_x", tag="rr")
        nc.vector.reciprocal(rrow, dn[0:1, :])
        rbs = pools.epool.tile([128, 512], F32, name="rbs_x", tag="rbs")
        nc.gpsimd.partition_broadcast(rbs, rrow)
        nc.vector.tensor_mul(attn_sb[64 * (h % 2):64 * (h % 2) + 64,
                                     ct, msl],
                             ob[hp:hp + 64, :], rbs[hp:hp + 64, :])


def _block(d, pools, prefix, h_sb):
    """One transformer block, software-pipelined at token-half (ms)
    granularity: the second half of each primitive overlaps the first half
    of its consumer, and the self-attention windows for ms0 run while the
    ms1 projections are still in flight."""
    nc = d.nc
    z = pools.state.tile([128, CT, M], BF16, name="z", tag="zbuf")
    attn = pools.state.tile([128, CT, M], BF16, name="attn", tag="attnbuf")
    qt = pools.state.tile([128, CT, M], BF16, name=prefix + "qt", tag="qbuf")
    kt_ = pools.state.tile([128, CT, M], BF16, name=prefix + "kt", tag="kbuf")
    v_sb = pools.state.tile([128, 8, C], BF16, name=prefix + "v", tag="vbuf")
    setattr(d, prefix + "qt_sb", qt)
    setattr(d, prefix + "kt_sb", kt_)

    wk2 = _load_w(d, pools, getattr(d, prefix + "wk2"), "wk2")
    wv2 = _load_w(d, pools, getattr(d, prefix + "wv2"), "wv2")
    wq1 = _load_w(d, pools, getattr(d, prefix + "wq1"), "wq1")
    wk1 = _load_w(d, pools, getattr(d, prefix + "wk1"), "wk1")
    wv1 = _load_w(d, pools, getattr(d, prefix + "wv1"), "wv1")
    bq1 = getattr(d, prefix + "bq1_sb")
    bk1 = getattr(d, prefix + "bk1_sb")
    bv1r = getattr(d, prefix + "bv1r_sb")

    # ---- LN1 + qkv + self attention, pipelined over ms
    _mark(d, prefix + ":ln1")
    _layernorm(d, pools, h_sb, z, prefix + "1")
    # context K/V: independent of the token stream; emitted here so its
    # matmuls fill the PE gap during the LN row chain and keep the PE
    # p-state warm into the projections
    _pe_warm(d, pools, 24)
    _mark(d, prefix + ":qkproj")
    _proj_act_ms(d, pools, wq1, bq1, z, qt, 0)
    _proj_act_ms(d, pools, wk1, bk1, z, kt_, 0)
    _v_proj_ms(d, pools, wv1, bv1r, z, v_sb, 0)
    _proj_act_ms(d, pools, wq1, bq1, z, qt, 1)
    _proj_act_ms(d, pools, wk1, bk1, z, kt_, 1)
    _v_proj_ms(d, pools, wv1, bv1r, z, v_sb, 1)
    _mark(d, prefix + ":selfattn")
    if prefix == "s":
        _attn_s(d, pools, v_sb, attn)
    else:
        _attn_g(d, pools, v_sb, attn)

    # context K/V: independent of the token stream; emitted here so its
    # matmuls fill PE gaps at the softmax tail (weights DMA'd long before)
    _ctx_kv(d, pools, prefix, wk2, wv2)


    # ---- o-projection (+res), LN2, q2, pipelined
    wo1 = _load_w(d, pools, getattr(d, prefix + "wo1"), "wo1")
    bo1 = getattr(d, prefix + "bo1_sb")
    _mark(d, prefix + ":oproj1")
    _proj_residual_ms(d, pools, wo1, bo1, attn, h_sb, 0)
    _proj_residual_ms(d, pools, wo1, bo1, attn, h_sb, 1)
    _mark(d, prefix + ":ln2")
    _layernorm(d, pools, h_sb, z, prefix + "2")
    wq2 = _load_w(d, pools, getattr(d, prefix + "wq2"), "wq2")
    bq2 = getattr(d, prefix + "bq2_sb")
    qt2 = pools.state.tile([128, CT, M], BF16, name=prefix + "qt2",
                           tag="kbuf")
    setattr(d, prefix + "qt2_sb", qt2)
    _pe_warm(d, pools, 24)
    _mark(d, prefix + ":q2proj")
    _proj_act_ms(d, pools, wq2, bq2, z, qt2, 0)
    _mark(d, prefix + ":xattn")
    _xattn_s1(d, pools, prefix, 0)
    _proj_act_ms(d, pools, wq2, bq2, z, qt2, 1)
    _xattn_s2(d, pools, prefix, attn, 0)
    wo2 = _load_w(d, pools, getattr(d, prefix + "wo2"), "wo2")
    bo2 = getattr(d, prefix + "bo2_sb")
    _xattn_s1(d, pools, prefix, 1)
    _mark(d, prefix + ":oproj2")
    _proj_residual_ms(d, pools, wo2, bo2, attn, h_sb, 0)
    _xattn_s2(d, pools, prefix, attn, 1)
    _proj_residual_ms(d, pools, wo2, bo2, attn, h_sb, 1)

    # ---- mlp, pipelined
    wf1 = _load_w(d, pools, getattr(d, prefix + "wf1"), "wf1")
    wf2 = _load_w(d, pools, getattr(d, prefix + "wf2"), "wf2")
    bf1 = getattr(d, prefix + "bf1_sb")
    bf2 = getattr(d, prefix + "bf2_sb")
    _mark(d, prefix + ":ln3")
    _layernorm(d, pools, h_sb, z, prefix + "3")
    gbuf = pools.state.tile([128, CT, M], BF16, name=prefix + "g",
                            tag="gbuf")

    def ep_gelu(ct, ms, ps):
        nc.scalar.activation(
            out=gbuf[:, ct, ms * 512:(ms + 1) * 512], in_=ps, func=AF.Gelu,
            bias=bf1[:, ct:ct + 1], scale=1.0)

    _pe_warm(d, pools, 24)
    _mark(d, prefix + ":mlp1")
    _linear_fm_ms(d, pools, wf1, z, ep_gelu, 0)
    _linear_fm_ms(d, pools, wf1, z, ep_gelu, 1)
    _mark(d, prefix + ":mlp2")
    _proj_residual_ms(d, pools, wf2, bf2, gbuf, h_sb, 0)
    _proj_residual_ms(d, pools, wf2, bf2, gbuf, h_sb, 1)


class Pools:
    pass


def _body(d, pools):
    """One full forward pass for this core's 1024 tokens."""
    nc = d.nc

    # constants and small tensors first (the weight DMAs are emitted lazily
    # at their use sites so the shared DMA queue can never head-of-line block
    # a load that an earlier weight's consumer depends on)
    xpt = pools.cpool.tile([48, M], BF16, name="xpt_sb")
    nc.sync.dma_start(out=xpt, in_=d.xpt.ap())
    wp = pools.cpool.tile([48, C], BF16, name="wp_sb")
    nc.sync.dma_start(out=wp, in_=d.wp.ap())
    d.ctxt_sb = _load_small(d, pools, d.ctxt, "ctxt")
    pev = _load_small(d, pools, d.pef, "pef")
    for p in ("s", "g"):
        for nm in ("bq1", "bk1", "bo1", "bq2", "bo2", "bf1", "bf2"):
            setattr(d, p + nm + "_sb",
                    _load_small(d, pools, getattr(d, p + nm), p + nm))
        setattr(d, p + "bv1r_sb",
                _load_small(d, pools, getattr(d, p + "bv1r"), p + "bv1r"))
    d.bh_sb = _load_small(d, pools, d.bh, "bh")

    h = pools.state.tile([128, CT, M], BF16, name="h", tag="hbuf")

    # ---- patch embedding + positional embeddings
    _mark(d, "embed")
    for ct in range(CT):
        for ms in range(2):
            msl = slice(ms * 512, (ms + 1) * 512)
            ps = pools.ps.tile([128, 512], F32, name="pe_ps", tag="ps")
            _mm(nc, ps, wp[:, ct * 128:(ct + 1) * 128],
                             xpt[:, msl], start=True, stop=True)
            # + (conv_b + t_pos + sp_pos), folded on the host
            nc.vector.tensor_add(h[:, ct, msl], ps, pev[:, ct, msl])

    # ---- block s (spatial windows = frames)
    _block(d, pools, "s", h)

    # ---- permute tokens to m' (window-major for the 4x4x4 windows)
    _mark(d, "permute")
    hg = pools.state.tile([128, CT, M], BF16, name="hg", tag="zbuf2")
    for ct in range(CT):
        for a in range(4):
            src = bass_view_perm(h, ct, a)
            nc.vector.tensor_copy(
                out=hg[:, ct, a * 256:(a + 1) * 256].rearrange(
                    "p (b f i j) -> p b f i j", b=4, f=4, i=4),
                in_=src)
    _block(d, pools, "g", hg)

    # ---- output head (m' order; host undoes the permutation)
    _mark(d, "head")
    d.wh_sb = _load_w(d, pools, d.wh, "wh")
    outT = pools.cpool.tile([48, M], F32, name="outT")
    for ms in range(2):
        msl = slice(ms * 512, (ms + 1) * 512)
        ps = pools.ps.tile([128, 512], F32, name="hd_ps", tag="ps")
        for kt in range(CT):
            _mm(nc, ps[0:48, :], d.wh_sb[:, kt, :], hg[:, kt, msl],
                             start=(kt == 0), stop=False)
        _mm(nc, ps[0:48, :], d.bh_sb, d.ones_row_b,
                         start=False, stop=True)
        nc.scalar.copy(out=outT[:, msl], in_=ps[0:48, :])
        nc.sync.dma_start(out=d.out.ap()[:, msl], in_=outT[:, msl])


def bass_view_perm(h, ct, a):
    """View of h[:, ct, :] selecting m' block a*256..a*256+255 in (b,f,i,j)
    nested order: m = f*256 + (4a+i)*16 + 4b + j."""
    base = h[:, ct, :]
    return bass_ap_nest(base, a)


def bass_ap_nest(base, a):
    import concourse.bass as bass  # noqa
    v = base.rearrange("p (f ph pw) -> p f ph pw", f=4, ph=16)
    # dims: f(stride 256), ph=4a+i (stride 16), pw=4b+j (stride 1)
    v2 = v[:, :, 4 * a:4 * a + 4, :].rearrange(
        "p f i (b j) -> p b f i j", b=4)
    return v2


def _build_nc(loop_n=1):
    key = loop_n
    if key in _BUILD_CACHE:
        return _BUILD_CACHE[key]
    nc = bacc.Bacc("TRN2", target_bir_lowering=False, debug=False,
                   num_devices=8)
    d = _declare_inputs(nc)
    with tile.TileContext(nc) as tc, nc.allow_low_precision(
            reason="f32r matmul operands; output tolerance is 2e-2"):
        pools = Pools()
        import contextlib
        stack = contextlib.ExitStack()
        with stack:
            pools.ps = stack.enter_context(
                tc.tile_pool(name="ps", bufs=8, space="PSUM"))
            pools.wpool = stack.enter_context(
                tc.tile_pool(name="wpool", bufs=5))
            pools.cpool = stack.enter_context(
                tc.tile_pool(name="cpool", bufs=1))
            pools.state = stack.enter_context(
                tc.tile_pool(name="state", bufs=1))
            pools.epool = stack.enter_context(
                tc.tile_pool(name="epool", bufs=6))
            pools.rows = stack.enter_context(
                tc.tile_pool(name="rows", bufs=4))
            ones_col = pools.cpool.tile([128, 1], F32, name="ones_col")
            nc.vector.memset(ones_col, 1.0)
            ones_row = pools.cpool.tile([1, 512], F32, name="ones_row")
            nc.vector.memset(ones_row, 1.0)
            d.ones_col = ones_col
            d.ones_row = ones_row
            ones_col_b = pools.cpool.tile([128, 1], BF16, name="ones_col_b")
            nc.vector.tensor_copy(out=ones_col_b, in_=ones_col)
            d.ones_col_b = ones_col_b
            ones_row_b = pools.cpool.tile([1, 512], BF16, name="ones_row_b")
            nc.vector.tensor_copy(out=ones_row_b, in_=ones_row)
            d.ones_row_b = ones_row_b
            warm_sb = pools.cpool.tile([128, 512], BF16, name="warm_sb")
            nc.vector.memset(warm_sb, 0.0)
            d.warm_sb = warm_sb
            eps_row = pools.cpool.tile([1, 1], F32, name="eps_row")
            nc.vector.memset(eps_row, EPS)
            d.eps_row = eps_row
            # python-level unroll for timing variants (a tc.For_i back
            # edge wedges the device here; the unrolled NEFF is equivalent)
            for _ in range(loop_n):
                _body(d, pools)
    nc.compile()
    _BUILD_CACHE[key] = nc
    return nc


# ---------------------------------------------------------------- entry point

_MPRIME = None


def kernel(**inputs) -> np.ndarray:
    global _MPRIME
    if _MPRIME is None:
        _MPRIME = _mprime_index()
    nc = _build_nc(1)
    in_maps = host_prep(inputs)
    res = run_bass_kernel_spmd(nc, in_maps, core_ids=list(range(8)))
    # reassemble: per-core outT [48, 1024] in m' order -> full output
    out = np.zeros((B, CIN, F, IMG, IMG), np.float32)
    for c in range(8):
        b, t = c // 4, c % 4
        ot = res.results[c]["out"]                 # [48, M] m'-order
        om = np.empty_like(ot)
        om[:, _MPRIME] = ot                        # -> m order
        # om[(p,q,cc), (f,ph,pw)] -> out[b, cc, 4t+f, ph*4+p, pw*4+q]
        om = om.reshape(P, P, CIN, 4, PD, PD)
        om = om.transpose(2, 3, 4, 0, 5, 1)        # (cc, f, ph, p, pw, q)
        out[b, :, 4 * t:4 * t + 4] = om.reshape(CIN, 4, IMG, IMG)
    return out


if __name__ == "__main__":
    import reference
    ins = reference.setup_inputs()
    ins = {k: np.asarray(v) for k, v in ins.items()}
    exp = np.asarray(reference.reference(**ins))
    act = kernel(**ins)
    err = np.abs(act - exp).max() / (np.abs(exp).max() + 1e-12)
    print("Relative error:", err)



# revision 68
# speedup vs baseline: 598.7557x; 1.0361x over previous
# Trainium2 Bass kernel for nn_DiT_89086211653924 (windowed-attention video DiT).
#
# Sharding (8 cores, zero collectives): core c -> (batch b = c//4, temporal
# group t = c%4) = frames [4t, 4t+4), i.e. 1024 of the 4096 tokens of batch b.
# Both the spatial windows (one frame = 256 contiguous tokens) and the
# spatio-temporal 4x4x4 windows (within one temporal group) are core-local, and
# the action-context cross-attention K/V is a per-batch constant, so each core
# runs the full two-block transformer on its 1024 tokens independently.
#
# On-device layout: activations are feature-major [c_in partition (128), c_tile
# (4), token (1024)] so every linear is matmul(lhsT=W[k_tile, c_out_slice],
# rhs=xT[k_tile, tokens]) with no transposes anywhere.  Value projections emit
# token-major V by using the activation tile as the stationary operand instead.
# LayerNorm gamma/beta are folded into the following matmul's weights+bias on
# the host.
#
# Performance notes (all validated against the TimelineSim cost model and HW):
# - All matmul operands are bf16 (weights converted on the host): 1 PE
#   cycle/row at every size vs 4 for fp32, and half the weight DMA.  The
#   residual stream h is bf16 as well; PSUM accumulation stays fp32.
#   Measured HW rel err 7.8e-3 vs the 2e-2 gate.
# - Engines execute their queues in emission order, so emission IS the
#   schedule: attention is emitted as a two-stage software pipeline
#   (scores+exp of unit i before denominators/AV of unit i-lag) so the PE
#   never head-blocks on an Act exp round trip.
# - Softmax and LN partition-broadcasts run on the otherwise-idle GPSIMD
#   engine (partition_broadcast); LN z = h*r + psi reads all-bf16 SBUF
#   operands, hitting the DVE 2x/4x 16-bit modes.
# - Cross-attention K/V depend only on the 16 action tokens and are emitted
#   in PE gaps at the self-attention tail.
# - PSUM pool depth 8 (all banks), weight pool depth 5, with emission order
#   arranged so pool-rotation WAR edges always point forward (no deadlock).
# - The PE clock ramps with continuous busy time (p-state): idle gaps during
#   LN row chains would hand the next projection a half-speed PE, so bursts
#   of dependency-free burner matmuls into a dead PSUM row (_pe_warm) bridge
#   those gaps.  Sim-tuned to 24 matmuls per site; worth ~60us end to end.
import sys

sys.path.insert(0, "/opt/trn_rl_repo")

import numpy as np

import concourse.bacc as bacc
import concourse.mybir as mybir
import concourse.tile as tile
from concourse.bass_utils import run_bass_kernel_spmd

F32 = mybir.dt.float32
F32R = mybir.dt.float32r
BF16 = mybir.dt.bfloat16
AF = mybir.ActivationFunctionType
ALU = mybir.AluOpType


def _mm(nc, out, lhsT, rhs, **kw):
    """Plain matmul; operand dtypes come from the tiles (all bf16 here:
    1 PE cycle/row at any size, arbitrary dst partition, fp32 PSUM)."""
    nc.tensor.matmul(out, lhsT, rhs, **kw)

B, CIN, F, IMG, P = 2, 3, 16, 64, 4
PD = IMG // P          # 16 patches per side
NP = PD * PD           # 256 patches per frame
C, NH = 512, 8
HD = C // NH           # 64
S = 16                 # action context length
SCALE = float(1.0 / np.sqrt(HD))
M = 1024               # tokens per core (4 frames)
CT = 4                 # c tiles of 128
EPS = 1e-5

_BUILD_CACHE = {}
_PHASE_MARKS = []


def _pe_warm(d, pools, n):
    """n dependency-free back-to-back matmuls into a scratch PSUM tile.

    The PE clock ramps (p-state): ~2.4ns/row after 3us of continuous busy,
    2x slower otherwise.  LN row chains leave the PE idle long enough to
    reset the ramp, so the projections that follow run at half clock.  These
    burner matmuls execute during the (otherwise idle) LN tail and hand the
    following projection a warm PE.  They write a dead PSUM tile and depend
    only on constants, so they never delay real work ahead of them in the
    queue -- only the slight risk of outliving the LN chain."""
    nc = d.nc
    ps = pools.ps.tile([128, 512], F32, name="warm_ps", tag="ps")
    for _ in range(n):
        _mm(nc, ps[0:1, :], d.ones_col_b, d.warm_sb, start=True, stop=True)


def _mark(d, name):
    """Record (phase, instruction counter) for trace attribution; consumes
    one instruction name (gaps are harmless)."""
    n = int(d.nc.get_next_instruction_name().split("-")[1])
    _PHASE_MARKS.append((name, n))


# ---------------------------------------------------------------- host prep

def _fold_block(p, w):
    """Fold LN gamma/beta into the consuming projections for block prefix p.

    Returns dict of device arrays for this block."""
    g, b = w[p + "_ln_g"], w[p + "_ln_b"]          # [3, C]
    qkv1, wo1, bo1 = w[p + "_qkv1"], w[p + "_wo1"], w[p + "_bo1"]
    qkv2, wo2, bo2 = w[p + "_qkv2"], w[p + "_wo2"], w[p + "_bo2"]
    f1w, f1b = w[p + "_fc1w"], w[p + "_fc1b"]
    f2w, f2b = w[p + "_fc2w"], w[p + "_fc2b"]
    d = {}
    # self-attn: q (scaled by 1/sqrt(HD)), k, v from LN1(h)
    d["wq1"] = (g[0][:, None] * qkv1[0]) * SCALE
    d["bq1"] = (b[0] @ qkv1[0]) * SCALE
    d["wk1"] = g[0][:, None] * qkv1[1]
    d["bk1"] = b[0] @ qkv1[1]
    d["wv1"] = g[0][:, None] * qkv1[2]
    d["bv1"] = b[0] @ qkv1[2]
    d["wo1"], d["bo1"] = wo1, bo1
    # cross-attn: q from LN2(h) (scaled); k,v from raw ctx (no LN, no bias)
    d["wq2"] = (g[1][:, None] * qkv2[0]) * SCALE
    d["bq2"] = (b[1] @ qkv2[0]) * SCALE
    d["wk2"] = qkv2[1]
    d["wv2"] = qkv2[2]
    d["wo2"], d["bo2"] = wo2, bo2
    # mlp from LN3(h)
    d["wf1"] = g[2][:, None] * f1w
    d["bf1"] = b[2] @ f1w + f1b
    d["wf2"], d["bf2"] = f2w, f2b
    return d


def _w4(a):
    """[512, N] k-major weight -> [4, 128, N] (k_tile, k_in_tile, N), bf16."""
    import ml_dtypes
    return np.ascontiguousarray(
        np.asarray(a).reshape(4, 128, -1).astype(ml_dtypes.bfloat16))


def _colscal(a):
    """[512] bias -> [128, 4] per-partition scalar layout (col = c_tile)."""
    return np.ascontiguousarray(a.reshape(4, 128).T.astype(np.float32))


def _mprime_index():
    """m' (block-g window-major order) -> m (frame-major order)."""
    a, bb, f, i, j = np.meshgrid(
        np.arange(4), np.arange(4), np.arange(4), np.arange(4), np.arange(4),
        indexing="ij",
    )
    return (f * 256 + (4 * a + i) * 16 + 4 * bb + j).reshape(-1)


def host_prep(inputs):
    """Full inputs -> (shared weight map, list of 8 per-core input maps)."""
    w = {k: np.asarray(v) for k, v in inputs.items()}
    x = w["x"].astype(np.float32)                  # (B, CIN, F, IMG, IMG)
    actions = np.asarray(w["actions"])             # (B, S) int

    import ml_dtypes
    bf = ml_dtypes.bfloat16
    shared = {}
    shared["wp"] = np.ascontiguousarray(
        w["conv_w"].reshape(C, CIN * P * P).T.astype(bf))           # [48, 512]

    shared["wh"] = _w4(w["head_w"])                                 # [4,128,48]
    shared["bh"] = np.ascontiguousarray(
        w["head_b"].reshape(1, 48).astype(bf))
    for p in ("s", "g"):
        fb = _fold_block(p, w)
        for nm in ("wq1", "wk1", "wv1", "wo1", "wq2", "wk2", "wv2", "wo2",
                   "wf1", "wf2"):
            shared[p + nm] = _w4(fb[nm])
        for nm in ("bq1", "bk1", "bo1", "bq2", "bo2", "bf1", "bf2"):
            shared[p + nm] = _colscal(fb[nm])
        # v biases enter via a K=1 matmul row (token-major output)
        shared[p + "bv1r"] = np.ascontiguousarray(
            fb["bv1"].reshape(1, C).astype(bf))

    per_core = []
    for c in range(8):
        b, t = c // 4, c % 4
        m = dict(shared)
        xs = x[b, :, 4 * t:4 * t + 4]              # (CIN, 4, IMG, IMG)
        xs = xs.reshape(CIN, 4, PD, P, PD, P)
        xs = xs.transpose(0, 3, 5, 1, 2, 4)        # (c, p, q, f, ph, pw)
        m["xpt"] = np.ascontiguousarray(
            xs.reshape(CIN * P * P, M).astype(ml_dtypes.bfloat16))  # [48, 1024]
        # conv_b + t_pos + sp_pos folded into one additive tensor in the
        # core's token order (m = f*256 + patch), feature-major
        pe = (w["conv_b"][None, None, :]
              + w["t_pos"][4 * t:4 * t + 4][:, None, :]
              + w["sp_pos"][None, :, :])                   # (4, NP, C)
        m["pef"] = np.ascontiguousarray(
            pe.reshape(M, C).T.reshape(4, 128, M)
            .astype(ml_dtypes.bfloat16))                            # [4,128,M]
        ctx = (w["act_table"][actions[b]] + w["act_pos"][0]).astype(np.float32)
        m["ctxt"] = np.ascontiguousarray(
            ctx.T.reshape(4, 128, S).astype(ml_dtypes.bfloat16))    # [4,128,16]
        per_core.append(m)
    return per_core


# ---------------------------------------------------------------- device build

class Dev:
    """Holds nc handles used across the builder functions."""


def _declare_inputs(nc):
    d = Dev()
    d.nc = nc
    mk = lambda name, shape, dt=F32: nc.dram_tensor(name, shape, dt,
                                                    kind="ExternalInput")
    d.xpt = mk("xpt", [48, M], BF16)
    d.pef = mk("pef", [4, 128, M], BF16)
    d.ctxt = mk("ctxt", [4, 128, S], BF16)
    d.wp = mk("wp", [48, C], BF16)
    d.wh = mk("wh", [4, 128, 48], BF16)
    d.bh = mk("bh", [1, 48], BF16)
    for p in ("s", "g"):
        for nm in ("wq1", "wk1", "wv1", "wo1", "wq2", "wk2", "wv2", "wo2",
                   "wf1", "wf2"):
            setattr(d, p + nm, mk(p + nm, [4, 128, C], BF16))
        for nm in ("bq1", "bk1", "bo1", "bq2", "bo2", "bf1", "bf2"):
            setattr(d, p + nm, mk(p + nm, [128, 4]))
        setattr(d, p + "bv1r", mk(p + "bv1r", [1, C], BF16))
    d.out = nc.dram_tensor("out", [48, M], F32, kind="ExternalOutput")
    return d


def _load_w(d, pools, dram, name):
    """DMA a [4,128,N] weight into sbuf [128, 4, N]."""
    nc = d.nc
    n = dram.shape[2]
    t = pools.wpool.tile([128, 4, n], dram.dtype, name=name, tag="w" + str(n))
    nc.sync.dma_start(out=t, in_=dram.ap().transpose([1, 0, 2]))
    return t


def _load_small(d, pools, dram, name):
    nc = d.nc
    shape = list(dram.shape)
    if len(shape) == 3:
        # [4, 128, N] dram -> [128, 4, N] sbuf (partition-major)
        t = pools.cpool.tile([shape[1], shape[0], shape[2]], dram.dtype,
                             name=name)
        nc.sync.dma_start(out=t, in_=dram.ap().transpose([1, 0, 2]))
    else:
        t = pools.cpool.tile(shape, dram.dtype, name=name)
        nc.sync.dma_start(out=t, in_=dram.ap())
    return t


def _linear_fm_ms(d, pools, w_sb, x_sb, epilogue, ms):
    """Feature-major linear for one token half ms: for each ct produce psum
    [128,512] = sum_kt w_sb[:,kt,ct*128:+128].T @ x_sb[:,kt,ms*512:+512],
    then call epilogue(ct, ms, psum)."""
    nc = d.nc
    for ct in range(CT):
        ps = pools.ps.tile([128, 512], F32, name="lin_ps", tag="ps")
        for kt in range(CT):
            _mm(nc,
                ps,
                w_sb[:, kt, ct * 128:(ct + 1) * 128],
                x_sb[:, kt, ms * 512:(ms + 1) * 512],
                start=(kt == 0), stop=(kt == 3),
            )
        epilogue(ct, ms, ps)


def _proj_act_ms(d, pools, w_sb, b_sb, x_sb, out_sb, ms):
    """Linear + per-c_out bias via ACT Identity, into feature-major out."""
    nc = d.nc

    def ep(ct, ms_, ps):
        nc.scalar.activation(
            out=out_sb[:, ct, ms_ * 512:(ms_ + 1) * 512], in_=ps,
            func=AF.Identity, bias=b_sb[:, ct:ct + 1], scale=1.0)

    _linear_fm_ms(d, pools, w_sb, x_sb, ep, ms)


def _proj_residual_ms(d, pools, w_sb, b_sb, x_sb, h_sb, ms):
    """h += x @ W + b  (one fused DVE op per tile)."""
    nc = d.nc

    def ep(ct, ms_, ps):
        sl = h_sb[:, ct, ms_ * 512:(ms_ + 1) * 512]
        nc.vector.scalar_tensor_tensor(
            out=sl, in0=ps, scalar=b_sb[:, ct:ct + 1], in1=sl,
            op0=ALU.add, op1=ALU.add)

    _linear_fm_ms(d, pools, w_sb, x_sb, ep, ms)


def _layernorm_ms(d, pools, h_sb, z_sb, tag, ms):
    """LN for one token half; kept for pipelined call sites."""
    _layernorm_emit(d, pools, h_sb, z_sb, tag, (ms,))


def _layernorm(d, pools, h_sb, z_sb, tag):
    _layernorm_emit(d, pools, h_sb, z_sb, tag, (0, 1))


def _layernorm_emit(d, pools, h_sb, z_sb, tag, mss):
    """z = (h - mean_c(h)) * rsqrt(var_c(h) + eps), feature-major.

    h stays fp32 (residual precision), so its column-sum matmul runs fp32;
    the squared-sum runs bf16 off the bf16 Square output.  Emission is
    batched stage-by-stage across the token halves so each engine gets both
    halves' work back-to-back; the broadcast PSUM tiles are consumed
    directly by the z DVE ops (one PSUM operand per DVE op)."""
    nc = d.nc
    if not hasattr(d, "_ln_sq"):
        d._ln_sq = {}
        d._ln_rows = {}
    if tag not in d._ln_sq or mss[0] == 0:
        d._ln_sq[tag] = pools.state.tile([128, CT, M], BF16, name="lnsq",
                                         tag="scratch16")
        d._ln_rows[tag] = pools.rows.tile([1, 4, M], BF16,
                                          name="lnrows_" + tag,
                                          tag="lnrows", bufs=1)
    sq = d._ln_sq[tag]
    rows = d._ln_rows[tag]
    MU, A, R, PSI = range(4)
    sl = lambda ms: slice(ms * 512, (ms + 1) * 512)
    mps = {}
    for ms in mss:
        ps = pools.ps.tile([128, 512], F32, name="ln_mps", tag="ps")
        for kt in range(CT):
            _mm(nc, ps[0:1, :], d.ones_col_b, h_sb[:, kt, sl(ms)],
                start=(kt == 0), stop=(kt == 3))
        mps[ms] = ps
    for ms in mss:
        nc.scalar.activation(out=sq[:, :, sl(ms)], in_=h_sb[:, :, sl(ms)],
                             func=AF.Square)
    vps = {}
    for ms in mss:
        nc.vector.tensor_scalar_mul(rows[:, MU, sl(ms)], mps[ms][0:1, :],
                                    1.0 / C)
        ps = pools.ps.tile([128, 512], F32, name="ln_vps", tag="ps")
        for kt in range(CT):
            _mm(nc, ps[0:1, :], d.ones_col_b, sq[:, kt, sl(ms)],
                start=(kt == 0), stop=(kt == 3))
        vps[ms] = ps
    for ms in mss:
        msl = sl(ms)
        nc.vector.tensor_scalar_mul(rows[:, A, msl], vps[ms][0:1, :], 1.0 / C)
    # var = A - mu^2 ; r = 1/sqrt(var + eps) ; psi = -mu * r
    # (kept per-half: z(ms0) must land as early as possible so the next
    # projection's ms0 matmuls can start while the ms1 chain still runs)
    for msl in [sl(ms) for ms in mss]:
        nc.vector.tensor_mul(rows[:, R, msl], rows[:, MU, msl],
                             rows[:, MU, msl])
        nc.vector.tensor_sub(rows[:, A, msl], rows[:, A, msl],
                             rows[:, R, msl])
        nc.scalar.activation(out=rows[:, A, msl], in_=rows[:, A, msl],
                             func=AF.Sqrt, bias=d.eps_row)
        nc.vector.reciprocal(rows[:, R, msl], rows[:, A, msl])
        nc.vector.scalar_tensor_tensor(
            out=rows[:, PSI, msl], in0=rows[:, MU, msl], scalar=-1.0,
            in1=rows[:, R, msl], op0=ALU.mult, op1=ALU.mult)
    # broadcast r and psi across partitions on the idle GPSIMD engine;
    # all-bf16 SBUF operands put the big z DVE ops in the 2x/4x perf modes
    for ms in mss:
        msl = sl(ms)
        rbs = pools.epool.tile([128, 2, 512], BF16, name="lnb", tag="lnb")
        nc.gpsimd.partition_broadcast(rbs[:, 0, :], rows[:, R, msl])
        nc.gpsimd.partition_broadcast(rbs[:, 1, :], rows[:, PSI, msl])
        zv = z_sb[:, :, msl]
        rv = rbs[:, 0, :].unsqueeze(1).broadcast_to([128, CT, 512])
        pv = rbs[:, 1, :].unsqueeze(1).broadcast_to([128, CT, 512])
        nc.vector.tensor_mul(zv, h_sb[:, :, msl], rv)
        nc.vector.tensor_add(zv, zv, pv)


def _v_proj_ms(d, pools, w_sb, brow_sb, x_sb, v_sb, ms):
    """Token-major value projection for token half ms: v_sb[128 tok, mt, C]
    with bias via an extra K=1 matmul against the bias row."""
    nc = d.nc
    for mt in range(4 * ms, 4 * ms + 4):
        ps = pools.ps.tile([128, 512], F32, name="v_ps", tag="ps")
        for kt in range(CT):
            _mm(nc,
                ps, x_sb[:, kt, mt * 128:(mt + 1) * 128],
                w_sb[:, kt, :], start=(kt == 0),
                stop=(kt == 3 and brow_sb is None))
        if brow_sb is not None:
            _mm(nc, ps, d.ones_row_b[:, 0:128], brow_sb,
                             start=False, stop=True)
        nc.scalar.copy(out=v_sb[:, mt, :], in_=ps)


def _attn_s_stage1(d, pools, w, pair):
    """Scores + exp for heads (2*pair, 2*pair+1) of window w."""
    nc = d.nc
    out = {}
    for ho in range(2):
        h = 2 * pair + ho
        hp, ct = 64 * (h % 2), h // 2
        krows = slice(hp, hp + 64)
        sc = pools.ps.tile([128, 2, 256], F32, name="sc", tag="ps")
        for st in range(2):
            _mm(nc,
                sc[:, st, :],
                d.skt_sb[krows, ct, w * 256 + st * 128:
                         w * 256 + st * 128 + 128],
                d.sqt_sb[krows, ct, w * 256:w * 256 + 256],
                start=True, stop=True)
        ex = pools.epool.tile([128, 2, 256], BF16, name="ex", tag="ex")
        nc.scalar.activation(out=ex, in_=sc, func=AF.Exp)
        out[ho] = ex
    return out


def _attn_s_stage2(d, pools, v_sb, attn_sb, w, pair, exs):
    """dn/ob matmuls, reciprocal, broadcast and normalize for one pair."""
    nc = d.nc
    dn = pools.ps.tile([128, 512], F32, name="dn", tag="ps")
    obs = {}
    for ho in range(2):
        h = 2 * pair + ho
        hp = 64 * (h % 2)
        ob = pools.ps.tile([128, 512], F32, name="ob", tag="ps")
        obs[ho] = ob
        for st in range(2):
            _mm(nc, dn[0:1, 256 * ho:256 * ho + 256], d.ones_col_b,
                exs[ho][:, st, :], start=(st == 0), stop=(st == 1),
                tile_position=(0, 0))
            _mm(nc,
                ob[hp:hp + 64, 0:256],
                v_sb[:, 2 * w + st, h * 64:h * 64 + 64],
                exs[ho][:, st, :], start=(st == 0), stop=(st == 1),
                tile_position=(0, hp))
    rrow = pools.rows.tile([1, 512], F32, name="rrow", tag="rr")
    nc.vector.reciprocal(rrow, dn[0:1, :])
    rbs = pools.epool.tile([128, 2, 256], F32, name="rbs", tag="rbs")
    for ho in range(2):
        h = 2 * pair + ho
        hp, ct = 64 * (h % 2), h // 2
        krows = slice(hp, hp + 64)
        nc.gpsimd.partition_broadcast(rbs[:, ho, :],
                                      rrow[:, 256 * ho:256 * ho + 256])
        nc.vector.tensor_mul(
            attn_sb[krows, ct, w * 256:w * 256 + 256],
            obs[ho][hp:hp + 64, 0:256], rbs[hp:hp + 64, ho, :])


def _attn_s(d, pools, v_sb, attn_sb, lag=1):
    """Block-s self attention, software-pipelined: stage1 (scores+exp) of
    unit i is emitted before stage2 of unit i-lag, so the in-order PE queue
    always has ready score matmuls ahead of denominator matmuls that wait
    on the Act exp round-trip."""
    units = [(w, pair) for w in range(4) for pair in range(4)]
    pend = {}
    for i in range(len(units) + lag):
        if i < len(units):
            w, pair = units[i]
            pend[i] = _attn_s_stage1(d, pools, w, pair)
        j = i - lag
        if j >= 0:
            w, pair = units[j]
            _attn_s_stage2(d, pools, v_sb, attn_sb, w, pair, pend.pop(j))


def _attn_g_stage1(d, pools, wp):
    """Scores + exp for one window-pair of block-g self attention.

    PSUM hazard rule (HW-verified): within one bank, concurrent matmuls with
    different row groups but overlapping column strips fault.  So scores are
    banked by head parity (source rows fixed per bank) and o/denominator by
    window parity (rows = 64j fixed per bank)."""
    nc = d.nc
    scs = [pools.ps.tile([128, 4, 64], F32, name=f"sc_g{p}", tag="ps")
           for p in range(2)]
    for h in range(NH):
        p, ct = h % 2, h // 2
        hp = 64 * p
        for j in range(2):
            w = 2 * wp + j
            _mm(nc,
                scs[p][64 * j:64 * j + 64, ct, :],
                d.gkt_sb[hp:hp + 64, ct, w * 64:w * 64 + 64],
                d.gqt_sb[hp:hp + 64, ct, w * 64:w * 64 + 64],
                start=True, stop=True, tile_position=(hp, 64 * j))
    exs = []
    for p in range(2):
        ex = pools.epool.tile([128, 4, 64], BF16, name=f"ex_g{p}",
                              tag="ex")
        nc.scalar.activation(out=ex, in_=scs[p], func=AF.Exp)
        exs.append(ex)
    return exs


def _attn_g_stage2(d, pools, v_sb, attn_sb, wp, exs):
    nc = d.nc
    ogs = [pools.ps.tile([128, 4, 64], F32, name=f"og{j}", tag="ps")
           for j in range(2)]
    dns = [pools.ps.tile([128, 512], F32, name=f"dn_g{j}", tag="ps")
           for j in range(2)]
    for h in range(NH):
        p, ct = h % 2, h // 2
        for j in range(2):
            jr = slice(64 * j, 64 * j + 64)
            _mm(nc,
                dns[j][0:1, h * 64:h * 64 + 64],
                d.ones_col_b[jr, :], exs[p][jr, ct, :],
                start=True, stop=True, tile_position=(64 * j, 0))
            _mm(nc,
                ogs[j][64 * p:64 * p + 64, ct, :],
                v_sb[jr, wp, h * 64:h * 64 + 64],
                exs[p][jr, ct, :],
                start=True, stop=True, tile_position=(64 * j, 64 * p))
    for j in range(2):
        rrow = pools.rows.tile([1, 512], F32, name=f"rr_g{j}", tag="rr")
        nc.vector.reciprocal(rrow, dns[j][0:1, :])
        rbs = pools.epool.tile([128, 512], F32, name=f"rbs_g{j}",
                               tag="rbs")
        nc.gpsimd.partition_broadcast(rbs, rrow)
        for p in range(2):
            pr = slice(64 * p, 64 * p + 64)
            out_view = attn_sb[pr, :, (2 * wp + j) * 64:
                               (2 * wp + j) * 64 + 64]
            in1 = rbs[pr, :].rearrange(
                "q (ct two l) -> q ct two l", two=2, l=64)[:, :, p, :]
            nc.vector.tensor_mul(out_view, ogs[j][pr, :, :], in1)


def _attn_g(d, pools, v_sb, attn_sb, lag=1):
    pend = {}
    for i in range(8 + lag):
        if i < 8:
            pend[i] = _attn_g_stage1(d, pools, i)
        j = i - lag
        if j >= 0:
            _attn_g_stage2(d, pools, v_sb, attn_sb, j, pend.pop(j))


def _ctx_kv(d, pools, prefix, wk, wv):
    """Cross-attention K/V from the 16 action-context tokens.  Independent
    of the token stream, so emitted during the LN1 row chain to stay off
    the critical path.  kT [128, 4, 16] feature-major; V [16, C] token-
    major, replicated to the four 32-partition strips."""
    nc = d.nc
    ktc = pools.state.tile([128, 4, S], BF16, name=prefix + "ktc",
                           tag=prefix + "ktc")
    for ct in range(CT):
        ps = pools.ps.tile([128, 512], F32, name="ktc_ps", tag="ps")
        for kt in range(CT):
            _mm(nc, ps[:, 0:S], wk[:, kt, ct * 128:(ct + 1) * 128],
                             d.ctxt_sb[:, kt, :], start=(kt == 0),
                             stop=(kt == 3))
        nc.scalar.copy(out=ktc[:, ct, :], in_=ps[:, 0:S])
    vc = pools.state.tile([128, 8, 64], BF16, name=prefix + "vc",
                          tag=prefix + "vc")
    ps = pools.ps.tile([128, 512], F32, name="vc_ps", tag="ps")
    for kt in range(CT):
        _mm(nc, ps[0:S, :], d.ctxt_sb[:, kt, :], wv[:, kt, :],
                         start=(kt == 0), stop=(kt == 3))
    vcv = vc.rearrange("p h d -> p (h d)")
    nc.scalar.copy(out=vcv[0:S, :], in_=ps[0:S, :])
    for q in range(1, 4):
        nc.sync.dma_start(out=vcv[32 * q:32 * q + S, :], in_=vcv[0:S, :])
    setattr(d, prefix + "ktc_sb", ktc)
    setattr(d, prefix + "vc_sb", vc)


def _xattn_s1(d, pools, prefix, ms):
    """Cross attention scores + exp for token half ms."""
    nc = d.nc
    qt = getattr(d, prefix + "qt2_sb")
    ktc = getattr(d, prefix + "ktc_sb")
    msl = slice(ms * 512, (ms + 1) * 512)
    exs = []
    for grp in range(2):                       # heads 4*grp .. 4*grp+3
        # no memset: rows outside the four 16-row strips are garbage but
        # are never read (dn/ob matmuls slice sr exactly)
        sc = pools.ps.tile([128, 512], F32, name="sc_x", tag="ps")
        for hh in range(4):
            h = 4 * grp + hh
            hp, ct = 64 * (h % 2), h // 2
            _mm(nc,
                sc[32 * hh:32 * hh + S, :],
                ktc[64 * (h % 2):64 * (h % 2) + 64, ct, :],
                qt[64 * (h % 2):64 * (h % 2) + 64, ct, msl],
                start=True, stop=True,
                tile_position=(hp, 32 * hh))
        ex = pools.epool.tile([128, 512], BF16, name="ex_x", tag="ex")
        nc.scalar.activation(out=ex, in_=sc, func=AF.Exp)
        exs.append(ex)
    setattr(d, prefix + "_xexs%d" % ms, exs)


def _xattn_s2(d, pools, prefix, attn_sb, ms):
    """Cross attention denominators/AV/normalize for token half ms."""
    nc = d.nc
    vc = getattr(d, prefix + "vc_sb")
    exs = getattr(d, prefix + "_xexs%d" % ms)
    msl = slice(ms * 512, (ms + 1) * 512)
    for h in range(NH):
        grp, hh = h // 4, h % 4
        sr = slice(32 * hh, 32 * hh + S)
        hp, ct = 64 * (h % 2), h // 2
        dn = pools.ps.tile([128, 512], F32, name="dn_x", tag="ps")
        _mm(nc, dn[0:1, :], d.ones_col_b[sr, :], exs[grp][sr, :],
                         start=True, stop=True,
                         tile_position=(32 * hh, 0))
        ob = pools.ps.tile([128, 512], F32, name="ob_x", tag="ps")
        _mm(nc, ob[hp:hp + 64, :], vc[sr, h, :],
                         exs[grp][sr, :], start=True, stop=True,
                         tile_position=(32 * hh, hp))
        rrow = pools.rows.tile([1, 512], F32, name="rr_x", tag="rr")
        nc.vector.reciprocal(rrow, dn[0:1, :])
        rbs = pools.epool.tile([128, 512], F32, name="rbs_x", tag="rbs")
        nc.gpsimd.partition_broadcast(rbs, rrow)
        nc.vector.tensor_mul(attn_sb[64 * (h % 2):64 * (h % 2) + 64,
                                     ct, msl],
                             ob[hp:hp + 64, :], rbs[hp:hp + 64, :])


def _block(d, pools, prefix, h_sb):
    """One transformer block, software-pipelined at token-half (ms)
    granularity: the second half of each primitive overlaps the first half
    of its consumer, and the self-attention windows for ms0 run while the
    ms1 projections are still in flight."""
    nc = d.nc
    z = pools.state.tile([128, CT, M], BF16, name="z", tag="zbuf")
    attn = pools.state.tile([128, CT, M], BF16, name="attn", tag="attnbuf")
    qt = pools.state.tile([128, CT, M], BF16, name=prefix + "qt", tag="qbuf")
    kt_ = pools.state.tile([128, CT, M], BF16, name=prefix + "kt", tag="kbuf")
    v_sb = pools.state.tile([128, 8, C], BF16, name=prefix + "v", tag="vbuf")
    setattr(d, prefix + "qt_sb", qt)
    setattr(d, prefix + "kt_sb", kt_)

    wk2 = _load_w(d, pools, getattr(d, prefix + "wk2"), "wk2")
    wv2 = _load_w(d, pools, getattr(d, prefix + "wv2"), "wv2")
    wq1 = _load_w(d, pools, getattr(d, prefix + "wq1"), "wq1")
    wk1 = _load_w(d, pools, getattr(d, prefix + "wk1"), "wk1")
    wv1 = _load_w(d, pools, getattr(d, prefix + "wv1"), "wv1")
    bq1 = getattr(d, prefix + "bq1_sb")
    bk1 = getattr(d, prefix + "bk1_sb")
    bv1r = getattr(d, prefix + "bv1r_sb")

    # ---- LN1 + qkv + self attention, pipelined over ms
    _mark(d, prefix + ":ln1")
    _layernorm(d, pools, h_sb, z, prefix + "1")
    # context K/V: independent of the token stream; emitted here so its
    # matmuls fill the PE gap during the LN row chain and keep the PE
    # p-state warm into the projections
    _pe_warm(d, pools, 24)
    _mark(d, prefix + ":qkproj")
    _proj_act_ms(d, pools, wq1, bq1, z, qt, 0)
    _proj_act_ms(d, pools, wk1, bk1, z, kt_, 0)
    _v_proj_ms(d, pools, wv1, bv1r, z, v_sb, 0)
    _proj_act_ms(d, pools, wq1, bq1, z, qt, 1)
    _proj_act_ms(d, pools, wk1, bk1, z, kt_, 1)
    _v_proj_ms(d, pools, wv1, bv1r, z, v_sb, 1)
    _mark(d, prefix + ":selfattn")
    if prefix == "s":
        _attn_s(d, pools, v_sb, attn)
    else:
        _attn_g(d, pools, v_sb, attn)

    # context K/V: independent of the token stream; emitted here so its
    # matmuls fill PE gaps at the softmax tail (weights DMA'd long before)
    _ctx_kv(d, pools, prefix, wk2, wv2)


    # ---- o-projection (+res), LN2, q2, pipelined
    wo1 = _load_w(d, pools, getattr(d, prefix + "wo1"), "wo1")
    bo1 = getattr(d, prefix + "bo1_sb")
    _mark(d, prefix + ":oproj1")
    _proj_residual_ms(d, pools, wo1, bo1, attn, h_sb, 0)
    _proj_residual_ms(d, pools, wo1, bo1, attn, h_sb, 1)
    _mark(d, prefix + ":ln2")
    _layernorm(d, pools, h_sb, z, prefix + "2")
    wq2 = _load_w(d, pools, getattr(d, prefix + "wq2"), "wq2")
    bq2 = getattr(d, prefix + "bq2_sb")
    qt2 = pools.state.tile([128, CT, M], BF16, name=prefix + "qt2",
                           tag="kbuf")
    setattr(d, prefix + "qt2_sb", qt2)
    _pe_warm(d, pools, 24)
    _mark(d, prefix + ":q2proj")
    _proj_act_ms(d, pools, wq2, bq2, z, qt2, 0)
    _mark(d, prefix + ":xattn")
    _xattn_s1(d, pools, prefix, 0)
    _proj_act_ms(d, pools, wq2, bq2, z, qt2, 1)
    _xattn_s2(d, pools, prefix, attn, 0)
    wo2 = _load_w(d, pools, getattr(d, prefix + "wo2"), "wo2")
    bo2 = getattr(d, prefix + "bo2_sb")
    _xattn_s1(d, pools, prefix, 1)
    _mark(d, prefix + ":oproj2")
    _proj_residual_ms(d, pools, wo2, bo2, attn, h_sb, 0)
    _xattn_s2(d, pools, prefix, attn, 1)
    _proj_residual_ms(d, pools, wo2, bo2, attn, h_sb, 1)

    # ---- mlp, pipelined
    wf1 = _load_w(d, pools, getattr(d, prefix + "wf1"), "wf1")
    wf2 = _load_w(d, pools, getattr(d, prefix + "wf2"), "wf2")
    bf1 = getattr(d, prefix + "bf1_sb")
    bf2 = getattr(d, prefix + "bf2_sb")
    _mark(d, prefix + ":ln3")
    _layernorm(d, pools, h_sb, z, prefix + "3")
    gbuf = pools.state.tile([128, CT, M], BF16, name=prefix + "g",
                            tag="gbuf")

    def ep_gelu(ct, ms, ps):
        nc.scalar.activation(
            out=gbuf[:, ct, ms * 512:(ms + 1) * 512], in_=ps, func=AF.Gelu,
            bias=bf1[:, ct:ct + 1], scale=1.0)

    _pe_warm(d, pools, 24)
    _mark(d, prefix + ":mlp1")
    _linear_fm_ms(d, pools, wf1, z, ep_gelu, 0)
    _linear_fm_ms(d, pools, wf1, z, ep_gelu, 1)
    _mark(d, prefix + ":mlp2")
    _proj_residual_ms(d, pools, wf2, bf2, gbuf, h_sb, 0)
    _proj_residual_ms(d, pools, wf2, bf2, gbuf, h_sb, 1)


class Pools:
    pass


def _body(d, pools):
    """One full forward pass for this core's 1024 tokens."""
    nc = d.nc

    # constants and small tensors first (the weight DMAs are emitted lazily
    # at their use sites so the shared DMA queue can never head-of-line block
    # a load that an earlier weight's consumer depends on)
    xpt = pools.cpool.tile([48, M], BF16, name="xpt_sb")
    nc.sync.dma_start(out=xpt, in_=d.xpt.ap())
    wp = pools.cpool.tile([48, C], BF16, name="wp_sb")
    nc.sync.dma_start(out=wp, in_=d.wp.ap())
    d.ctxt_sb = _load_small(d, pools, d.ctxt, "ctxt")
    pev = _load_small(d, pools, d.pef, "pef")
    for p in ("s", "g"):
        for nm in ("bq1", "bk1", "bo1", "bq2", "bo2", "bf1", "bf2"):
            setattr(d, p + nm + "_sb",
                    _load_small(d, pools, getattr(d, p + nm), p + nm))
        setattr(d, p + "bv1r_sb",
                _load_small(d, pools, getattr(d, p + "bv1r"), p + "bv1r"))
    d.bh_sb = _load_small(d, pools, d.bh, "bh")

    h = pools.state.tile([128, CT, M], BF16, name="h", tag="hbuf")

    # ---- patch embedding + positional embeddings
    _mark(d, "embed")
    for ct in range(CT):
        for ms in range(2):
            msl = slice(ms * 512, (ms + 1) * 512)
            ps = pools.ps.tile([128, 512], F32, name="pe_ps", tag="ps")
            _mm(nc, ps, wp[:, ct * 128:(ct + 1) * 128],
                             xpt[:, msl], start=True, stop=True)
            # + (conv_b + t_pos + sp_pos), folded on the host
            nc.vector.tensor_add(h[:, ct, msl], ps, pev[:, ct, msl])

    # ---- block s (spatial windows = frames)
    _block(d, pools, "s", h)

    # ---- permute tokens to m' (window-major for the 4x4x4 windows)
    _mark(d, "permute")
    hg = pools.state.tile([128, CT, M], BF16, name="hg", tag="zbuf2")
    for ct in range(CT):
        for a in range(4):
            src = bass_view_perm(h, ct, a)
            nc.vector.tensor_copy(
                out=hg[:, ct, a * 256:(a + 1) * 256].rearrange(
                    "p (b f i j) -> p b f i j", b=4, f=4, i=4),
                in_=src)
    _block(d, pools, "g", hg)

    # ---- output head (m' order; host undoes the permutation)
    _mark(d, "head")
    d.wh_sb = _load_w(d, pools, d.wh, "wh")
    outT = pools.cpool.tile([48, M], F32, name="outT")
    for ms in range(2):
        msl = slice(ms * 512, (ms + 1) * 512)
        ps = pools.ps.tile([128, 512], F32, name="hd_ps", tag="ps")
        for kt in range(CT):
            _mm(nc, ps[0:48, :], d.wh_sb[:, kt, :], hg[:, kt, msl],
                             start=(kt == 0), stop=False)
        _mm(nc, ps[0:48, :], d.bh_sb, d.ones_row_b,
                         start=False, stop=True)
        nc.scalar.copy(out=outT[:, msl], in_=ps[0:48, :])
        nc.sync.dma_start(out=d.out.ap()[:, msl], in_=outT[:, msl])


def bass_view_perm(h, ct, a):
    """View of h[:, ct, :] selecting m' block a*256..a*256+255 in (b,f,i,j)
    nested order: m = f*256 + (4a+i)*16 + 4b + j."""
    base = h[:, ct, :]
    return bass_ap_nest(base, a)


def bass_ap_nest(base, a):
    import concourse.bass as bass  # noqa
    v = base.rearrange("p (f ph pw) -> p f ph pw", f=4, ph=16)
    # dims: f(stride 256), ph=4a+i (stride 16), pw=4b+j (stride 1)
    v2 = v[:, :, 4 * a:4 * a + 4, :].rearrange(
        "p f i (b j) -> p b f i j", b=4)
    return v2


def _build_nc(loop_n=1):
    key = loop_n
    if key in _BUILD_CACHE:
        return _BUILD_CACHE[key]
    nc = bacc.Bacc("TRN2", target_bir_lowering=False, debug=False,
                   num_devices=8)
    d = _declare_inputs(nc)
    with tile.TileContext(nc) as tc, nc.allow_low_precision(
            reason="f32r matmul operands; output tolerance is 2e-2"):
        pools = Pools()
        import contextlib
        stack = contextlib.ExitStack()
        with stack:
            pools.ps = stack.enter_context(
                tc.tile_pool(name="ps", bufs=8, space="PSUM"))
            pools.wpool = stack.enter_context(
                tc.tile_pool(name="wpool", bufs=5))
            pools.cpool = stack.enter_context(
                tc.tile_pool(name="cpool", bufs=1))
            pools.state = stack.enter_context(
                tc.tile_pool(name="state", bufs=1))
            pools.epool = stack.enter_context(
                tc.tile_pool(name="epool", bufs=6))
            pools.rows = stack.enter_context(
                tc.tile_pool(name="rows", bufs=4))
            ones_col = pools.cpool.tile([128, 1], F32, name="ones_col")
            nc.vector.memset(ones_col, 1.0)
            ones_row = pools.cpool.tile([1, 512], F32, name="ones_row")
            nc.vector.memset(ones_row, 1.0)
            d.ones_col = ones_col
            d.ones_row = ones_row
            ones_col_b = pools.cpool.tile([128, 1], BF16, name="ones_col_b")
            nc.vector.tensor_copy(out=ones_col_b, in_=ones_col)
            d.ones_col_b = ones_col_b
            ones_row_b = pools.cpool.tile([1, 512], BF16, name="ones_row_b")
            nc.vector.tensor_copy(out=ones_row_b, in_=ones_row)
            d.ones_row_b = ones_row_b
            warm_sb = pools.cpool.tile([128, 512], BF16, name="warm_sb")
            nc.vector.memset(warm_sb, 0.0)
            d.warm_sb = warm_sb
            eps_row = pools.cpool.tile([1, 1], F32, name="eps_row")
            nc.vector.memset(eps_row, EPS)
            d.eps_row = eps_row
            # python-level unroll for timing variants (a tc.For_i back
            # edge wedges the device here; the unrolled NEFF is equivalent)
            for _ in range(loop_n):
                _body(d, pools)
    nc.compile()
    _BUILD_CACHE[key] = nc
    return nc


# ---------------------------------------------------------------- entry point

_MPRIME = None


def kernel(**inputs) -> np.ndarray:
    global _MPRIME
    if _MPRIME is None:
        _MPRIME = _mprime_index()
    nc = _build_nc(1)
    in_maps = host_prep(inputs)
    res = run_bass_kernel_spmd(nc, in_maps, core_ids=list(range(8)))
    # reassemble: per-core outT [48, 1024] in m' order -> full output
    out = np.zeros((B, CIN, F, IMG, IMG), np.float32)
    for c in range(8):
        b, t = c // 4, c % 4
        ot = res.results[c]["out"]                 # [48, M] m'-order
        om = np.empty_like(ot)
        om[:, _MPRIME] = ot                        # -> m order
        # om[(p,q,cc), (f,ph,pw)] -> out[b, cc, 4t+f, ph*4+p, pw*4+q]
        om = om.reshape(P, P, CIN, 4, PD, PD)
        om = om.transpose(2, 3, 4, 0, 5, 1)        # (cc, f, ph, p, pw, q)
        out[b, :, 4 * t:4 * t + 4] = om.reshape(CIN, 4, IMG, IMG)
    return out


if __name__ == "__main__":
    import reference
    ins = reference.setup_inputs()
    ins = {k: np.asarray(v) for k, v in ins.items()}
    exp = np.asarray(reference.reference(**ins))
    act = kernel(**ins)
    err = np.abs(act - exp).max() / (np.abs(exp).max() + 1e-12)
    print("Relative error:", err)

